# revision 1
# baseline (speedup 1.0000x reference)
"""Sliding-window multi-head attention (Longformer-style band attention) for
Trainium2, distributed over 8 NeuronCores.

Sharding: data-parallel over batch (B=2) x tensor-parallel over heads
(16 heads -> 4 groups of 4). Core c handles batch c//4, heads
[4*(c%4), 4*(c%4)+4). Each core computes the QKV projection for its head
group, band attention over 128-key tiles, and a partial output projection;
the host sums the 4 partials per batch and adds bo.

Fast path (all-ones padding mask, zero qkv bias): bf16 datapath end to end.
Scores are computed transposed ([key 128, query 128] tiles, 5 key tiles per
128-query block), exp'd on the scalar engine into bf16 probabilities,
triangular band masks applied on gpsimd, and PV accumulated as [query, 65]
with an appended ones column giving the softmax denominator for free.
Normalization is a per-partition reciprocal+scale on the vector engine; the
normalized context pair is PE-transposed and staged for the output
projection, which writes bf16 partials DMA'd from SBUF. QKV projection work
items are interleaved into the attention loop to keep the tensor engine
saturated, with scratch warm-up matmuls absorbing the PE clock ramp at
startup.

Generic path (padding masks / nonzero qkv bias) falls back to an f32r
implementation of the same blocking.
"""
import sys
import numpy as np
import ml_dtypes

try:
    import concourse.bass as bass
except ImportError:
    sys.path.insert(0, "/opt/trn_rl_repo")
    import concourse.bass as bass
import concourse.mybir as mybir
import concourse.tile as tile
from concourse import bacc
from concourse.bass_utils import run_bass_kernel_spmd

dt = mybir.dt
bf16 = ml_dtypes.bfloat16

B, S, E, H, W = 2, 4096, 1024, 16, 512
HD = E // H          # 64
NH_CORE = 4
w = W // 2           # 256
NT = S // 128        # 32 key tiles of 128
NQT = S // 128       # 32 query tiles of 128
NBC = S // 512       # 8 qkv token chunks of 512
NCC = S // 256       # generic path: 16 query chunks of 256
NEG = -9e15

_cache = {}


def _build_fast(depth=2, b_prol=2, mask_eng='dve', qkcopy_eng='dve',
                d_delay=1, cx_bufs=2, big_bufs=2, tp_delay=1, warm_n=0,
                osb_eng='gpsimd', pd_pool='big', xq_eng='sync', b_slack=8,
                d_hold=0, tp_pool='cx', pv_first=0):
    nc = bacc.Bacc("TRN2", target_bir_lowering=False, debug=False,
                   num_devices=8)

    XT = nc.dram_tensor("xT", [8, 128, 8, 512], dt.bfloat16,
                        kind="ExternalInput")
    WQK = nc.dram_tensor("wqk", [128, 8, 512], dt.bfloat16,
                         kind="ExternalInput")
    WV = nc.dram_tensor("wv", [128, 8, 256], dt.bfloat16,
                        kind="ExternalInput")
    WO = nc.dram_tensor("wo", [2, 128, 1024], dt.bfloat16,
                        kind="ExternalInput")
    OUT = nc.dram_tensor("out", [S, E], dt.bfloat16, kind="ExternalOutput")

    p_i = np.arange(128)[:, None]
    c_i = np.arange(128)[None, :]
    lo = (p_i >= c_i).astype(bf16)   # tile g==qt-2: valid kr >= qr
    up = (p_i <= c_i).astype(bf16)   # tile g==qt+2: valid kr <= qr
    MASKS = nc.inline_tensor(np.ascontiguousarray(
        np.stack([lo, up], axis=1)), name="trimasks")   # [128, 2, 128]
    IDENT = nc.inline_tensor(np.eye(128, dtype=bf16), name="ident")

    with tile.TileContext(nc) as tc:
        with tc.tile_pool(name="const", bufs=1) as cpool, \
             tc.tile_pool(name="qkTp", bufs=1) as qkpool, \
             tc.tile_pool(name="vaugp", bufs=1) as vpool, \
             tc.tile_pool(name="ctxTp", bufs=1) as ctpool, \
             tc.tile_pool(name="xq", bufs=4) as xqpool, \
             tc.tile_pool(name="pt", bufs=7) as ptpool, \
             tc.tile_pool(name="recp", bufs=4) as recpool, \
             tc.tile_pool(name="cnp", bufs=4) as cnpool, \
             tc.tile_pool(name="osbp", bufs=3) as opool, \
             tc.tile_pool(name="stp", bufs=2, space="PSUM") as sapool, \
             tc.tile_pool(name="cxp", bufs=cx_bufs, space="PSUM") as cxpool, \
             tc.tile_pool(name="bigp", bufs=big_bufs, space="PSUM") as bigpool:

            # ---- constants / weights ----
            wqk = cpool.tile([128, 8, 512], dt.bfloat16)
            wv = cpool.tile([128, 8, 256], dt.bfloat16)
            wo = cpool.tile([128, 2, 1024], dt.bfloat16)
            masks = cpool.tile([128, 2, 128], dt.bfloat16)
            ident = cpool.tile([128, 128], dt.bfloat16)
            # k-slice granularity so the first QKV matmuls start early
            # (subtile deps gate each accumulation step on its own slice);
            # scalar-engine HWDGE triggers: cheap and off the SP queue.
            # wqk/xq0 slices interleaved so slice pairs land together.
            xq0 = xqpool.tile([128, 8, 512], dt.bfloat16, tag="xq",
                              name="xq")
            for kh in range(2):
                ks = slice(kh * 4, kh * 4 + 4)
                nc.scalar.dma_start(out=wqk[:, ks, :], in_=WQK[:, ks, :])
                nc.sync.dma_start(out=xq0[:, ks, :], in_=XT[0, :, ks, :])
            nc.scalar.dma_start(out=wv, in_=WV[:, :, :])
            nc.scalar.dma_start(out=masks, in_=MASKS[:, :, :])
            nc.scalar.dma_start(out=ident, in_=IDENT[:, :])
            nc.scalar.dma_start(out=wo[:, 0, :], in_=WO[0, :, :])
            nc.scalar.dma_start(out=wo[:, 1, :], in_=WO[1, :, :])

            # PE warmup: scratch matmuls absorb the p-state ramp while the
            # first input DMAs are still streaming in.
            if warm_n:
                wsrc = cpool.tile([128, 512], dt.bfloat16)
                nc.vector.memset(wsrc, 0.0)
                wdst = bigpool.tile([128, 512], dt.float32, tag="big",
                                    name="wdst")
                for i in range(warm_n):
                    nc.tensor.matmul(wdst, wsrc[:, 0:128], wsrc)

            # ---- persistent intermediates ----
            qkT = [qkpool.tile([128, S], dt.bfloat16, name=f"qkT{cb}")
                   for cb in range(4)]          # 0,1: q head pairs; 2,3: k
            vaug = vpool.tile([128, NT, NH_CORE, 65], dt.bfloat16)
            with nc.allow_low_precision(reason="ones col"):
                nc.vector.memset(vaug[:, :, :, 64], 1.0)
            ctxT = [ctpool.tile([128, S], dt.bfloat16, name=f"ctxT{p}")
                    for p in range(2)]

            # ---------------- phase B: QKV projection ----------------
            def make_xq(s0):
                xq = xqpool.tile([128, 8, 512], dt.bfloat16, tag="xq",
                                 name="xq")
                xeng = nc.scalar if xq_eng == 'act' else nc.sync
                xeng.dma_start(out=xq, in_=XT[s0, :, :, :])
                return xq

            def b_items():
                pre = [xq0, make_xq(1)]
                for s0 in range(NBC):
                    xq = pre[0]
                    pre = pre[1:]
                    if s0 + 2 < NBC:
                        pre.append(make_xq(s0 + 2))  # prefetch 2 ahead

                    # chunk 0: two-pass accumulation so the first matmuls
                    # only need the first half of wqk/xq0 (still streaming)
                    if s0 == 0:
                        pgs = {}

                        def qk_half(cb, kh):
                            if kh == 0:
                                pgs[cb] = bigpool.tile(
                                    [128, 512], dt.float32, tag="big",
                                    name="pg")
                            pg = pgs[cb]
                            for k8 in range(kh * 4, kh * 4 + 4):
                                nc.tensor.matmul(
                                    pg, wqk[:, k8, cb * 128:(cb + 1) * 128],
                                    xq[:, k8, :], start=(k8 == 0),
                                    stop=(k8 == 7))
                            if kh == 1:
                                dst = qkT[cb][:, 0:512]
                                with nc.allow_low_precision(reason="bf16"):
                                    nc.vector.tensor_copy(dst, pg)

                        # pairwise interleave: at most 2 open psum groups
                        # (ring=2), first items need only the first halves
                        for cb0 in (0, 2):
                            yield (lambda cb=cb0: qk_half(cb, 0))
                            yield (lambda cb=cb0 + 1: qk_half(cb, 0))
                            yield (lambda cb=cb0: qk_half(cb, 1))
                            yield (lambda cb=cb0 + 1: qk_half(cb, 1))

                        def v_item0(ts):
                            pv = bigpool.tile([128, 4, 64], dt.float32,
                                              tag="big", name="pv")
                            for k8 in range(8):
                                nc.tensor.matmul(
                                    pv, xq[:, k8, ts * 128:(ts + 1) * 128],
                                    wv[:, k8, :], start=(k8 == 0),
                                    stop=(k8 == 7))
                            with nc.allow_low_precision(reason="bf16"):
                                nc.vector.tensor_copy(
                                    vaug[:, ts, :, 0:64], pv)
                        for ts in range(4):
                            yield (lambda ts=ts: v_item0(ts))
                        continue

                    def qk_item(s0=s0, xq=xq, cb=0):
                        pg = bigpool.tile([128, 512], dt.float32, tag="big",
                                          name="pg")
                        for k8 in range(8):
                            nc.tensor.matmul(
                                pg, wqk[:, k8, cb * 128:(cb + 1) * 128],
                                xq[:, k8, :], start=(k8 == 0),
                                stop=(k8 == 7))
                        dst = qkT[cb][:, s0 * 512:(s0 + 1) * 512]
                        with nc.allow_low_precision(reason="bf16"):
                            if qkcopy_eng == 'act':
                                nc.scalar.copy(dst, pg)
                            elif qkcopy_eng == 'mix':
                                nc.scalar.copy(dst[:, 0:256], pg[:, 0:256])
                                nc.vector.tensor_copy(dst[:, 256:512],
                                                      pg[:, 256:512])
                            elif qkcopy_eng == 'dve2':
                                nc.vector.tensor_copy(dst[:, 0:256],
                                                      pg[:, 0:256])
                                nc.vector.tensor_copy(dst[:, 256:512],
                                                      pg[:, 256:512])
                            else:
                                nc.vector.tensor_copy(dst, pg)
                    for cb in range(4):
                        yield (lambda s0=s0, xq=xq, cb=cb:
                               qk_item(s0, xq, cb))

                    def v_item(s0=s0, xq=xq, ts=0):
                        pv = bigpool.tile([128, 4, 64], dt.float32,
                                          tag="big", name="pv")
                        for k8 in range(8):
                            nc.tensor.matmul(
                                pv, xq[:, k8, ts * 128:(ts + 1) * 128],
                                wv[:, k8, :], start=(k8 == 0),
                                stop=(k8 == 7))
                        st = s0 * 4 + ts
                        with nc.allow_low_precision(reason="bf16"):
                            nc.vector.tensor_copy(
                                vaug[:, st, :, 0:64], pv)
                    for ts in range(4):
                        yield (lambda s0=s0, xq=xq, ts=ts: v_item(s0, xq, ts))

            b_gen = b_items()
            b_total = 12 + (NBC - 1) * 8   # chunk 0 split into 12 items
            b_emitted = 0

            def emit_b(n):
                done = 0
                for _ in range(n):
                    item = next(b_gen, None)
                    if item is None:
                        break
                    item()
                    done += 1
                return done

            # ---------------- phase C: band attention ----------------
            from collections import deque
            pending = deque()
            _dq = deque()
            _held = []

            cur_stp = [None]  # most recent score tile (slice 5 = tp scratch)

            def score_unit(h, qt):
                pr, po = h // 2, (h % 2) * 64
                gs = [g for g in range(qt - 2, qt + 3) if 0 <= g < NT]
                nA = len(gs)
                stp = sapool.tile([128, 6, 128], dt.float32, tag="stp",
                                  name="stp")
                cur_stp[0] = stp
                for j in range(nA):
                    g = gs[j]
                    nc.tensor.matmul(
                        stp[:, j, :],
                        qkT[2 + pr][po:po + 64, g * 128:(g + 1) * 128],
                        qkT[pr][po:po + 64, qt * 128:(qt + 1) * 128])
                ptA = ptpool.tile([128, 5, 128], dt.bfloat16, tag="pt",
                                  name="ptA")
                nc.scalar.activation(ptA[:, 0:nA, :], stp[:, 0:nA, :],
                                     mybir.ActivationFunctionType.Exp,
                                     scale=1.0 / np.sqrt(HD))
                meng = nc.vector if mask_eng == 'dve' else nc.gpsimd
                with nc.allow_low_precision(reason="bf16"):
                    if gs[0] == qt - 2:
                        meng.tensor_mul(ptA[:, 0, :], ptA[:, 0, :],
                                        masks[:, 0, :])
                    if gs[-1] == qt + 2:
                        meng.tensor_mul(ptA[:, nA - 1, :],
                                        ptA[:, nA - 1, :], masks[:, 1, :])
                return (gs, nA, ptA)

            cn_ref = [None, None]  # per parity: pending pair ctxn tile
            _tq = deque()          # deferred ctxT transpose: (h, qt, ctxn2)

            def pv_unit(h, qt, gs, nA, ptA):
                pr = h // 2
                ctx = cxpool.tile([128, 65], dt.float32, tag="cx",
                                  name="ctx")
                n = len(gs)
                # masked slices (0 and n-1) go last: their mask ops on the
                # mask engine get the longest lead time
                order = list(range(1, n - 1)) + [n - 1, 0] if n > 2 \
                    else list(range(n))
                for i, j in enumerate(order):
                    nc.tensor.matmul(ctx, ptA[:, j, :], vaug[:, gs[j], h, :],
                                     start=(i == 0), stop=(i == n - 1))
                rec = recpool.tile([128, 1], dt.float32, tag="rec",
                                   name="rec")
                nc.vector.reciprocal(rec, ctx[:, 64:65])
                if h % 2 == 0:
                    cn_ref[pr] = cnpool.tile([128, 2, 64], dt.bfloat16,
                                             tag="cn", name="ctxn2")
                ctxn2 = cn_ref[pr]
                with nc.allow_low_precision(reason="bf16"):
                    nc.vector.tensor_scalar_mul(ctxn2[:, h % 2, :],
                                                ctx[:, 0:64], rec)
                _tq.append((h, qt, ctxn2))

            def tp_unit(h, qt, ctxn2):
                # PE-transpose a head pair's normalized context in one shot:
                # ctxn2 [128 q, 128 pairdims] -> tp [128 pairdims, 128 q].
                # Scratch = slice 5 of the score tile in flight (never used
                # for scores), viewed as bf16.
                if h % 2 == 1:
                    pr = h // 2
                    if tp_pool == 'big':
                        tp = bigpool.tile([128, 128], dt.bfloat16,
                                          tag="big", name="tp")
                    else:
                        tp = cxpool.tile([128, 128], dt.bfloat16, tag="cx",
                                         name="tp")
                    nc.tensor.transpose(tp, ctxn2, ident)
                    with nc.allow_low_precision(reason="bf16"):
                        nc.vector.tensor_copy(
                            ctxT[pr][:, qt * 128:(qt + 1) * 128], tp)
                if h == NH_CORE - 1:
                    _dq.append(qt)

            def emit_d(qt, split_dma=False):
                osb = opool.tile([128, 1024], dt.bfloat16, tag="osb",
                                 name="osb")
                deng = nc.gpsimd if osb_eng == 'gpsimd' else nc.sync
                for nn in range(2):
                    use_cx = (pd_pool == 'cx' or
                              (pd_pool == 'split' and nn == 0))
                    if use_cx:
                        pD = cxpool.tile([128, 512], dt.float32, tag="cx",
                                         name="pD")
                    else:
                        pD = bigpool.tile([128, 512], dt.float32, tag="big",
                                          name="pD")
                    for p in range(2):
                        nc.tensor.matmul(
                            pD, ctxT[p][:, qt * 128:(qt + 1) * 128],
                            wo[:, p, nn * 512:(nn + 1) * 512],
                            start=(p == 0), stop=(p == 1))
                    with nc.allow_low_precision(reason="bf16 partials"):
                        if nn == 0:
                            nc.scalar.copy(
                                osb[:, nn * 512:(nn + 1) * 512], pD)
                        else:
                            nc.vector.tensor_copy(
                                osb[:, nn * 512:(nn + 1) * 512], pD)
                    if split_dma:
                        deng.dma_start(
                            out=OUT[qt * 128:(qt + 1) * 128,
                                    nn * 512:(nn + 1) * 512],
                            in_=osb[:, nn * 512:(nn + 1) * 512])
                if not split_dma:
                    deng.dma_start(out=OUT[qt * 128:(qt + 1) * 128, :],
                                   in_=osb)

            # pacing: unit qt needs qkT/vaug through token (qt+2)*128+128,
            # i.e. chunks 0..ceil((qt*128+384)/512)-1 done.
            b_emitted += emit_b(8 * b_prol)
            for qt in range(NQT):
                # scores of qt need chunks covering tokens to (qt+3)*128-1,
                # i.e. chunks 0..(qt+2)//4 done; b_slack items of margin.
                need = min(b_total, 12 + 8 * ((qt + 2) // 4) + b_slack)
                # heads in order (0,2,1,3): staggers the two pair-chains
                for h in (0, 2, 1, 3):
                    dd = d_delay if qt < NQT - 2 else 0
                    while _dq and len(_dq) > dd:
                        dqt = _dq.popleft()
                        # park a few mid-sequence o-proj blocks: they become
                        # dependency-free PE work overlapping the final
                        # attention drain
                        if d_hold and len(_held) < d_hold and 16 <= dqt < 28:
                            _held.append(dqt)
                        else:
                            emit_d(dqt, split_dma=(dqt >= NQT - 2))
                    want = need - b_emitted
                    if want > 0:
                        per = max(1, (want + (NH_CORE - h) - 1)
                                  // (NH_CORE - h))
                        b_emitted += emit_b(per)
                    if pv_first == 2 and len(_tq) > tp_delay:
                        tp_unit(*_tq.popleft())
                    if pv_first == 1 and len(pending) >= depth:
                        pv_unit(*pending.popleft())
                        pending.append((h, qt) + score_unit(h, qt))
                    else:
                        pending.append((h, qt) + score_unit(h, qt))
                        if len(pending) > depth:
                            pv_unit(*pending.popleft())
                    if pv_first != 2 and len(_tq) > tp_delay:
                        tp_unit(*_tq.popleft())
            while pending:
                pv_unit(*pending.popleft())
                if len(_tq) > 1:
                    tp_unit(*_tq.popleft())
            while _tq:
                tp_unit(*_tq.popleft())
                while _dq:
                    emit_d(_dq.popleft())
            b_emitted += emit_b(b_total)
            while _dq:
                emit_d(_dq.popleft(), split_dma=True)
            for dqt in _held:
                emit_d(dqt, split_dma=True)

    nc.compile()
    return nc


def _prep_fast(x, Wqkv, Wo):
    """Per-core input maps (bf16)."""
    xT_b = []
    for b in range(B):
        xt = np.ascontiguousarray(x[b].T).astype(bf16)     # [E, S]
        xT_b.append(np.ascontiguousarray(
            xt.reshape(8, 128, 8, 512).transpose(2, 1, 0, 3)))
        # xT_b[b][s0, p, k, t] = xt[k*128+p, s0*512+t]
    in_maps = []
    for c in range(8):
        b, hg = c // 4, c % 4
        heads = range(4 * hg, 4 * hg + 4)
        qcols = np.concatenate([np.arange(h * 192, h * 192 + 64)
                                for h in heads])
        kcols = qcols + 64
        vcols = qcols + 128
        wqk_cols = np.concatenate([qcols, kcols])           # [512]
        wqk = np.ascontiguousarray(
            Wqkv[:, wqk_cols].reshape(8, 128, 512).transpose(1, 0, 2)
        ).astype(bf16)
        wv = np.ascontiguousarray(
            Wqkv[:, vcols].reshape(8, 128, 256).transpose(1, 0, 2)
        ).astype(bf16)
        orows = np.concatenate([np.arange(h * 64, h * 64 + 64)
                                for h in heads])
        wo = np.ascontiguousarray(Wo[orows].reshape(2, 128, 1024)).astype(bf16)
        in_maps.append({"xT": xT_b[b], "wqk": wqk, "wv": wv, "wo": wo})
    return in_maps



def _build_generic(vbias=True, st_bufs=2, po_bufs=1, bc_bufs=1, cx_bufs=2,
           mask_eng='dve', bccopy_eng='act', pt_bufs=8,
           osbcopy_eng='dve', bcast_via='pe', paired=True, depth=1,
           fuse_b=True, pb_bufs=2, b_lead=3, b_prol=2,
           norm_src='sbuf', ctxcopy_eng='act'):
    if fuse_b:
        pt_bufs = min(pt_bufs, 6)
    _nb = 2 if fuse_b else 3
    nc = bacc.Bacc("TRN2", target_bir_lowering=False, debug=False, num_devices=8)

    XT = nc.dram_tensor("xT", [128, 16, 8, 256], dt.float32r, kind="ExternalInput")
    WQK = nc.dram_tensor("wqk", [128, 8, 4, 128], dt.float32r, kind="ExternalInput")
    WV = nc.dram_tensor("wv", [128, 8, 256], dt.float32r, kind="ExternalInput")
    WO = nc.dram_tensor("wo", [2, 128, 1024], dt.float32r, kind="ExternalInput")
    BQK = nc.dram_tensor("bqk", [128, 4], dt.float32, kind="ExternalInput")
    BV = nc.dram_tensor("bv", [1, 256], dt.float32, kind="ExternalInput")
    MV8 = nc.dram_tensor("mv8", [128, 32], dt.float32, kind="ExternalInput")
    OUT = nc.dram_tensor("out", [S, E], dt.float32, kind="ExternalOutput")

    # constant 0/1 triangular band masks for u in {-2,-1,2,3}
    p_i = np.arange(128)[:, None]
    r_i = np.arange(256)[None, :]
    mask_np = {}
    for u in (-2, -1, 2, 3):
        mask_np[u] = ((u * 128 + p_i - r_i >= -w) & (u * 128 + p_i - r_i <= w)
                      ).astype(np.float32)
    MASKS = nc.inline_tensor(
        np.ascontiguousarray(
            np.stack([mask_np[u] for u in (-2, -1, 2, 3)]).transpose(1, 0, 2)),
        name="trimasks")
    ONES = nc.inline_tensor(np.ones((1, 128), dtype=np.float32), name="onesrow")

    with tile.TileContext(nc) as tc:
        with tc.tile_pool(name="const", bufs=1) as cpool, \
             tc.tile_pool(name="qkT", bufs=1) as qkpool, \
             tc.tile_pool(name="vaug", bufs=1) as vpool, \
             tc.tile_pool(name="ctxT", bufs=1) as ctxpool:

            wo = [cpool.tile([128, 1024], dt.float32r, name=f"wo{p}") for p in range(2)]
            bqk = cpool.tile([128, 4], dt.float32)
            nc.gpsimd.dma_start(out=bqk, in_=BQK[:, :])
            bv_f = cpool.tile([1, 256], dt.float32)
            nc.gpsimd.dma_start(out=bv_f, in_=BV[:, :])
            mv8 = cpool.tile([128, 32], dt.float32)
            nc.gpsimd.dma_start(out=mv8, in_=MV8[:, :])
            masks = cpool.tile([128, 4, 256], dt.float32)
            mask_idx = {-2: 0, -1: 1, 2: 2, 3: 3}
            ones_f = cpool.tile([1, 128], dt.float32)
            nc.gpsimd.dma_start(out=ones_f, in_=ONES[:, :])
            ones_r = cpool.tile([1, 128], dt.float32r)
            bv_r = cpool.tile([1, 256], dt.float32r)
            with nc.allow_low_precision(reason="f32r matmul pipeline"):
                nc.vector.tensor_copy(ones_r, ones_f)
                nc.vector.tensor_copy(bv_r, bv_f)

            # persistent intermediates
            qkT = [qkpool.tile([128, S], dt.float32r, name=f"qkT{cb}")
                   for cb in range(4)]  # 0,1: q pairs; 2,3: k pairs
            vaug = [vpool.tile([128, NT, 65], dt.float32r, name=f"vaug{h}")
                    for h in range(NH_CORE)]
            ones32 = cpool.tile([128, NT], dt.float32)
            nc.vector.memset(ones32, 1.0)
            for h in range(NH_CORE):
                with nc.allow_low_precision(reason="f32r"):
                    nc.vector.tensor_copy(vaug[h][:, :, 64], ones32)
            ctxT = [ctxpool.tile([128, S], dt.float32r, name=f"ctxT{p}")
                    for p in range(2)]

            # ---------------- Phase B: QKV projection ----------------
            # Emitted either up front (fuse_b=False) or as fine-grained work
            # items interleaved into the attention loop's idle PE slots.
            bwpool = ctx_pools = None
            import contextlib
            _bstack = contextlib.ExitStack()
            bwpool = _bstack.enter_context(tc.tile_pool(name="bw", bufs=1))
            xqpool = _bstack.enter_context(
                tc.tile_pool(name="xq", bufs=(2 if fuse_b else 3)))
            pbpool = _bstack.enter_context(
                tc.tile_pool(name="pb", bufs=(pb_bufs if fuse_b else 8),
                             space="PSUM"))
            wqk = bwpool.tile([128, 8, 4, 128], dt.float32r)
            wv = bwpool.tile([128, 8, 256], dt.float32r)
            xq0 = [xqpool.tile([128, 4, 256], dt.float32r, tag=f"xq{i}",
                               name="xq") for i in range(2)]
            for i in range(2):
                nc.sync.dma_start(out=xq0[i], in_=XT[:, 0, i * 4:(i + 1) * 4, :])
            for kt in range(8):
                nc.sync.dma_start(out=wqk[:, kt, :, :], in_=WQK[:, kt, :, :])
            nc.sync.dma_start(out=wv[:, 0:4, :], in_=WV[:, 0:4, :])
            nc.sync.dma_start(out=wv[:, 4:8, :], in_=WV[:, 4:8, :])

            def b_items():
                for s0 in range(16):  # 256-token chunks of S
                    if s0 == 0:
                        xq = xq0
                    else:
                        xq = [xqpool.tile([128, 4, 256], dt.float32r,
                                          tag=f"xq{i}", name="xq")
                              for i in range(2)]
                        for i in range(2):
                            nc.sync.dma_start(
                                out=xq[i], in_=XT[:, s0, i * 4:(i + 1) * 4, :])

                    def qk_item(s0=s0, xq=xq, cb=0):
                        pg = pbpool.tile([128, 256], dt.float32, tag="pb",
                                         name="pqk")
                        for k8 in range(8):
                            nc.tensor.matmul(pg, wqk[:, k8, cb, :],
                                             xq[k8 // 4][:, k8 % 4, :],
                                             start=(k8 == 0), stop=(k8 == 7))
                        nc.scalar.activation(
                            qkT[cb][:, s0 * 256:(s0 + 1) * 256], pg,
                            mybir.ActivationFunctionType.Identity,
                            bias=bqk[:, cb:cb + 1])
                    for cb in range(4):
                        yield (lambda s0=s0, xq=xq, cb=cb: qk_item(s0, xq, cb))

                    def v_item(s0=s0, xq=xq, hf=0):
                        pv = pbpool.tile([128, 256], dt.float32, tag="pb",
                                         name="pv")
                        for k8 in range(8):
                            nc.tensor.matmul(
                                pv,
                                xq[k8 // 4][:, k8 % 4, hf * 128:(hf + 1) * 128],
                                wv[:, k8, :], start=(k8 == 0),
                                stop=(k8 == 7 and not vbias))
                        if vbias:
                            nc.tensor.matmul(pv, ones_r, bv_r,
                                             start=False, stop=True)
                        st = s0 * 2 + hf
                        for h in range(NH_CORE):
                            with nc.allow_low_precision(reason="f32r"):
                                nc.vector.tensor_copy(
                                    vaug[h][:, st, 0:64],
                                    pv[:, h * 64:(h + 1) * 64])
                    for hf in range(2):
                        yield (lambda s0=s0, xq=xq, hf=hf: v_item(s0, xq, hf))

            b_gen = b_items()
            b_total = 16 * 6
            b_emitted = 0

            def emit_b(n):
                emitted = 0
                for _ in range(n):
                    item = next(b_gen, None)
                    if item is None:
                        break
                    item()
                    emitted += 1
                return emitted

            if not fuse_b:
                b_emitted += emit_b(b_total)
                _bstack.close()

            nc.gpsimd.dma_start(out=masks, in_=MASKS[:, :, :])
            for p in range(2):
                nc.gpsimd.dma_start(out=wo[p], in_=WO[p, :, :])
            # ------- Phase C: band attention, with output projection folded in -------
            import contextlib
            _cstack = contextlib.ExitStack()
            with _cstack:
                stpool = _cstack.enter_context(
                    tc.tile_pool(name="stp", bufs=st_bufs, space="PSUM"))
                cxpool = _cstack.enter_context(
                    tc.tile_pool(name="ctxp", bufs=cx_bufs, space="PSUM"))
                if bcast_via == 'pe':
                    bcpool = _cstack.enter_context(
                        tc.tile_pool(name="bcp", bufs=bc_bufs, space="PSUM"))
                else:
                    drpool = _cstack.enter_context(
                        tc.tile_pool(name="dr", bufs=4, space="DRAM"))
                popool = _cstack.enter_context(
                    tc.tile_pool(name="po", bufs=po_bufs, space="PSUM"))
                ptpool = _cstack.enter_context(
                    tc.tile_pool(name="pt", bufs=pt_bufs))
                bcsb = _cstack.enter_context(tc.tile_pool(name="bcs", bufs=_nb))
                opool = _cstack.enter_context(tc.tile_pool(name="osb", bufs=2))
                rcpool = _cstack.enter_context(tc.tile_pool(name="rcp", bufs=_nb))

                def score_stage(h, cc):
                    # returns list of (gts, pt, jslices) where pt holds exp'd
                    # probabilities for the key tiles in gts
                    pr, po = h // 2, (h % 2) * 64
                    out = []
                    if paired:
                        # all-ones padding: exp has no per-key bias, so key
                        # tiles are processed in aligned pairs (one psum bank,
                        # one exp, one mask-mul per pair)
                        for ub in (-2, 0, 2):
                            gts = [2 * cc + ub, 2 * cc + ub + 1]
                            if gts[0] < 0 or gts[1] >= NT:
                                continue
                            stp = stpool.tile([128, 2, 256], dt.float32,
                                              tag="st", name="stp")
                            for j, gt in enumerate(gts):
                                nc.tensor.matmul(
                                    stp[:, j, :],
                                    qkT[2 + pr][po:po + 64,
                                                gt * 128:(gt + 1) * 128],
                                    qkT[pr][po:po + 64,
                                            cc * 256:(cc + 1) * 256])
                            pt = ptpool.tile([128, 2, 256], dt.float32r,
                                             tag="pt", name="pt")
                            nc.scalar.activation(
                                pt, stp, mybir.ActivationFunctionType.Exp,
                                scale=1.0 / np.sqrt(HD))
                            if ub != 0:
                                mi = 0 if ub == -2 else 2
                                with nc.allow_low_precision(reason="f32r"):
                                    eng = (nc.gpsimd if mask_eng == 'gpsimd'
                                           else nc.vector)
                                    eng.tensor_mul(pt, pt,
                                                   masks[:, mi:mi + 2, :])
                            out.append((gts, pt))
                        return out
                    for u in range(-2, 4):
                        gt = 2 * cc + u
                        if not 0 <= gt < NT:
                            continue
                        stp = stpool.tile([128, 256], dt.float32, tag="st",
                                          name="stp")
                        nc.tensor.matmul(
                            stp,
                            qkT[2 + pr][po:po + 64, gt * 128:(gt + 1) * 128],
                            qkT[pr][po:po + 64, cc * 256:(cc + 1) * 256])
                        pt = ptpool.tile([128, 256], dt.float32r, tag="pt",
                                         name="pt")
                        nc.scalar.activation(pt, stp,
                                             mybir.ActivationFunctionType.Exp,
                                             bias=mv8[:, gt:gt + 1],
                                             scale=1.0 / np.sqrt(HD))
                        if u in mask_idx:
                            with nc.allow_low_precision(reason="f32r"):
                                eng = (nc.gpsimd if mask_eng == 'gpsimd'
                                       else nc.vector)
                                eng.tensor_mul(pt, pt,
                                               masks[:, mask_idx[u], :])
                        out.append(([gt], pt))
                    return out

                def pv_stage(h, cc, pts):
                    if _dq:
                        emit_d(_dq.popleft())
                    pr, po = h // 2, (h % 2) * 64
                    ctx = cxpool.tile([65, 256], dt.float32, tag="cx",
                                      name="ctx")
                    nmm = sum(len(gts) for gts, _ in pts)
                    j = 0
                    for gts, pt in pts:
                        for jj, gt in enumerate(gts):
                            rhs = pt[:, jj, :] if len(gts) > 1 else pt
                            nc.tensor.matmul(ctx, vaug[h][:, gt, :], rhs,
                                             start=(j == 0),
                                             stop=(j == nmm - 1))
                            j += 1
                    if norm_src == 'sbuf':
                        # copy ctx out of PSUM first: frees the cx slot early
                        # and the final multiply reads bc straight from PSUM
                        cxs = bcsb.tile([65, 256], dt.float32, tag="bcs",
                                        name="cxs")
                        if ctxcopy_eng == 'act':
                            nc.scalar.copy(cxs, ctx)
                        else:
                            nc.vector.tensor_copy(cxs, ctx)
                        ctx = cxs
                    rec = rcpool.tile([1, 256], dt.float32r, tag="rc",
                                      name="rec")
                    with nc.allow_low_precision(reason="f32r"):
                        nc.vector.reciprocal(rec, ctx[64:65, :])
                    bcs = None
                    if norm_src != 'sbuf':
                        bcs = bcsb.tile([64, 256], dt.float32, tag="bcs",
                                        name="bcs")
                    if bcast_via == 'dma':
                        drec = drpool.tile([1, 256], dt.float32r, tag="dr",
                                           name="drec")
                        nc.sync.dma_start(out=drec, in_=rec)
                        dbc = bass.AP(tensor=drec.tensor, offset=drec.offset,
                                      ap=[[0, 64]] + drec.ap[1:])
                        nc.sync.dma_start(out=bcs.bitcast(dt.float32r), in_=dbc)
                    else:
                        bc = bcpool.tile([64, 256], dt.float32, tag="bc",
                                         name="bc")
                        nc.tensor.matmul(bc, ones_r[:, 0:64], rec)
                        if norm_src == 'sbuf':
                            bcs = bc
                        elif bccopy_eng == 'act':
                            nc.scalar.copy(bcs, bc)
                        else:
                            nc.vector.tensor_copy(bcs, bc)
                    with nc.allow_low_precision(reason="f32r"):
                        nc.vector.tensor_mul(
                            ctxT[pr][po:po + 64, cc * 256:(cc + 1) * 256],
                            ctx[0:64, :], bcs)
                    if h == NH_CORE - 1:
                        _dq.append(2 * cc)
                        _dq.append(2 * cc + 1)

                def emit_d(qt):
                    osb = opool.tile([128, 1024], dt.float32, tag="osb",
                                     name="osb")
                    for nn in range(2):
                        pD = popool.tile([128, 512], dt.float32, tag="po",
                                         name="pD")
                        for p in range(2):
                            nc.tensor.matmul(pD,
                                             ctxT[p][:, qt * 128:(qt + 1) * 128],
                                             wo[p][:, nn * 512:(nn + 1) * 512],
                                             start=(p == 0), stop=(p == 1))
                        if osbcopy_eng == 'act':
                            nc.scalar.copy(osb[:, nn * 512:(nn + 1) * 512], pD)
                        else:
                            nc.vector.tensor_copy(osb[:, nn * 512:(nn + 1) * 512], pD)
                    nc.gpsimd.dma_start(out=OUT[qt * 128:(qt + 1) * 128, :],
                                        in_=osb)

                from collections import deque
                pending = deque()
                _dq = deque()
                if fuse_b:
                    # prologue: cover key tiles for the first two query chunks
                    b_emitted += emit_b(6 * b_prol)
                step = 0
                for cc in range(NCC):
                    for h in range(NH_CORE):
                        if fuse_b:
                            # pace remaining B so chunk cc+2 is done before
                            # attention chunk cc+1 starts
                            target = min(b_total, 6 * (cc + b_lead))
                            want = target - b_emitted
                            per = max(1, (want + (NH_CORE - h) - 1)
                                      // (NH_CORE - h))
                            if want > 0:
                                b_emitted += emit_b(per)
                        pts = score_stage(h, cc)
                        pending.append((h, cc, pts))
                        if len(pending) > depth:
                            pv_stage(*pending.popleft())
                        step += 1
                while pending:
                    pv_stage(*pending.popleft())
                while _dq:
                    emit_d(_dq.popleft())
                if fuse_b:
                    b_emitted += emit_b(b_total)

            _bstack.close()

    nc.compile()
    return nc



def _prep_generic(x, Wqkv, bqkv, Wo, pm):
    in_maps = []
    xT_b = []
    for b in range(B):
        xt = np.ascontiguousarray(x[b].T)                      # [E, S]
        xT_b.append(np.ascontiguousarray(
            xt.reshape(8, 128, 16, 256).transpose(1, 2, 0, 3)))
    mv8_b = []
    for b in range(B):
        # mv8[p, t] = (0 if valid else NEG)/8 for key index t*128+p
        mv = np.where(pm[b], 0.0, NEG).astype(np.float32) / 8.0
        mv8_b.append(np.ascontiguousarray(mv.reshape(32, 128).T))

    for c in range(8):
        b, hg = c // 4, c % 4
        heads = range(4 * hg, 4 * hg + 4)
        qcols = np.concatenate([np.arange(h * 192, h * 192 + 64) for h in heads])
        kcols = qcols + 64
        vcols = qcols + 128
        wqk_cols = np.concatenate([qcols, kcols])               # [512]
        wqk = np.ascontiguousarray(
            Wqkv[:, wqk_cols].reshape(8, 128, 4, 128).transpose(1, 0, 2, 3))
        wv = np.ascontiguousarray(
            Wqkv[:, vcols].reshape(8, 128, 256).transpose(1, 0, 2))
        orows = np.concatenate([np.arange(h * 64, h * 64 + 64) for h in heads])
        wo = np.ascontiguousarray(Wo[orows].reshape(2, 128, 1024))
        in_maps.append({
            "xT": xT_b[b],
            "wqk": wqk,
            "wv": wv,
            "wo": wo,
            "bqk": np.ascontiguousarray(bqkv[wqk_cols].reshape(4, 128).T),
            "bv": np.ascontiguousarray(bqkv[vcols].reshape(1, 256)),
            "mv8": mv8_b[b],
        })
    return in_maps


def kernel(x, Wqkv, bqkv, Wo, bo, padding_mask, num_heads, window_size):
    assert int(num_heads) == H and int(window_size) == W
    x = np.asarray(x, dtype=np.float32)
    Wqkv = np.asarray(Wqkv, dtype=np.float32)
    bqkv = np.asarray(bqkv, dtype=np.float32)
    Wo = np.asarray(Wo, dtype=np.float32)
    bo = np.asarray(bo, dtype=np.float32)
    pm = np.asarray(padding_mask).astype(bool)
    assert x.shape == (B, S, E)

    fast = bool(pm.all()) and not np.any(bqkv)
    if fast:
        if "fast" not in _cache:
            _cache["fast"] = _build_fast(depth=3, tp_delay=1, d_delay=2,
                                         warm_n=8, osb_eng='sync',
                                         pd_pool='split', xq_eng='act',
                                         mask_eng='gpsimd', b_slack=2,
                                         b_prol=1, pv_first=2)
        nc = _cache["fast"]
        in_maps = _prep_fast(x, Wqkv, Wo)
    else:
        vbias = bool(np.any(bqkv.reshape(H, 3, HD)[:, 2, :] != 0.0))
        key = ("nc", vbias, False)
        if key not in _cache:
            _cache[key] = _build_generic(vbias=vbias, paired=False)
        nc = _cache[key]
        in_maps = _prep_generic(x, Wqkv, bqkv, Wo, pm)

    res = run_bass_kernel_spmd(nc, in_maps, list(range(8)))
    kernel._last_results = res

    out = np.empty((B, S, E), dtype=np.float32)
    for b in range(B):
        acc = res.results[4 * b]["out"].astype(np.float32)
        for g in range(1, 4):
            acc = acc + res.results[4 * b + g]["out"].astype(np.float32)
        out[b] = acc + bo
    return out



# revision 14
# speedup vs baseline: 1.0909x; 1.0909x over previous
"""Sliding-window multi-head attention (Longformer-style band attention) for
Trainium2, distributed over 8 NeuronCores.

Sharding: data-parallel over batch (B=2) x tensor-parallel over heads
(16 heads -> 4 groups of 4). Core c handles batch c//4, heads
[4*(c%4), 4*(c%4)+4). Each core computes the QKV projection for its head
group, band attention over 128-key tiles, and a partial output projection;
the host sums the 4 partials per batch and adds bo.

Fast path (all-ones padding mask, zero qkv bias): bf16 datapath end to end.
Scores are computed transposed ([key 128, query 128] tiles, 5 key tiles per
128-query block), exp'd on the scalar engine into bf16 probabilities,
triangular band masks applied on gpsimd, and PV accumulated as [query, 65]
with an appended ones column giving the softmax denominator for free.
Normalization is a per-partition reciprocal+scale on the vector engine; the
normalized context pair is PE-transposed and staged for the output
projection, which writes bf16 partials DMA'd from SBUF. QKV projection work
items are interleaved into the attention loop to keep the tensor engine
saturated, with scratch warm-up matmuls absorbing the PE clock ramp at
startup.

Generic path (padding masks / nonzero qkv bias) falls back to an f32r
implementation of the same blocking.
"""
import sys
import numpy as np
import ml_dtypes

try:
    import concourse.bass as bass
except ImportError:
    sys.path.insert(0, "/opt/trn_rl_repo")
    import concourse.bass as bass
import concourse.mybir as mybir
import concourse.tile as tile
from concourse import bacc
from concourse.bass_utils import run_bass_kernel_spmd

dt = mybir.dt
bf16 = ml_dtypes.bfloat16

B, S, E, H, W = 2, 4096, 1024, 16, 512
HD = E // H          # 64
NH_CORE = 4
w = W // 2           # 256
NT = S // 128        # 32 key tiles of 128
NQT = S // 128       # 32 query tiles of 128
NBC = S // 512       # 8 qkv token chunks of 512
NCC = S // 256       # generic path: 16 query chunks of 256
NEG = -9e15

_cache = {}


def _build_fast(depth=2, b_prol=2, mask_eng='dve', qkcopy_eng='dve',
                d_delay=1, cx_bufs=2, big_bufs=2, tp_delay=1, warm_n=0,
                osb_eng='gpsimd', pd_pool='big', xq_eng='sync', b_slack=8,
                d_hold=0, tp_pool='cx', pv_first=0):
    nc = bacc.Bacc("TRN2", target_bir_lowering=False, debug=False,
                   num_devices=8)

    # fp8 DoubleRow with host-side error compensation: x = x8 + dx8 and
    # W = W8 + dW8 (each fp8e4); three product chains x8W8 + x8dW8 + dx8W8
    # restore bf16-grade accuracy at 0.75x the bf16 PE cost (DoubleRow
    # contracts 256 rows per instruction at 0.5 cycles/row).
    # Layouts: [partition p, t (256-row ktile), s (main/residual), i
    # (DoubleRow pair), cols] with contraction index c = 256t + 128i + p.
    XT = nc.dram_tensor("xT", [8, 128, 4, 2, 2, 512], dt.float8e4,
                        kind="ExternalInput")
    WQK = nc.dram_tensor("wqk", [128, 4, 2, 2, 512], dt.float8e4,
                         kind="ExternalInput")
    WV = nc.dram_tensor("wv", [128, 4, 2, 2, 256], dt.float8e4,
                        kind="ExternalInput")
    WO = nc.dram_tensor("wo", [2, 128, 1024], dt.bfloat16,
                        kind="ExternalInput")
    OUT = nc.dram_tensor("out", [S, E], dt.bfloat16, kind="ExternalOutput")

    p_i = np.arange(128)[:, None]
    c_i = np.arange(128)[None, :]
    lo = (p_i >= c_i).astype(bf16)   # tile g==qt-2: valid kr >= qr
    up = (p_i <= c_i).astype(bf16)   # tile g==qt+2: valid kr <= qr
    MASKS = nc.inline_tensor(np.ascontiguousarray(
        np.stack([lo, up], axis=1)), name="trimasks")   # [128, 2, 128]
    IDENT = nc.inline_tensor(np.eye(128, dtype=bf16), name="ident")

    with tile.TileContext(nc) as tc:
        with tc.tile_pool(name="const", bufs=1) as cpool, \
             tc.tile_pool(name="qkTp", bufs=1) as qkpool, \
             tc.tile_pool(name="vaugp", bufs=1) as vpool, \
             tc.tile_pool(name="ctxTp", bufs=1) as ctpool, \
             tc.tile_pool(name="xq", bufs=4) as xqpool, \
             tc.tile_pool(name="pt", bufs=7) as ptpool, \
             tc.tile_pool(name="recp", bufs=4) as recpool, \
             tc.tile_pool(name="cnp", bufs=4) as cnpool, \
             tc.tile_pool(name="osbp", bufs=3) as opool, \
             tc.tile_pool(name="stp", bufs=2, space="PSUM") as sapool, \
             tc.tile_pool(name="cxp", bufs=cx_bufs, space="PSUM") as cxpool, \
             tc.tile_pool(name="bigp", bufs=big_bufs, space="PSUM") as bigpool:

            # ---- constants / weights ----
            wqk = cpool.tile([128, 4, 2, 2, 512], dt.float8e4)
            wv = cpool.tile([128, 4, 2, 2, 256], dt.float8e4)
            wo = cpool.tile([128, 2, 1024], dt.bfloat16)
            masks = cpool.tile([128, 2, 128], dt.bfloat16)
            ident = cpool.tile([128, 128], dt.bfloat16)
            # t-slice granularity so the first QKV matmuls start early
            # (subtile deps gate each accumulation step on its own slice);
            # scalar-engine HWDGE triggers: cheap and off the SP queue.
            # wqk/xq0 slices interleaved so slice pairs land together.
            xq0 = xqpool.tile([128, 4, 2, 2, 512], dt.float8e4, tag="xq",
                              name="xq")
            for kh in range(2):
                ks = slice(kh * 2, kh * 2 + 2)
                nc.scalar.dma_start(out=wqk[:, ks], in_=WQK[:, ks])
                nc.sync.dma_start(out=xq0[:, ks], in_=XT[0, :, ks])
            nc.scalar.dma_start(out=wv, in_=WV[:, :, :, :, :])
            nc.scalar.dma_start(out=masks, in_=MASKS[:, :, :])
            nc.scalar.dma_start(out=ident, in_=IDENT[:, :])
            nc.scalar.dma_start(out=wo[:, 0, :], in_=WO[0, :, :])
            nc.scalar.dma_start(out=wo[:, 1, :], in_=WO[1, :, :])

            # PE warmup: scratch matmuls absorb the p-state ramp while the
            # first input DMAs are still streaming in.
            if warm_n:
                wsrc = cpool.tile([128, 512], dt.bfloat16)
                nc.vector.memset(wsrc, 0.0)
                wdst = bigpool.tile([128, 512], dt.float32, tag="big",
                                    name="wdst")
                for i in range(warm_n):
                    nc.tensor.matmul(wdst, wsrc[:, 0:128], wsrc)

            # ---- persistent intermediates ----
            qkT = [qkpool.tile([128, S], dt.bfloat16, name=f"qkT{cb}")
                   for cb in range(4)]          # 0,1: q head pairs; 2,3: k
            vaug = vpool.tile([128, NT, NH_CORE, 65], dt.bfloat16)
            with nc.allow_low_precision(reason="ones col"):
                nc.vector.memset(vaug[:, :, :, 64], 1.0)
            ctxT = [ctpool.tile([128, S], dt.bfloat16, name=f"ctxT{p}")
                    for p in range(2)]

            # ---------------- phase B: QKV projection ----------------
            # (sw, sx) product chains: x8·W8 + dx8·W8 + x8·dW8
            CHAINS = ((0, 0), (1, 0), (0, 1))
            DR = mybir.MatmulPerfMode.DoubleRow

            def make_xq(s0):
                xq = xqpool.tile([128, 4, 2, 2, 512], dt.float8e4, tag="xq",
                                 name="xq")
                xeng = nc.scalar if xq_eng == 'act' else nc.sync
                xeng.dma_start(out=xq, in_=XT[s0])
                return xq

            def b_items():
                pre = [xq0, make_xq(1)]
                for s0 in range(NBC):
                    xq = pre[0]
                    pre = pre[1:]
                    if s0 + 2 < NBC:
                        pre.append(make_xq(s0 + 2))  # prefetch 2 ahead

                    # chunk 0: two-pass accumulation so the first matmuls
                    # only need the first half of wqk/xq0 (still streaming)
                    if s0 == 0:
                        pgs = {}

                        def qk_half(cb, kh):
                            if kh == 0:
                                pgs[cb] = bigpool.tile(
                                    [128, 512], dt.float32, tag="big",
                                    name="pg")
                            pg = pgs[cb]
                            mm = kh * 6
                            for t in (kh * 2, kh * 2 + 1):
                                for sw, sx in CHAINS:
                                    nc.tensor.matmul(
                                        pg,
                                        wqk[:, t, sw, :,
                                            cb * 128:(cb + 1) * 128],
                                        xq[:, t, sx, :, :],
                                        start=(mm == 0), stop=(mm == 11),
                                        perf_mode=DR)
                                    mm += 1
                            if kh == 1:
                                dst = qkT[cb][:, 0:512]
                                with nc.allow_low_precision(reason="bf16"):
                                    nc.vector.tensor_scalar_mul(
                                        dst, pg, 1.0 / WSCALE)

                        # pairwise interleave: at most 2 open psum groups
                        # (ring=2), first items need only the first halves
                        for cb0 in (0, 2):
                            yield (lambda cb=cb0: qk_half(cb, 0))
                            yield (lambda cb=cb0 + 1: qk_half(cb, 0))
                            yield (lambda cb=cb0: qk_half(cb, 1))
                            yield (lambda cb=cb0 + 1: qk_half(cb, 1))

                        def v_item0(ts):
                            pv = bigpool.tile([128, 4, 64], dt.float32,
                                              tag="big", name="pv")
                            mm = 0
                            for t in range(4):
                                for sx, sw in CHAINS:
                                    nc.tensor.matmul(
                                        pv,
                                        xq[:, t, sx, :,
                                           ts * 128:(ts + 1) * 128],
                                        wv[:, t, sw, :, :],
                                        start=(mm == 0), stop=(mm == 11),
                                        perf_mode=DR)
                                    mm += 1
                            with nc.allow_low_precision(reason="bf16"):
                                nc.vector.tensor_scalar_mul(
                                    vaug[:, ts, :, 0:64], pv, 1.0 / WSCALE)
                        for ts in range(4):
                            yield (lambda ts=ts: v_item0(ts))
                        continue

                    def qk_item(s0=s0, xq=xq, cb=0):
                        pg = bigpool.tile([128, 512], dt.float32, tag="big",
                                          name="pg")
                        mm = 0
                        for t in range(4):
                            for sw, sx in CHAINS:
                                nc.tensor.matmul(
                                    pg,
                                    wqk[:, t, sw, :, cb * 128:(cb + 1) * 128],
                                    xq[:, t, sx, :, :],
                                    start=(mm == 0), stop=(mm == 11),
                                    perf_mode=DR)
                                mm += 1
                        dst = qkT[cb][:, s0 * 512:(s0 + 1) * 512]
                        with nc.allow_low_precision(reason="bf16"):
                            if qkcopy_eng == 'act':
                                nc.scalar.mul(dst, pg, 1.0 / WSCALE)
                            else:
                                nc.vector.tensor_scalar_mul(
                                    dst, pg, 1.0 / WSCALE)
                    for cb in range(4):
                        yield (lambda s0=s0, xq=xq, cb=cb:
                               qk_item(s0, xq, cb))

                    def v_item(s0=s0, xq=xq, ts=0):
                        pv = bigpool.tile([128, 4, 64], dt.float32,
                                          tag="big", name="pv")
                        mm = 0
                        for t in range(4):
                            for sx, sw in CHAINS:
                                nc.tensor.matmul(
                                    pv,
                                    xq[:, t, sx, :, ts * 128:(ts + 1) * 128],
                                    wv[:, t, sw, :, :],
                                    start=(mm == 0), stop=(mm == 11),
                                    perf_mode=DR)
                                mm += 1
                        st = s0 * 4 + ts
                        with nc.allow_low_precision(reason="bf16"):
                            nc.vector.tensor_scalar_mul(
                                vaug[:, st, :, 0:64], pv, 1.0 / WSCALE)
                    for ts in range(4):
                        yield (lambda s0=s0, xq=xq, ts=ts: v_item(s0, xq, ts))

            b_gen = b_items()
            b_total = 12 + (NBC - 1) * 8   # chunk 0 split into 12 items
            b_emitted = 0

            def emit_b(n):
                done = 0
                for _ in range(n):
                    item = next(b_gen, None)
                    if item is None:
                        break
                    item()
                    done += 1
                return done

            # ---------------- phase C: band attention ----------------
            from collections import deque
            pending = deque()
            _dq = deque()
            _held = []

            cur_stp = [None]  # most recent score tile (slice 5 = tp scratch)

            def score_unit(h, qt):
                pr, po = h // 2, (h % 2) * 64
                gs = [g for g in range(qt - 2, qt + 3) if 0 <= g < NT]
                nA = len(gs)
                stp = sapool.tile([128, 6, 128], dt.float32, tag="stp",
                                  name="stp")
                cur_stp[0] = stp
                for j in range(nA):
                    g = gs[j]
                    nc.tensor.matmul(
                        stp[:, j, :],
                        qkT[2 + pr][po:po + 64, g * 128:(g + 1) * 128],
                        qkT[pr][po:po + 64, qt * 128:(qt + 1) * 128])
                ptA = ptpool.tile([128, 5, 128], dt.bfloat16, tag="pt",
                                  name="ptA")
                nc.scalar.activation(ptA[:, 0:nA, :], stp[:, 0:nA, :],
                                     mybir.ActivationFunctionType.Exp,
                                     scale=1.0 / np.sqrt(HD))
                meng = nc.vector if mask_eng == 'dve' else nc.gpsimd
                with nc.allow_low_precision(reason="bf16"):
                    if gs[0] == qt - 2:
                        meng.tensor_mul(ptA[:, 0, :], ptA[:, 0, :],
                                        masks[:, 0, :])
                    if gs[-1] == qt + 2:
                        meng.tensor_mul(ptA[:, nA - 1, :],
                                        ptA[:, nA - 1, :], masks[:, 1, :])
                return (gs, nA, ptA)

            cn_ref = [None, None]  # per parity: pending pair ctxn tile
            _tq = deque()          # deferred ctxT transpose: (h, qt, ctxn2)

            def pv_unit(h, qt, gs, nA, ptA):
                pr = h // 2
                ctx = cxpool.tile([128, 65], dt.float32, tag="cx",
                                  name="ctx")
                n = len(gs)
                # masked slices (0 and n-1) go last: their mask ops on the
                # mask engine get the longest lead time
                order = list(range(1, n - 1)) + [n - 1, 0] if n > 2 \
                    else list(range(n))
                for i, j in enumerate(order):
                    nc.tensor.matmul(ctx, ptA[:, j, :], vaug[:, gs[j], h, :],
                                     start=(i == 0), stop=(i == n - 1))
                rec = recpool.tile([128, 1], dt.float32, tag="rec",
                                   name="rec")
                nc.vector.reciprocal(rec, ctx[:, 64:65])
                if h % 2 == 0:
                    cn_ref[pr] = cnpool.tile([128, 2, 64], dt.bfloat16,
                                             tag="cn", name="ctxn2")
                ctxn2 = cn_ref[pr]
                with nc.allow_low_precision(reason="bf16"):
                    nc.vector.tensor_scalar_mul(ctxn2[:, h % 2, :],
                                                ctx[:, 0:64], rec)
                _tq.append((h, qt, ctxn2))

            def tp_unit(h, qt, ctxn2):
                # PE-transpose a head pair's normalized context in one shot:
                # ctxn2 [128 q, 128 pairdims] -> tp [128 pairdims, 128 q].
                # Scratch = slice 5 of the score tile in flight (never used
                # for scores), viewed as bf16.
                if h % 2 == 1:
                    pr = h // 2
                    if tp_pool == 'big':
                        tp = bigpool.tile([128, 128], dt.bfloat16,
                                          tag="big", name="tp")
                    else:
                        tp = cxpool.tile([128, 128], dt.bfloat16, tag="cx",
                                         name="tp")
                    nc.tensor.transpose(tp, ctxn2, ident)
                    with nc.allow_low_precision(reason="bf16"):
                        nc.vector.tensor_copy(
                            ctxT[pr][:, qt * 128:(qt + 1) * 128], tp)
                if h == NH_CORE - 1:
                    _dq.append(qt)

            def emit_d(qt, split_dma=False):
                osb = opool.tile([128, 1024], dt.bfloat16, tag="osb",
                                 name="osb")
                deng = nc.gpsimd if osb_eng == 'gpsimd' else nc.sync
                for nn in range(2):
                    use_cx = (pd_pool == 'cx' or
                              (pd_pool == 'split' and nn == 0))
                    if use_cx:
                        pD = cxpool.tile([128, 512], dt.float32, tag="cx",
                                         name="pD")
                    else:
                        pD = bigpool.tile([128, 512], dt.float32, tag="big",
                                          name="pD")
                    for p in range(2):
                        nc.tensor.matmul(
                            pD, ctxT[p][:, qt * 128:(qt + 1) * 128],
                            wo[:, p, nn * 512:(nn + 1) * 512],
                            start=(p == 0), stop=(p == 1))
                    with nc.allow_low_precision(reason="bf16 partials"):
                        if nn == 0:
                            nc.scalar.copy(
                                osb[:, nn * 512:(nn + 1) * 512], pD)
                        else:
                            nc.vector.tensor_copy(
                                osb[:, nn * 512:(nn + 1) * 512], pD)
                    if split_dma:
                        deng.dma_start(
                            out=OUT[qt * 128:(qt + 1) * 128,
                                    nn * 512:(nn + 1) * 512],
                            in_=osb[:, nn * 512:(nn + 1) * 512])
                if not split_dma:
                    deng.dma_start(out=OUT[qt * 128:(qt + 1) * 128, :],
                                   in_=osb)

            # pacing: unit qt needs qkT/vaug through token (qt+2)*128+128,
            # i.e. chunks 0..ceil((qt*128+384)/512)-1 done.
            b_emitted += emit_b(8 * b_prol)
            for qt in range(NQT):
                # scores of qt need chunks covering tokens to (qt+3)*128-1,
                # i.e. chunks 0..(qt+2)//4 done; b_slack items of margin.
                need = min(b_total, 12 + 8 * ((qt + 2) // 4) + b_slack)
                # heads in order (0,2,1,3): staggers the two pair-chains
                for h in (0, 2, 1, 3):
                    dd = d_delay if qt < NQT - 2 else 0
                    while _dq and len(_dq) > dd:
                        dqt = _dq.popleft()
                        # park a few mid-sequence o-proj blocks: they become
                        # dependency-free PE work overlapping the final
                        # attention drain
                        if d_hold and len(_held) < d_hold and 16 <= dqt < 28:
                            _held.append(dqt)
                        else:
                            emit_d(dqt, split_dma=(dqt >= NQT - 2))
                    want = need - b_emitted
                    if want > 0:
                        per = max(1, (want + (NH_CORE - h) - 1)
                                  // (NH_CORE - h))
                        b_emitted += emit_b(per)
                    if pv_first == 2 and len(_tq) > tp_delay:
                        tp_unit(*_tq.popleft())
                    if pv_first == 1 and len(pending) >= depth:
                        pv_unit(*pending.popleft())
                        pending.append((h, qt) + score_unit(h, qt))
                    else:
                        pending.append((h, qt) + score_unit(h, qt))
                        if len(pending) > depth:
                            pv_unit(*pending.popleft())
                    if pv_first != 2 and len(_tq) > tp_delay:
                        tp_unit(*_tq.popleft())
            while pending:
                pv_unit(*pending.popleft())
                if len(_tq) > 1:
                    tp_unit(*_tq.popleft())
            while _tq:
                tp_unit(*_tq.popleft())
                while _dq:
                    emit_d(_dq.popleft())
            b_emitted += emit_b(b_total)
            while _dq:
                emit_d(_dq.popleft(), split_dma=True)
            for dqt in _held:
                emit_d(dqt, split_dma=True)

    nc.compile()
    return nc


f8 = ml_dtypes.float8_e4m3


def _split8(a):
    """a (f32) -> (a8, da8) fp8e4 with a ~= a8 + da8 (compensated split)."""
    a8 = a.astype(f8)
    d8 = (a - a8.astype(np.float32)).astype(f8)
    return a8, d8


WSCALE = 128.0  # lifts W (and its residual) out of e4m3's subnormal range


def _pack_w8(wcols, ncol):
    """[1024, ncol] f32 -> [128, 4t, 2s, 2i, ncol] fp8 with contraction
    index c = 256t + 128i + p. Weights are pre-scaled by WSCALE; the
    psum->sbuf copy divides it back out."""
    w8, dw8 = _split8(wcols * WSCALE)
    ws = np.stack([w8, dw8])                     # [s, 1024, ncol]
    ws = ws.reshape(2, 4, 2, 128, ncol)          # [s, t, i, p, col]
    return np.ascontiguousarray(ws.transpose(3, 1, 0, 2, 4))


def _prep_fast(x, Wqkv, Wo):
    """Per-core input maps (compensated fp8 QKV operands, bf16 Wo)."""
    xT_b = []
    for b in range(B):
        xt = np.ascontiguousarray(x[b].T)              # [E, S] f32
        x8, dx8 = _split8(xt)
        xs = np.stack([x8, dx8])                       # [s, E, S]
        xs = xs.reshape(2, 4, 2, 128, 8, 512)          # [s, t, i, p, s0, tok]
        xT_b.append(np.ascontiguousarray(xs.transpose(4, 3, 1, 0, 2, 5)))
        # xT_b[b][s0, p, t, s, i, tok] = xs[s, 256t+128i+p, 512*s0+tok]
    in_maps = []
    for c in range(8):
        b, hg = c // 4, c % 4
        heads = range(4 * hg, 4 * hg + 4)
        qcols = np.concatenate([np.arange(h * 192, h * 192 + 64)
                                for h in heads])
        kcols = qcols + 64
        vcols = qcols + 128
        wqk_cols = np.concatenate([qcols, kcols])           # [512]
        wqk = _pack_w8(Wqkv[:, wqk_cols], 512)
        wv = _pack_w8(Wqkv[:, vcols], 256)
        orows = np.concatenate([np.arange(h * 64, h * 64 + 64)
                                for h in heads])
        wo = np.ascontiguousarray(Wo[orows].reshape(2, 128, 1024)).astype(bf16)
        in_maps.append({"xT": xT_b[b], "wqk": wqk, "wv": wv, "wo": wo})
    return in_maps



def _build_generic(vbias=True, st_bufs=2, po_bufs=1, bc_bufs=1, cx_bufs=2,
           mask_eng='dve', bccopy_eng='act', pt_bufs=8,
           osbcopy_eng='dve', bcast_via='pe', paired=True, depth=1,
           fuse_b=True, pb_bufs=2, b_lead=3, b_prol=2,
           norm_src='sbuf', ctxcopy_eng='act'):
    if fuse_b:
        pt_bufs = min(pt_bufs, 6)
    _nb = 2 if fuse_b else 3
    nc = bacc.Bacc("TRN2", target_bir_lowering=False, debug=False, num_devices=8)

    XT = nc.dram_tensor("xT", [128, 16, 8, 256], dt.float32r, kind="ExternalInput")
    WQK = nc.dram_tensor("wqk", [128, 8, 4, 128], dt.float32r, kind="ExternalInput")
    WV = nc.dram_tensor("wv", [128, 8, 256], dt.float32r, kind="ExternalInput")
    WO = nc.dram_tensor("wo", [2, 128, 1024], dt.float32r, kind="ExternalInput")
    BQK = nc.dram_tensor("bqk", [128, 4], dt.float32, kind="ExternalInput")
    BV = nc.dram_tensor("bv", [1, 256], dt.float32, kind="ExternalInput")
    MV8 = nc.dram_tensor("mv8", [128, 32], dt.float32, kind="ExternalInput")
    OUT = nc.dram_tensor("out", [S, E], dt.float32, kind="ExternalOutput")

    # constant 0/1 triangular band masks for u in {-2,-1,2,3}
    p_i = np.arange(128)[:, None]
    r_i = np.arange(256)[None, :]
    mask_np = {}
    for u in (-2, -1, 2, 3):
        mask_np[u] = ((u * 128 + p_i - r_i >= -w) & (u * 128 + p_i - r_i <= w)
                      ).astype(np.float32)
    MASKS = nc.inline_tensor(
        np.ascontiguousarray(
            np.stack([mask_np[u] for u in (-2, -1, 2, 3)]).transpose(1, 0, 2)),
        name="trimasks")
    ONES = nc.inline_tensor(np.ones((1, 128), dtype=np.float32), name="onesrow")

    with tile.TileContext(nc) as tc:
        with tc.tile_pool(name="const", bufs=1) as cpool, \
             tc.tile_pool(name="qkT", bufs=1) as qkpool, \
             tc.tile_pool(name="vaug", bufs=1) as vpool, \
             tc.tile_pool(name="ctxT", bufs=1) as ctxpool:

            wo = [cpool.tile([128, 1024], dt.float32r, name=f"wo{p}") for p in range(2)]
            bqk = cpool.tile([128, 4], dt.float32)
            nc.gpsimd.dma_start(out=bqk, in_=BQK[:, :])
            bv_f = cpool.tile([1, 256], dt.float32)
            nc.gpsimd.dma_start(out=bv_f, in_=BV[:, :])
            mv8 = cpool.tile([128, 32], dt.float32)
            nc.gpsimd.dma_start(out=mv8, in_=MV8[:, :])
            masks = cpool.tile([128, 4, 256], dt.float32)
            mask_idx = {-2: 0, -1: 1, 2: 2, 3: 3}
            ones_f = cpool.tile([1, 128], dt.float32)
            nc.gpsimd.dma_start(out=ones_f, in_=ONES[:, :])
            ones_r = cpool.tile([1, 128], dt.float32r)
            bv_r = cpool.tile([1, 256], dt.float32r)
            with nc.allow_low_precision(reason="f32r matmul pipeline"):
                nc.vector.tensor_copy(ones_r, ones_f)
                nc.vector.tensor_copy(bv_r, bv_f)

            # persistent intermediates
            qkT = [qkpool.tile([128, S], dt.float32r, name=f"qkT{cb}")
                   for cb in range(4)]  # 0,1: q pairs; 2,3: k pairs
            vaug = [vpool.tile([128, NT, 65], dt.float32r, name=f"vaug{h}")
                    for h in range(NH_CORE)]
            ones32 = cpool.tile([128, NT], dt.float32)
            nc.vector.memset(ones32, 1.0)
            for h in range(NH_CORE):
                with nc.allow_low_precision(reason="f32r"):
                    nc.vector.tensor_copy(vaug[h][:, :, 64], ones32)
            ctxT = [ctxpool.tile([128, S], dt.float32r, name=f"ctxT{p}")
                    for p in range(2)]

            # ---------------- Phase B: QKV projection ----------------
            # Emitted either up front (fuse_b=False) or as fine-grained work
            # items interleaved into the attention loop's idle PE slots.
            bwpool = ctx_pools = None
            import contextlib
            _bstack = contextlib.ExitStack()
            bwpool = _bstack.enter_context(tc.tile_pool(name="bw", bufs=1))
            xqpool = _bstack.enter_context(
                tc.tile_pool(name="xq", bufs=(2 if fuse_b else 3)))
            pbpool = _bstack.enter_context(
                tc.tile_pool(name="pb", bufs=(pb_bufs if fuse_b else 8),
                             space="PSUM"))
            wqk = bwpool.tile([128, 8, 4, 128], dt.float32r)
            wv = bwpool.tile([128, 8, 256], dt.float32r)
            xq0 = [xqpool.tile([128, 4, 256], dt.float32r, tag=f"xq{i}",
                               name="xq") for i in range(2)]
            for i in range(2):
                nc.sync.dma_start(out=xq0[i], in_=XT[:, 0, i * 4:(i + 1) * 4, :])
            for kt in range(8):
                nc.sync.dma_start(out=wqk[:, kt, :, :], in_=WQK[:, kt, :, :])
            nc.sync.dma_start(out=wv[:, 0:4, :], in_=WV[:, 0:4, :])
            nc.sync.dma_start(out=wv[:, 4:8, :], in_=WV[:, 4:8, :])

            def b_items():
                for s0 in range(16):  # 256-token chunks of S
                    if s0 == 0:
                        xq = xq0
                    else:
                        xq = [xqpool.tile([128, 4, 256], dt.float32r,
                                          tag=f"xq{i}", name="xq")
                              for i in range(2)]
                        for i in range(2):
                            nc.sync.dma_start(
                                out=xq[i], in_=XT[:, s0, i * 4:(i + 1) * 4, :])

                    def qk_item(s0=s0, xq=xq, cb=0):
                        pg = pbpool.tile([128, 256], dt.float32, tag="pb",
                                         name="pqk")
                        for k8 in range(8):
                            nc.tensor.matmul(pg, wqk[:, k8, cb, :],
                                             xq[k8 // 4][:, k8 % 4, :],
                                             start=(k8 == 0), stop=(k8 == 7))
                        nc.scalar.activation(
                            qkT[cb][:, s0 * 256:(s0 + 1) * 256], pg,
                            mybir.ActivationFunctionType.Identity,
                            bias=bqk[:, cb:cb + 1])
                    for cb in range(4):
                        yield (lambda s0=s0, xq=xq, cb=cb: qk_item(s0, xq, cb))

                    def v_item(s0=s0, xq=xq, hf=0):
                        pv = pbpool.tile([128, 256], dt.float32, tag="pb",
                                         name="pv")
                        for k8 in range(8):
                            nc.tensor.matmul(
                                pv,
                                xq[k8 // 4][:, k8 % 4, hf * 128:(hf + 1) * 128],
                                wv[:, k8, :], start=(k8 == 0),
                                stop=(k8 == 7 and not vbias))
                        if vbias:
                            nc.tensor.matmul(pv, ones_r, bv_r,
                                             start=False, stop=True)
                        st = s0 * 2 + hf
                        for h in range(NH_CORE):
                            with nc.allow_low_precision(reason="f32r"):
                                nc.vector.tensor_copy(
                                    vaug[h][:, st, 0:64],
                                    pv[:, h * 64:(h + 1) * 64])
                    for hf in range(2):
                        yield (lambda s0=s0, xq=xq, hf=hf: v_item(s0, xq, hf))

            b_gen = b_items()
            b_total = 16 * 6
            b_emitted = 0

            def emit_b(n):
                emitted = 0
                for _ in range(n):
                    item = next(b_gen, None)
                    if item is None:
                        break
                    item()
                    emitted += 1
                return emitted

            if not fuse_b:
                b_emitted += emit_b(b_total)
                _bstack.close()

            nc.gpsimd.dma_start(out=masks, in_=MASKS[:, :, :])
            for p in range(2):
                nc.gpsimd.dma_start(out=wo[p], in_=WO[p, :, :])
            # ------- Phase C: band attention, with output projection folded in -------
            import contextlib
            _cstack = contextlib.ExitStack()
            with _cstack:
                stpool = _cstack.enter_context(
                    tc.tile_pool(name="stp", bufs=st_bufs, space="PSUM"))
                cxpool = _cstack.enter_context(
                    tc.tile_pool(name="ctxp", bufs=cx_bufs, space="PSUM"))
                if bcast_via == 'pe':
                    bcpool = _cstack.enter_context(
                        tc.tile_pool(name="bcp", bufs=bc_bufs, space="PSUM"))
                else:
                    drpool = _cstack.enter_context(
                        tc.tile_pool(name="dr", bufs=4, space="DRAM"))
                popool = _cstack.enter_context(
                    tc.tile_pool(name="po", bufs=po_bufs, space="PSUM"))
                ptpool = _cstack.enter_context(
                    tc.tile_pool(name="pt", bufs=pt_bufs))
                bcsb = _cstack.enter_context(tc.tile_pool(name="bcs", bufs=_nb))
                opool = _cstack.enter_context(tc.tile_pool(name="osb", bufs=2))
                rcpool = _cstack.enter_context(tc.tile_pool(name="rcp", bufs=_nb))

                def score_stage(h, cc):
                    # returns list of (gts, pt, jslices) where pt holds exp'd
                    # probabilities for the key tiles in gts
                    pr, po = h // 2, (h % 2) * 64
                    out = []
                    if paired:
                        # all-ones padding: exp has no per-key bias, so key
                        # tiles are processed in aligned pairs (one psum bank,
                        # one exp, one mask-mul per pair)
                        for ub in (-2, 0, 2):
                            gts = [2 * cc + ub, 2 * cc + ub + 1]
                            if gts[0] < 0 or gts[1] >= NT:
                                continue
                            stp = stpool.tile([128, 2, 256], dt.float32,
                                              tag="st", name="stp")
                            for j, gt in enumerate(gts):
                                nc.tensor.matmul(
                                    stp[:, j, :],
                                    qkT[2 + pr][po:po + 64,
                                                gt * 128:(gt + 1) * 128],
                                    qkT[pr][po:po + 64,
                                            cc * 256:(cc + 1) * 256])
                            pt = ptpool.tile([128, 2, 256], dt.float32r,
                                             tag="pt", name="pt")
                            nc.scalar.activation(
                                pt, stp, mybir.ActivationFunctionType.Exp,
                                scale=1.0 / np.sqrt(HD))
                            if ub != 0:
                                mi = 0 if ub == -2 else 2
                                with nc.allow_low_precision(reason="f32r"):
                                    eng = (nc.gpsimd if mask_eng == 'gpsimd'
                                           else nc.vector)
                                    eng.tensor_mul(pt, pt,
                                                   masks[:, mi:mi + 2, :])
                            out.append((gts, pt))
                        return out
                    for u in range(-2, 4):
                        gt = 2 * cc + u
                        if not 0 <= gt < NT:
                            continue
                        stp = stpool.tile([128, 256], dt.float32, tag="st",
                                          name="stp")
                        nc.tensor.matmul(
                            stp,
                            qkT[2 + pr][po:po + 64, gt * 128:(gt + 1) * 128],
                            qkT[pr][po:po + 64, cc * 256:(cc + 1) * 256])
                        pt = ptpool.tile([128, 256], dt.float32r, tag="pt",
                                         name="pt")
                        nc.scalar.activation(pt, stp,
                                             mybir.ActivationFunctionType.Exp,
                                             bias=mv8[:, gt:gt + 1],
                                             scale=1.0 / np.sqrt(HD))
                        if u in mask_idx:
                            with nc.allow_low_precision(reason="f32r"):
                                eng = (nc.gpsimd if mask_eng == 'gpsimd'
                                       else nc.vector)
                                eng.tensor_mul(pt, pt,
                                               masks[:, mask_idx[u], :])
                        out.append(([gt], pt))
                    return out

                def pv_stage(h, cc, pts):
                    if _dq:
                        emit_d(_dq.popleft())
                    pr, po = h // 2, (h % 2) * 64
                    ctx = cxpool.tile([65, 256], dt.float32, tag="cx",
                                      name="ctx")
                    nmm = sum(len(gts) for gts, _ in pts)
                    j = 0
                    for gts, pt in pts:
                        for jj, gt in enumerate(gts):
                            rhs = pt[:, jj, :] if len(gts) > 1 else pt
                            nc.tensor.matmul(ctx, vaug[h][:, gt, :], rhs,
                                             start=(j == 0),
                                             stop=(j == nmm - 1))
                            j += 1
                    if norm_src == 'sbuf':
                        # copy ctx out of PSUM first: frees the cx slot early
                        # and the final multiply reads bc straight from PSUM
                        cxs = bcsb.tile([65, 256], dt.float32, tag="bcs",
                                        name="cxs")
                        if ctxcopy_eng == 'act':
                            nc.scalar.copy(cxs, ctx)
                        else:
                            nc.vector.tensor_copy(cxs, ctx)
                        ctx = cxs
                    rec = rcpool.tile([1, 256], dt.float32r, tag="rc",
                                      name="rec")
                    with nc.allow_low_precision(reason="f32r"):
                        nc.vector.reciprocal(rec, ctx[64:65, :])
                    bcs = None
                    if norm_src != 'sbuf':
                        bcs = bcsb.tile([64, 256], dt.float32, tag="bcs",
                                        name="bcs")
                    if bcast_via == 'dma':
                        drec = drpool.tile([1, 256], dt.float32r, tag="dr",
                                           name="drec")
                        nc.sync.dma_start(out=drec, in_=rec)
                        dbc = bass.AP(tensor=drec.tensor, offset=drec.offset,
                                      ap=[[0, 64]] + drec.ap[1:])
                        nc.sync.dma_start(out=bcs.bitcast(dt.float32r), in_=dbc)
                    else:
                        bc = bcpool.tile([64, 256], dt.float32, tag="bc",
                                         name="bc")
                        nc.tensor.matmul(bc, ones_r[:, 0:64], rec)
                        if norm_src == 'sbuf':
                            bcs = bc
                        elif bccopy_eng == 'act':
                            nc.scalar.copy(bcs, bc)
                        else:
                            nc.vector.tensor_copy(bcs, bc)
                    with nc.allow_low_precision(reason="f32r"):
                        nc.vector.tensor_mul(
                            ctxT[pr][po:po + 64, cc * 256:(cc + 1) * 256],
                            ctx[0:64, :], bcs)
                    if h == NH_CORE - 1:
                        _dq.append(2 * cc)
                        _dq.append(2 * cc + 1)

                def emit_d(qt):
                    osb = opool.tile([128, 1024], dt.float32, tag="osb",
                                     name="osb")
                    for nn in range(2):
                        pD = popool.tile([128, 512], dt.float32, tag="po",
                                         name="pD")
                        for p in range(2):
                            nc.tensor.matmul(pD,
                                             ctxT[p][:, qt * 128:(qt + 1) * 128],
                                             wo[p][:, nn * 512:(nn + 1) * 512],
                                             start=(p == 0), stop=(p == 1))
                        if osbcopy_eng == 'act':
                            nc.scalar.copy(osb[:, nn * 512:(nn + 1) * 512], pD)
                        else:
                            nc.vector.tensor_copy(osb[:, nn * 512:(nn + 1) * 512], pD)
                    nc.gpsimd.dma_start(out=OUT[qt * 128:(qt + 1) * 128, :],
                                        in_=osb)

                from collections import deque
                pending = deque()
                _dq = deque()
                if fuse_b:
                    # prologue: cover key tiles for the first two query chunks
                    b_emitted += emit_b(6 * b_prol)
                step = 0
                for cc in range(NCC):
                    for h in range(NH_CORE):
                        if fuse_b:
                            # pace remaining B so chunk cc+2 is done before
                            # attention chunk cc+1 starts
                            target = min(b_total, 6 * (cc + b_lead))
                            want = target - b_emitted
                            per = max(1, (want + (NH_CORE - h) - 1)
                                      // (NH_CORE - h))
                            if want > 0:
                                b_emitted += emit_b(per)
                        pts = score_stage(h, cc)
                        pending.append((h, cc, pts))
                        if len(pending) > depth:
                            pv_stage(*pending.popleft())
                        step += 1
                while pending:
                    pv_stage(*pending.popleft())
                while _dq:
                    emit_d(_dq.popleft())
                if fuse_b:
                    b_emitted += emit_b(b_total)

            _bstack.close()

    nc.compile()
    return nc



def _prep_generic(x, Wqkv, bqkv, Wo, pm):
    in_maps = []
    xT_b = []
    for b in range(B):
        xt = np.ascontiguousarray(x[b].T)                      # [E, S]
        xT_b.append(np.ascontiguousarray(
            xt.reshape(8, 128, 16, 256).transpose(1, 2, 0, 3)))
    mv8_b = []
    for b in range(B):
        # mv8[p, t] = (0 if valid else NEG)/8 for key index t*128+p
        mv = np.where(pm[b], 0.0, NEG).astype(np.float32) / 8.0
        mv8_b.append(np.ascontiguousarray(mv.reshape(32, 128).T))

    for c in range(8):
        b, hg = c // 4, c % 4
        heads = range(4 * hg, 4 * hg + 4)
        qcols = np.concatenate([np.arange(h * 192, h * 192 + 64) for h in heads])
        kcols = qcols + 64
        vcols = qcols + 128
        wqk_cols = np.concatenate([qcols, kcols])               # [512]
        wqk = np.ascontiguousarray(
            Wqkv[:, wqk_cols].reshape(8, 128, 4, 128).transpose(1, 0, 2, 3))
        wv = np.ascontiguousarray(
            Wqkv[:, vcols].reshape(8, 128, 256).transpose(1, 0, 2))
        orows = np.concatenate([np.arange(h * 64, h * 64 + 64) for h in heads])
        wo = np.ascontiguousarray(Wo[orows].reshape(2, 128, 1024))
        in_maps.append({
            "xT": xT_b[b],
            "wqk": wqk,
            "wv": wv,
            "wo": wo,
            "bqk": np.ascontiguousarray(bqkv[wqk_cols].reshape(4, 128).T),
            "bv": np.ascontiguousarray(bqkv[vcols].reshape(1, 256)),
            "mv8": mv8_b[b],
        })
    return in_maps


def kernel(x, Wqkv, bqkv, Wo, bo, padding_mask, num_heads, window_size):
    assert int(num_heads) == H and int(window_size) == W
    x = np.asarray(x, dtype=np.float32)
    Wqkv = np.asarray(Wqkv, dtype=np.float32)
    bqkv = np.asarray(bqkv, dtype=np.float32)
    Wo = np.asarray(Wo, dtype=np.float32)
    bo = np.asarray(bo, dtype=np.float32)
    pm = np.asarray(padding_mask).astype(bool)
    assert x.shape == (B, S, E)

    fast = bool(pm.all()) and not np.any(bqkv)
    if fast:
        if "fast" not in _cache:
            _cache["fast"] = _build_fast(depth=3, tp_delay=1, d_delay=2,
                                         warm_n=8, osb_eng='sync',
                                         pd_pool='split', xq_eng='act',
                                         mask_eng='gpsimd', b_slack=2,
                                         b_prol=1, pv_first=2)
        nc = _cache["fast"]
        in_maps = _prep_fast(x, Wqkv, Wo)
    else:
        vbias = bool(np.any(bqkv.reshape(H, 3, HD)[:, 2, :] != 0.0))
        key = ("nc", vbias, False)
        if key not in _cache:
            _cache[key] = _build_generic(vbias=vbias, paired=False)
        nc = _cache[key]
        in_maps = _prep_generic(x, Wqkv, bqkv, Wo, pm)

    res = run_bass_kernel_spmd(nc, in_maps, list(range(8)))
    kernel._last_results = res

    out = np.empty((B, S, E), dtype=np.float32)
    for b in range(B):
        acc = res.results[4 * b]["out"].astype(np.float32)
        for g in range(1, 4):
            acc = acc + res.results[4 * b + g]["out"].astype(np.float32)
        out[b] = acc + bo
    return out



# revision 19
# speedup vs baseline: 1.0951x; 1.0039x over previous
"""Sliding-window multi-head attention (Longformer-style band attention) for
Trainium2, distributed over 8 NeuronCores.

Sharding: data-parallel over batch (B=2) x tensor-parallel over heads
(16 heads -> 4 groups of 4). Core c handles batch c//4, heads
[4*(c%4), 4*(c%4)+4). Each core computes the QKV projection for its head
group, band attention over 128-key tiles, and a partial output projection;
the host sums the 4 partials per batch and adds bo.

Fast path (all-ones padding mask, zero qkv bias): bf16 datapath end to end.
Scores are computed transposed ([key 128, query 128] tiles, 5 key tiles per
128-query block), exp'd on the scalar engine into bf16 probabilities,
triangular band masks applied on gpsimd, and PV accumulated as [query, 65]
with an appended ones column giving the softmax denominator for free.
Normalization is a per-partition reciprocal+scale on the vector engine; the
normalized context pair is PE-transposed and staged for the output
projection, which writes bf16 partials DMA'd from SBUF. QKV projection work
items are interleaved into the attention loop to keep the tensor engine
saturated, with scratch warm-up matmuls absorbing the PE clock ramp at
startup.

Generic path (padding masks / nonzero qkv bias) falls back to an f32r
implementation of the same blocking.
"""
import sys
import numpy as np
import ml_dtypes

try:
    import concourse.bass as bass
except ImportError:
    sys.path.insert(0, "/opt/trn_rl_repo")
    import concourse.bass as bass
import concourse.mybir as mybir
import concourse.tile as tile
from concourse import bacc
from concourse.bass_utils import run_bass_kernel_spmd

dt = mybir.dt
bf16 = ml_dtypes.bfloat16

B, S, E, H, W = 2, 4096, 1024, 16, 512
HD = E // H          # 64
NH_CORE = 4
w = W // 2           # 256
NT = S // 128        # 32 key tiles of 128
NQT = S // 128       # 32 query tiles of 128
NBC = S // 512       # 8 qkv token chunks of 512
NCC = S // 256       # generic path: 16 query chunks of 256
NEG = -9e15

_cache = {}


def _build_fast(depth=2, b_prol=2, mask_eng='dve', qkcopy_eng='dve',
                d_delay=1, cx_bufs=2, big_bufs=2, tp_delay=1, warm_n=0,
                osb_eng='gpsimd', pd_pool='big', xq_eng='sync', b_slack=8,
                d_hold=0, tp_pool='cx', pv_first=0):
    nc = bacc.Bacc("TRN2", target_bir_lowering=False, debug=False,
                   num_devices=8)

    # fp8 DoubleRow with host-side error compensation: x = x8 + dx8 and
    # W = W8 + dW8 (each fp8e4); three product chains x8W8 + x8dW8 + dx8W8
    # restore bf16-grade accuracy at 0.75x the bf16 PE cost (DoubleRow
    # contracts 256 rows per instruction at 0.5 cycles/row).
    # Layouts: [partition p, t (256-row ktile), s (main/residual), i
    # (DoubleRow pair), cols] with contraction index c = 256t + 128i + p.
    XT = nc.dram_tensor("xT", [8, 128, 4, 2, 2, 512], dt.float8e4,
                        kind="ExternalInput")
    WQK = nc.dram_tensor("wqk", [128, 4, 2, 2, 512], dt.float8e4,
                         kind="ExternalInput")
    WV = nc.dram_tensor("wv", [128, 4, 2, 2, 256], dt.float8e4,
                        kind="ExternalInput")
    WO = nc.dram_tensor("wo", [2, 128, 1024], dt.bfloat16,
                        kind="ExternalInput")
    OUT = nc.dram_tensor("out", [S, E], dt.bfloat16, kind="ExternalOutput")

    p_i = np.arange(128)[:, None]
    c_i = np.arange(128)[None, :]
    lo = (p_i >= c_i).astype(bf16)   # tile g==qt-2: valid kr >= qr
    up = (p_i <= c_i).astype(bf16)   # tile g==qt+2: valid kr <= qr
    MASKS = nc.inline_tensor(np.ascontiguousarray(
        np.stack([lo, up], axis=1)), name="trimasks")   # [128, 2, 128]
    IDENT = nc.inline_tensor(np.eye(128, dtype=bf16), name="ident")

    with tile.TileContext(nc) as tc:
        with tc.tile_pool(name="const", bufs=1) as cpool, \
             tc.tile_pool(name="qkTp", bufs=1) as qkpool, \
             tc.tile_pool(name="vaugp", bufs=1) as vpool, \
             tc.tile_pool(name="ctxTp", bufs=1) as ctpool, \
             tc.tile_pool(name="xq", bufs=4) as xqpool, \
             tc.tile_pool(name="pt", bufs=7) as ptpool, \
             tc.tile_pool(name="recp", bufs=4) as recpool, \
             tc.tile_pool(name="cnp", bufs=4) as cnpool, \
             tc.tile_pool(name="osbp", bufs=3) as opool, \
             tc.tile_pool(name="stp", bufs=2, space="PSUM") as sapool, \
             tc.tile_pool(name="cxp", bufs=cx_bufs, space="PSUM") as cxpool, \
             tc.tile_pool(name="bigp", bufs=big_bufs, space="PSUM") as bigpool:

            # ---- constants / weights ----
            wqk = cpool.tile([128, 4, 2, 2, 512], dt.float8e4)
            wv = cpool.tile([128, 4, 2, 2, 256], dt.float8e4)
            wo = cpool.tile([128, 2, 1024], dt.bfloat16)
            masks = cpool.tile([128, 2, 128], dt.bfloat16)
            ident = cpool.tile([128, 128], dt.bfloat16)
            # t-slice granularity so the first QKV matmuls start early
            # (subtile deps gate each accumulation step on its own slice);
            # scalar-engine HWDGE triggers: cheap and off the SP queue.
            # wqk/xq0 slices interleaved so slice pairs land together.
            xq0 = xqpool.tile([128, 4, 2, 2, 512], dt.float8e4, tag="xq",
                              name="xq")
            for kh in range(2):
                ks = slice(kh * 2, kh * 2 + 2)
                nc.scalar.dma_start(out=wqk[:, ks], in_=WQK[:, ks])
                nc.sync.dma_start(out=xq0[:, ks], in_=XT[0, :, ks])
            nc.scalar.dma_start(out=wv, in_=WV[:, :, :, :, :])
            nc.scalar.dma_start(out=masks, in_=MASKS[:, :, :])
            nc.scalar.dma_start(out=ident, in_=IDENT[:, :])
            nc.scalar.dma_start(out=wo[:, 0, :], in_=WO[0, :, :])
            nc.scalar.dma_start(out=wo[:, 1, :], in_=WO[1, :, :])

            # PE warmup: scratch matmuls absorb the p-state ramp while the
            # first input DMAs are still streaming in.
            if warm_n:
                wsrc = cpool.tile([128, 512], dt.bfloat16)
                nc.vector.memset(wsrc, 0.0)
                wdst = bigpool.tile([128, 512], dt.float32, tag="big",
                                    name="wdst")
                for i in range(warm_n):
                    nc.tensor.matmul(wdst, wsrc[:, 0:128], wsrc)

            # ---- persistent intermediates ----
            # q/k stored as fp8 DoubleRow slot pairs (scores run in fp8-DR
            # at half the bf16 PE cost). q: slot0 A = fp8(8q), slot1
            # B = fp8(8q - A); k: both slots fp8(-8k). The DR slot sum
            # k_n*A + k_n*B = k_n*8q cancels q's quantization error exactly;
            # only k's single-fp8 error remains. Score psum = -64*qk, undone
            # by a negative exp scale.
            qkT = [qkpool.tile([128, 2, S], dt.float8e4, name=f"qkT{cb}")
                   for cb in range(4)]          # 0,1: q head pairs; 2,3: k
            vaug = vpool.tile([128, NT, NH_CORE, 65], dt.bfloat16)
            with nc.allow_low_precision(reason="ones col"):
                nc.vector.memset(vaug[:, :, :, 64], 1.0)
            ctxT = [ctpool.tile([128, S], dt.bfloat16, name=f"ctxT{p}")
                    for p in range(2)]

            # ---------------- phase B: QKV projection ----------------
            # (sw, sx) product chains: x8·W8 + dx8·W8 + x8·dW8
            CHAINS = ((0, 0), (1, 0), (0, 1))
            DR = mybir.MatmulPerfMode.DoubleRow
            QS = 8.0 / WSCALE   # psum (q*WSCALE) -> stored 8q / -8k

            def qk_store(cb, sl, pg):
                d0 = qkT[cb][:, 0, sl]
                d1 = qkT[cb][:, 1, sl]
                with nc.allow_low_precision(reason="fp8 score operands"):
                    if cb < 2:   # q: slot A, then residual B = 8q - A
                        nc.vector.tensor_scalar_mul(d0, pg, QS)
                        nc.vector.ln_bwd_dx(d1, pg, d0, 1.0 / QS, 0.0,
                                            scale=QS)
                    else:        # k: fp8(-8k) duplicated into both slots
                        nc.vector.tensor_scalar_mul(d0, pg, -QS)
                        nc.vector.tensor_scalar_mul(d1, pg, -QS)

            def make_xq(s0):
                xq = xqpool.tile([128, 4, 2, 2, 512], dt.float8e4, tag="xq",
                                 name="xq")
                xeng = nc.scalar if xq_eng == 'act' else nc.sync
                xeng.dma_start(out=xq, in_=XT[s0])
                return xq

            def b_items():
                pre = [xq0, make_xq(1)]
                for s0 in range(NBC):
                    xq = pre[0]
                    pre = pre[1:]
                    if s0 + 2 < NBC:
                        pre.append(make_xq(s0 + 2))  # prefetch 2 ahead

                    # chunk 0: two-pass accumulation so the first matmuls
                    # only need the first half of wqk/xq0 (still streaming)
                    if s0 == 0:
                        pgs = {}

                        def qk_half(cb, kh):
                            if kh == 0:
                                pgs[cb] = bigpool.tile(
                                    [128, 512], dt.float32, tag="big",
                                    name="pg")
                            pg = pgs[cb]
                            mm = kh * 6
                            for t in (kh * 2, kh * 2 + 1):
                                for sw, sx in CHAINS:
                                    nc.tensor.matmul(
                                        pg,
                                        wqk[:, t, sw, :,
                                            cb * 128:(cb + 1) * 128],
                                        xq[:, t, sx, :, :],
                                        start=(mm == 0), stop=(mm == 11),
                                        perf_mode=DR)
                                    mm += 1
                            if kh == 1:
                                qk_store(cb, slice(0, 512), pg)

                        # pairwise interleave: at most 2 open psum groups
                        # (ring=2), first items need only the first halves
                        for cb0 in (0, 2):
                            yield (lambda cb=cb0: qk_half(cb, 0))
                            yield (lambda cb=cb0 + 1: qk_half(cb, 0))
                            yield (lambda cb=cb0: qk_half(cb, 1))
                            yield (lambda cb=cb0 + 1: qk_half(cb, 1))

                        def v_item0(ts):
                            pv = bigpool.tile([128, 4, 64], dt.float32,
                                              tag="big", name="pv")
                            mm = 0
                            for t in range(4):
                                for sx, sw in CHAINS:
                                    nc.tensor.matmul(
                                        pv,
                                        xq[:, t, sx, :,
                                           ts * 128:(ts + 1) * 128],
                                        wv[:, t, sw, :, :],
                                        start=(mm == 0), stop=(mm == 11),
                                        perf_mode=DR)
                                    mm += 1
                            with nc.allow_low_precision(reason="bf16"):
                                nc.vector.tensor_scalar_mul(
                                    vaug[:, ts, :, 0:64], pv, 1.0 / WSCALE)
                        for ts in range(4):
                            yield (lambda ts=ts: v_item0(ts))
                        continue

                    def qk_item(s0=s0, xq=xq, cb=0):
                        pg = bigpool.tile([128, 512], dt.float32, tag="big",
                                          name="pg")
                        mm = 0
                        for t in range(4):
                            for sw, sx in CHAINS:
                                nc.tensor.matmul(
                                    pg,
                                    wqk[:, t, sw, :, cb * 128:(cb + 1) * 128],
                                    xq[:, t, sx, :, :],
                                    start=(mm == 0), stop=(mm == 11),
                                    perf_mode=DR)
                                mm += 1
                        qk_store(cb, slice(s0 * 512, (s0 + 1) * 512), pg)
                    for cb in range(4):
                        yield (lambda s0=s0, xq=xq, cb=cb:
                               qk_item(s0, xq, cb))

                    def v_item(s0=s0, xq=xq, ts=0):
                        pv = bigpool.tile([128, 4, 64], dt.float32,
                                          tag="big", name="pv")
                        mm = 0
                        for t in range(4):
                            for sx, sw in CHAINS:
                                nc.tensor.matmul(
                                    pv,
                                    xq[:, t, sx, :, ts * 128:(ts + 1) * 128],
                                    wv[:, t, sw, :, :],
                                    start=(mm == 0), stop=(mm == 11),
                                    perf_mode=DR)
                                mm += 1
                        st = s0 * 4 + ts
                        with nc.allow_low_precision(reason="bf16"):
                            nc.vector.tensor_scalar_mul(
                                vaug[:, st, :, 0:64], pv, 1.0 / WSCALE)
                    for ts in range(4):
                        yield (lambda s0=s0, xq=xq, ts=ts: v_item(s0, xq, ts))

            b_gen = b_items()
            b_total = 12 + (NBC - 1) * 8   # chunk 0 split into 12 items
            b_emitted = 0

            def emit_b(n):
                done = 0
                for _ in range(n):
                    item = next(b_gen, None)
                    if item is None:
                        break
                    item()
                    done += 1
                return done

            # ---------------- phase C: band attention ----------------
            from collections import deque
            pending = deque()
            _dq = deque()
            _held = []

            cur_stp = [None]  # most recent score tile (slice 5 = tp scratch)

            def score_unit(h, qt):
                pr, po = h // 2, (h % 2) * 64
                gs = [g for g in range(qt - 2, qt + 3) if 0 <= g < NT]
                nA = len(gs)
                stp = sapool.tile([128, 6, 128], dt.float32, tag="stp",
                                  name="stp")
                cur_stp[0] = stp
                for j in range(nA):
                    g = gs[j]
                    nc.tensor.matmul(
                        stp[:, j, :],
                        qkT[2 + pr][po:po + 64, :, g * 128:(g + 1) * 128],
                        qkT[pr][po:po + 64, :, qt * 128:(qt + 1) * 128],
                        perf_mode=DR)
                ptA = ptpool.tile([128, 5, 128], dt.bfloat16, tag="pt",
                                  name="ptA")
                # psum holds -64*qk; negative scale restores exp(qk/8)
                nc.scalar.activation(ptA[:, 0:nA, :], stp[:, 0:nA, :],
                                     mybir.ActivationFunctionType.Exp,
                                     scale=-1.0 / (64.0 * np.sqrt(HD)))
                meng = nc.vector if mask_eng == 'dve' else nc.gpsimd
                with nc.allow_low_precision(reason="bf16"):
                    if gs[0] == qt - 2:
                        meng.tensor_mul(ptA[:, 0, :], ptA[:, 0, :],
                                        masks[:, 0, :])
                    if gs[-1] == qt + 2:
                        meng.tensor_mul(ptA[:, nA - 1, :],
                                        ptA[:, nA - 1, :], masks[:, 1, :])
                return (gs, nA, ptA)

            cn_ref = [None, None]  # per parity: pending pair ctxn tile
            _tq = deque()          # deferred ctxT transpose: (h, qt, ctxn2)

            def pv_unit(h, qt, gs, nA, ptA):
                pr = h // 2
                ctx = cxpool.tile([128, 65], dt.float32, tag="cx",
                                  name="ctx")
                n = len(gs)
                # masked slices (0 and n-1) go last: their mask ops on the
                # mask engine get the longest lead time
                order = list(range(1, n - 1)) + [n - 1, 0] if n > 2 \
                    else list(range(n))
                for i, j in enumerate(order):
                    nc.tensor.matmul(ctx, ptA[:, j, :], vaug[:, gs[j], h, :],
                                     start=(i == 0), stop=(i == n - 1))
                rec = recpool.tile([128, 1], dt.float32, tag="rec",
                                   name="rec")
                nc.vector.reciprocal(rec, ctx[:, 64:65])
                if h % 2 == 0:
                    cn_ref[pr] = cnpool.tile([128, 2, 64], dt.bfloat16,
                                             tag="cn", name="ctxn2")
                ctxn2 = cn_ref[pr]
                with nc.allow_low_precision(reason="bf16"):
                    nc.vector.tensor_scalar_mul(ctxn2[:, h % 2, :],
                                                ctx[:, 0:64], rec)
                _tq.append((h, qt, ctxn2))

            def tp_unit(h, qt, ctxn2):
                # PE-transpose a head pair's normalized context in one shot:
                # ctxn2 [128 q, 128 pairdims] -> tp [128 pairdims, 128 q].
                # Scratch = slice 5 of the score tile in flight (never used
                # for scores), viewed as bf16.
                if h % 2 == 1:
                    pr = h // 2
                    if tp_pool == 'big':
                        tp = bigpool.tile([128, 128], dt.bfloat16,
                                          tag="big", name="tp")
                    else:
                        tp = cxpool.tile([128, 128], dt.bfloat16, tag="cx",
                                         name="tp")
                    nc.tensor.transpose(tp, ctxn2, ident)
                    with nc.allow_low_precision(reason="bf16"):
                        nc.vector.tensor_copy(
                            ctxT[pr][:, qt * 128:(qt + 1) * 128], tp)
                if h == NH_CORE - 1:
                    _dq.append(qt)

            def emit_d(qt, split_dma=False):
                osb = opool.tile([128, 1024], dt.bfloat16, tag="osb",
                                 name="osb")
                deng = nc.gpsimd if osb_eng == 'gpsimd' else nc.sync
                for nn in range(2):
                    use_cx = (pd_pool == 'cx' or
                              (pd_pool == 'split' and nn == 0))
                    if use_cx:
                        pD = cxpool.tile([128, 512], dt.float32, tag="cx",
                                         name="pD")
                    else:
                        pD = bigpool.tile([128, 512], dt.float32, tag="big",
                                          name="pD")
                    for p in range(2):
                        nc.tensor.matmul(
                            pD, ctxT[p][:, qt * 128:(qt + 1) * 128],
                            wo[:, p, nn * 512:(nn + 1) * 512],
                            start=(p == 0), stop=(p == 1))
                    with nc.allow_low_precision(reason="bf16 partials"):
                        if nn == 0:
                            nc.scalar.copy(
                                osb[:, nn * 512:(nn + 1) * 512], pD)
                        else:
                            nc.vector.tensor_copy(
                                osb[:, nn * 512:(nn + 1) * 512], pD)
                    if split_dma:
                        deng.dma_start(
                            out=OUT[qt * 128:(qt + 1) * 128,
                                    nn * 512:(nn + 1) * 512],
                            in_=osb[:, nn * 512:(nn + 1) * 512])
                if not split_dma:
                    deng.dma_start(out=OUT[qt * 128:(qt + 1) * 128, :],
                                   in_=osb)

            # pacing: unit qt needs qkT/vaug through token (qt+2)*128+128,
            # i.e. chunks 0..ceil((qt*128+384)/512)-1 done.
            b_emitted += emit_b(8 * b_prol)
            for qt in range(NQT):
                # scores of qt need chunks covering tokens to (qt+3)*128-1,
                # i.e. chunks 0..(qt+2)//4 done; b_slack items of margin.
                need = min(b_total, 12 + 8 * ((qt + 2) // 4) + b_slack)
                # heads in order (0,2,1,3): staggers the two pair-chains
                for h in (0, 2, 1, 3):
                    dd = d_delay if qt < NQT - 2 else 0
                    while _dq and len(_dq) > dd:
                        dqt = _dq.popleft()
                        # park a few mid-sequence o-proj blocks: they become
                        # dependency-free PE work overlapping the final
                        # attention drain
                        if d_hold and len(_held) < d_hold and 16 <= dqt < 28:
                            _held.append(dqt)
                        else:
                            emit_d(dqt, split_dma=(dqt >= NQT - 2))
                    want = need - b_emitted
                    if want > 0:
                        per = max(1, (want + (NH_CORE - h) - 1)
                                  // (NH_CORE - h))
                        b_emitted += emit_b(per)
                    if pv_first == 2 and len(_tq) > tp_delay:
                        tp_unit(*_tq.popleft())
                    if pv_first == 1 and len(pending) >= depth:
                        pv_unit(*pending.popleft())
                        pending.append((h, qt) + score_unit(h, qt))
                    else:
                        pending.append((h, qt) + score_unit(h, qt))
                        if len(pending) > depth:
                            pv_unit(*pending.popleft())
                    if pv_first != 2 and len(_tq) > tp_delay:
                        tp_unit(*_tq.popleft())
            while pending:
                pv_unit(*pending.popleft())
                if len(_tq) > 1:
                    tp_unit(*_tq.popleft())
            while _tq:
                tp_unit(*_tq.popleft())
                while _dq:
                    emit_d(_dq.popleft())
            b_emitted += emit_b(b_total)
            while _dq:
                emit_d(_dq.popleft(), split_dma=True)
            for dqt in _held:
                emit_d(dqt, split_dma=True)

    nc.compile()
    return nc


f8 = ml_dtypes.float8_e4m3


def _split8(a):
    """a (f32) -> (a8, da8) fp8e4 with a ~= a8 + da8 (compensated split)."""
    a8 = a.astype(f8)
    d8 = (a - a8.astype(np.float32)).astype(f8)
    return a8, d8


WSCALE = 128.0  # lifts W (and its residual) out of e4m3's subnormal range


def _pack_w8(wcols, ncol):
    """[1024, ncol] f32 -> [128, 4t, 2s, 2i, ncol] fp8 with contraction
    index c = 256t + 128i + p. Weights are pre-scaled by WSCALE; the
    psum->sbuf copy divides it back out."""
    w8, dw8 = _split8(wcols * WSCALE)
    ws = np.stack([w8, dw8])                     # [s, 1024, ncol]
    ws = ws.reshape(2, 4, 2, 128, ncol)          # [s, t, i, p, col]
    return np.ascontiguousarray(ws.transpose(3, 1, 0, 2, 4))


def _prep_fast(x, Wqkv, Wo):
    """Per-core input maps (compensated fp8 QKV operands, bf16 Wo)."""
    xT_b = []
    for b in range(B):
        xt = np.ascontiguousarray(x[b].T)              # [E, S] f32
        x8, dx8 = _split8(xt)
        xs = np.stack([x8, dx8])                       # [s, E, S]
        xs = xs.reshape(2, 4, 2, 128, 8, 512)          # [s, t, i, p, s0, tok]
        xT_b.append(np.ascontiguousarray(xs.transpose(4, 3, 1, 0, 2, 5)))
        # xT_b[b][s0, p, t, s, i, tok] = xs[s, 256t+128i+p, 512*s0+tok]
    in_maps = []
    for c in range(8):
        b, hg = c // 4, c % 4
        heads = range(4 * hg, 4 * hg + 4)
        qcols = np.concatenate([np.arange(h * 192, h * 192 + 64)
                                for h in heads])
        kcols = qcols + 64
        vcols = qcols + 128
        wqk_cols = np.concatenate([qcols, kcols])           # [512]
        wqk = _pack_w8(Wqkv[:, wqk_cols], 512)
        wv = _pack_w8(Wqkv[:, vcols], 256)
        orows = np.concatenate([np.arange(h * 64, h * 64 + 64)
                                for h in heads])
        wo = np.ascontiguousarray(Wo[orows].reshape(2, 128, 1024)).astype(bf16)
        in_maps.append({"xT": xT_b[b], "wqk": wqk, "wv": wv, "wo": wo})
    return in_maps



def _build_generic(vbias=True, st_bufs=2, po_bufs=1, bc_bufs=1, cx_bufs=2,
           mask_eng='dve', bccopy_eng='act', pt_bufs=8,
           osbcopy_eng='dve', bcast_via='pe', paired=True, depth=1,
           fuse_b=True, pb_bufs=2, b_lead=3, b_prol=2,
           norm_src='sbuf', ctxcopy_eng='act'):
    if fuse_b:
        pt_bufs = min(pt_bufs, 6)
    _nb = 2 if fuse_b else 3
    nc = bacc.Bacc("TRN2", target_bir_lowering=False, debug=False, num_devices=8)

    XT = nc.dram_tensor("xT", [128, 16, 8, 256], dt.float32r, kind="ExternalInput")
    WQK = nc.dram_tensor("wqk", [128, 8, 4, 128], dt.float32r, kind="ExternalInput")
    WV = nc.dram_tensor("wv", [128, 8, 256], dt.float32r, kind="ExternalInput")
    WO = nc.dram_tensor("wo", [2, 128, 1024], dt.float32r, kind="ExternalInput")
    BQK = nc.dram_tensor("bqk", [128, 4], dt.float32, kind="ExternalInput")
    BV = nc.dram_tensor("bv", [1, 256], dt.float32, kind="ExternalInput")
    MV8 = nc.dram_tensor("mv8", [128, 32], dt.float32, kind="ExternalInput")
    OUT = nc.dram_tensor("out", [S, E], dt.float32, kind="ExternalOutput")

    # constant 0/1 triangular band masks for u in {-2,-1,2,3}
    p_i = np.arange(128)[:, None]
    r_i = np.arange(256)[None, :]
    mask_np = {}
    for u in (-2, -1, 2, 3):
        mask_np[u] = ((u * 128 + p_i - r_i >= -w) & (u * 128 + p_i - r_i <= w)
                      ).astype(np.float32)
    MASKS = nc.inline_tensor(
        np.ascontiguousarray(
            np.stack([mask_np[u] for u in (-2, -1, 2, 3)]).transpose(1, 0, 2)),
        name="trimasks")
    ONES = nc.inline_tensor(np.ones((1, 128), dtype=np.float32), name="onesrow")

    with tile.TileContext(nc) as tc:
        with tc.tile_pool(name="const", bufs=1) as cpool, \
             tc.tile_pool(name="qkT", bufs=1) as qkpool, \
             tc.tile_pool(name="vaug", bufs=1) as vpool, \
             tc.tile_pool(name="ctxT", bufs=1) as ctxpool:

            wo = [cpool.tile([128, 1024], dt.float32r, name=f"wo{p}") for p in range(2)]
            bqk = cpool.tile([128, 4], dt.float32)
            nc.gpsimd.dma_start(out=bqk, in_=BQK[:, :])
            bv_f = cpool.tile([1, 256], dt.float32)
            nc.gpsimd.dma_start(out=bv_f, in_=BV[:, :])
            mv8 = cpool.tile([128, 32], dt.float32)
            nc.gpsimd.dma_start(out=mv8, in_=MV8[:, :])
            masks = cpool.tile([128, 4, 256], dt.float32)
            mask_idx = {-2: 0, -1: 1, 2: 2, 3: 3}
            ones_f = cpool.tile([1, 128], dt.float32)
            nc.gpsimd.dma_start(out=ones_f, in_=ONES[:, :])
            ones_r = cpool.tile([1, 128], dt.float32r)
            bv_r = cpool.tile([1, 256], dt.float32r)
            with nc.allow_low_precision(reason="f32r matmul pipeline"):
                nc.vector.tensor_copy(ones_r, ones_f)
                nc.vector.tensor_copy(bv_r, bv_f)

            # persistent intermediates
            qkT = [qkpool.tile([128, S], dt.float32r, name=f"qkT{cb}")
                   for cb in range(4)]  # 0,1: q pairs; 2,3: k pairs
            vaug = [vpool.tile([128, NT, 65], dt.float32r, name=f"vaug{h}")
                    for h in range(NH_CORE)]
            ones32 = cpool.tile([128, NT], dt.float32)
            nc.vector.memset(ones32, 1.0)
            for h in range(NH_CORE):
                with nc.allow_low_precision(reason="f32r"):
                    nc.vector.tensor_copy(vaug[h][:, :, 64], ones32)
            ctxT = [ctxpool.tile([128, S], dt.float32r, name=f"ctxT{p}")
                    for p in range(2)]

            # ---------------- Phase B: QKV projection ----------------
            # Emitted either up front (fuse_b=False) or as fine-grained work
            # items interleaved into the attention loop's idle PE slots.
            bwpool = ctx_pools = None
            import contextlib
            _bstack = contextlib.ExitStack()
            bwpool = _bstack.enter_context(tc.tile_pool(name="bw", bufs=1))
            xqpool = _bstack.enter_context(
                tc.tile_pool(name="xq", bufs=(2 if fuse_b else 3)))
            pbpool = _bstack.enter_context(
                tc.tile_pool(name="pb", bufs=(pb_bufs if fuse_b else 8),
                             space="PSUM"))
            wqk = bwpool.tile([128, 8, 4, 128], dt.float32r)
            wv = bwpool.tile([128, 8, 256], dt.float32r)
            xq0 = [xqpool.tile([128, 4, 256], dt.float32r, tag=f"xq{i}",
                               name="xq") for i in range(2)]
            for i in range(2):
                nc.sync.dma_start(out=xq0[i], in_=XT[:, 0, i * 4:(i + 1) * 4, :])
            for kt in range(8):
                nc.sync.dma_start(out=wqk[:, kt, :, :], in_=WQK[:, kt, :, :])
            nc.sync.dma_start(out=wv[:, 0:4, :], in_=WV[:, 0:4, :])
            nc.sync.dma_start(out=wv[:, 4:8, :], in_=WV[:, 4:8, :])

            def b_items():
                for s0 in range(16):  # 256-token chunks of S
                    if s0 == 0:
                        xq = xq0
                    else:
                        xq = [xqpool.tile([128, 4, 256], dt.float32r,
                                          tag=f"xq{i}", name="xq")
                              for i in range(2)]
                        for i in range(2):
                            nc.sync.dma_start(
                                out=xq[i], in_=XT[:, s0, i * 4:(i + 1) * 4, :])

                    def qk_item(s0=s0, xq=xq, cb=0):
                        pg = pbpool.tile([128, 256], dt.float32, tag="pb",
                                         name="pqk")
                        for k8 in range(8):
                            nc.tensor.matmul(pg, wqk[:, k8, cb, :],
                                             xq[k8 // 4][:, k8 % 4, :],
                                             start=(k8 == 0), stop=(k8 == 7))
                        nc.scalar.activation(
                            qkT[cb][:, s0 * 256:(s0 + 1) * 256], pg,
                            mybir.ActivationFunctionType.Identity,
                            bias=bqk[:, cb:cb + 1])
                    for cb in range(4):
                        yield (lambda s0=s0, xq=xq, cb=cb: qk_item(s0, xq, cb))

                    def v_item(s0=s0, xq=xq, hf=0):
                        pv = pbpool.tile([128, 256], dt.float32, tag="pb",
                                         name="pv")
                        for k8 in range(8):
                            nc.tensor.matmul(
                                pv,
                                xq[k8 // 4][:, k8 % 4, hf * 128:(hf + 1) * 128],
                                wv[:, k8, :], start=(k8 == 0),
                                stop=(k8 == 7 and not vbias))
                        if vbias:
                            nc.tensor.matmul(pv, ones_r, bv_r,
                                             start=False, stop=True)
                        st = s0 * 2 + hf
                        for h in range(NH_CORE):
                            with nc.allow_low_precision(reason="f32r"):
                                nc.vector.tensor_copy(
                                    vaug[h][:, st, 0:64],
                                    pv[:, h * 64:(h + 1) * 64])
                    for hf in range(2):
                        yield (lambda s0=s0, xq=xq, hf=hf: v_item(s0, xq, hf))

            b_gen = b_items()
            b_total = 16 * 6
            b_emitted = 0

            def emit_b(n):
                emitted = 0
                for _ in range(n):
                    item = next(b_gen, None)
                    if item is None:
                        break
                    item()
                    emitted += 1
                return emitted

            if not fuse_b:
                b_emitted += emit_b(b_total)
                _bstack.close()

            nc.gpsimd.dma_start(out=masks, in_=MASKS[:, :, :])
            for p in range(2):
                nc.gpsimd.dma_start(out=wo[p], in_=WO[p, :, :])
            # ------- Phase C: band attention, with output projection folded in -------
            import contextlib
            _cstack = contextlib.ExitStack()
            with _cstack:
                stpool = _cstack.enter_context(
                    tc.tile_pool(name="stp", bufs=st_bufs, space="PSUM"))
                cxpool = _cstack.enter_context(
                    tc.tile_pool(name="ctxp", bufs=cx_bufs, space="PSUM"))
                if bcast_via == 'pe':
                    bcpool = _cstack.enter_context(
                        tc.tile_pool(name="bcp", bufs=bc_bufs, space="PSUM"))
                else:
                    drpool = _cstack.enter_context(
                        tc.tile_pool(name="dr", bufs=4, space="DRAM"))
                popool = _cstack.enter_context(
                    tc.tile_pool(name="po", bufs=po_bufs, space="PSUM"))
                ptpool = _cstack.enter_context(
                    tc.tile_pool(name="pt", bufs=pt_bufs))
                bcsb = _cstack.enter_context(tc.tile_pool(name="bcs", bufs=_nb))
                opool = _cstack.enter_context(tc.tile_pool(name="osb", bufs=2))
                rcpool = _cstack.enter_context(tc.tile_pool(name="rcp", bufs=_nb))

                def score_stage(h, cc):
                    # returns list of (gts, pt, jslices) where pt holds exp'd
                    # probabilities for the key tiles in gts
                    pr, po = h // 2, (h % 2) * 64
                    out = []
                    if paired:
                        # all-ones padding: exp has no per-key bias, so key
                        # tiles are processed in aligned pairs (one psum bank,
                        # one exp, one mask-mul per pair)
                        for ub in (-2, 0, 2):
                            gts = [2 * cc + ub, 2 * cc + ub + 1]
                            if gts[0] < 0 or gts[1] >= NT:
                                continue
                            stp = stpool.tile([128, 2, 256], dt.float32,
                                              tag="st", name="stp")
                            for j, gt in enumerate(gts):
                                nc.tensor.matmul(
                                    stp[:, j, :],
                                    qkT[2 + pr][po:po + 64,
                                                gt * 128:(gt + 1) * 128],
                                    qkT[pr][po:po + 64,
                                            cc * 256:(cc + 1) * 256])
                            pt = ptpool.tile([128, 2, 256], dt.float32r,
                                             tag="pt", name="pt")
                            nc.scalar.activation(
                                pt, stp, mybir.ActivationFunctionType.Exp,
                                scale=1.0 / np.sqrt(HD))
                            if ub != 0:
                                mi = 0 if ub == -2 else 2
                                with nc.allow_low_precision(reason="f32r"):
                                    eng = (nc.gpsimd if mask_eng == 'gpsimd'
                                           else nc.vector)
                                    eng.tensor_mul(pt, pt,
                                                   masks[:, mi:mi + 2, :])
                            out.append((gts, pt))
                        return out
                    for u in range(-2, 4):
                        gt = 2 * cc + u
                        if not 0 <= gt < NT:
                            continue
                        stp = stpool.tile([128, 256], dt.float32, tag="st",
                                          name="stp")
                        nc.tensor.matmul(
                            stp,
                            qkT[2 + pr][po:po + 64, gt * 128:(gt + 1) * 128],
                            qkT[pr][po:po + 64, cc * 256:(cc + 1) * 256])
                        pt = ptpool.tile([128, 256], dt.float32r, tag="pt",
                                         name="pt")
                        nc.scalar.activation(pt, stp,
                                             mybir.ActivationFunctionType.Exp,
                                             bias=mv8[:, gt:gt + 1],
                                             scale=1.0 / np.sqrt(HD))
                        if u in mask_idx:
                            with nc.allow_low_precision(reason="f32r"):
                                eng = (nc.gpsimd if mask_eng == 'gpsimd'
                                       else nc.vector)
                                eng.tensor_mul(pt, pt,
                                               masks[:, mask_idx[u], :])
                        out.append(([gt], pt))
                    return out

                def pv_stage(h, cc, pts):
                    if _dq:
                        emit_d(_dq.popleft())
                    pr, po = h // 2, (h % 2) * 64
                    ctx = cxpool.tile([65, 256], dt.float32, tag="cx",
                                      name="ctx")
                    nmm = sum(len(gts) for gts, _ in pts)
                    j = 0
                    for gts, pt in pts:
                        for jj, gt in enumerate(gts):
                            rhs = pt[:, jj, :] if len(gts) > 1 else pt
                            nc.tensor.matmul(ctx, vaug[h][:, gt, :], rhs,
                                             start=(j == 0),
                                             stop=(j == nmm - 1))
                            j += 1
                    if norm_src == 'sbuf':
                        # copy ctx out of PSUM first: frees the cx slot early
                        # and the final multiply reads bc straight from PSUM
                        cxs = bcsb.tile([65, 256], dt.float32, tag="bcs",
                                        name="cxs")
                        if ctxcopy_eng == 'act':
                            nc.scalar.copy(cxs, ctx)
                        else:
                            nc.vector.tensor_copy(cxs, ctx)
                        ctx = cxs
                    rec = rcpool.tile([1, 256], dt.float32r, tag="rc",
                                      name="rec")
                    with nc.allow_low_precision(reason="f32r"):
                        nc.vector.reciprocal(rec, ctx[64:65, :])
                    bcs = None
                    if norm_src != 'sbuf':
                        bcs = bcsb.tile([64, 256], dt.float32, tag="bcs",
                                        name="bcs")
                    if bcast_via == 'dma':
                        drec = drpool.tile([1, 256], dt.float32r, tag="dr",
                                           name="drec")
                        nc.sync.dma_start(out=drec, in_=rec)
                        dbc = bass.AP(tensor=drec.tensor, offset=drec.offset,
                                      ap=[[0, 64]] + drec.ap[1:])
                        nc.sync.dma_start(out=bcs.bitcast(dt.float32r), in_=dbc)
                    else:
                        bc = bcpool.tile([64, 256], dt.float32, tag="bc",
                                         name="bc")
                        nc.tensor.matmul(bc, ones_r[:, 0:64], rec)
                        if norm_src == 'sbuf':
                            bcs = bc
                        elif bccopy_eng == 'act':
                            nc.scalar.copy(bcs, bc)
                        else:
                            nc.vector.tensor_copy(bcs, bc)
                    with nc.allow_low_precision(reason="f32r"):
                        nc.vector.tensor_mul(
                            ctxT[pr][po:po + 64, cc * 256:(cc + 1) * 256],
                            ctx[0:64, :], bcs)
                    if h == NH_CORE - 1:
                        _dq.append(2 * cc)
                        _dq.append(2 * cc + 1)

                def emit_d(qt):
                    osb = opool.tile([128, 1024], dt.float32, tag="osb",
                                     name="osb")
                    for nn in range(2):
                        pD = popool.tile([128, 512], dt.float32, tag="po",
                                         name="pD")
                        for p in range(2):
                            nc.tensor.matmul(pD,
                                             ctxT[p][:, qt * 128:(qt + 1) * 128],
                                             wo[p][:, nn * 512:(nn + 1) * 512],
                                             start=(p == 0), stop=(p == 1))
                        if osbcopy_eng == 'act':
                            nc.scalar.copy(osb[:, nn * 512:(nn + 1) * 512], pD)
                        else:
                            nc.vector.tensor_copy(osb[:, nn * 512:(nn + 1) * 512], pD)
                    nc.gpsimd.dma_start(out=OUT[qt * 128:(qt + 1) * 128, :],
                                        in_=osb)

                from collections import deque
                pending = deque()
                _dq = deque()
                if fuse_b:
                    # prologue: cover key tiles for the first two query chunks
                    b_emitted += emit_b(6 * b_prol)
                step = 0
                for cc in range(NCC):
                    for h in range(NH_CORE):
                        if fuse_b:
                            # pace remaining B so chunk cc+2 is done before
                            # attention chunk cc+1 starts
                            target = min(b_total, 6 * (cc + b_lead))
                            want = target - b_emitted
                            per = max(1, (want + (NH_CORE - h) - 1)
                                      // (NH_CORE - h))
                            if want > 0:
                                b_emitted += emit_b(per)
                        pts = score_stage(h, cc)
                        pending.append((h, cc, pts))
                        if len(pending) > depth:
                            pv_stage(*pending.popleft())
                        step += 1
                while pending:
                    pv_stage(*pending.popleft())
                while _dq:
                    emit_d(_dq.popleft())
                if fuse_b:
                    b_emitted += emit_b(b_total)

            _bstack.close()

    nc.compile()
    return nc



def _prep_generic(x, Wqkv, bqkv, Wo, pm):
    in_maps = []
    xT_b = []
    for b in range(B):
        xt = np.ascontiguousarray(x[b].T)                      # [E, S]
        xT_b.append(np.ascontiguousarray(
            xt.reshape(8, 128, 16, 256).transpose(1, 2, 0, 3)))
    mv8_b = []
    for b in range(B):
        # mv8[p, t] = (0 if valid else NEG)/8 for key index t*128+p
        mv = np.where(pm[b], 0.0, NEG).astype(np.float32) / 8.0
        mv8_b.append(np.ascontiguousarray(mv.reshape(32, 128).T))

    for c in range(8):
        b, hg = c // 4, c % 4
        heads = range(4 * hg, 4 * hg + 4)
        qcols = np.concatenate([np.arange(h * 192, h * 192 + 64) for h in heads])
        kcols = qcols + 64
        vcols = qcols + 128
        wqk_cols = np.concatenate([qcols, kcols])               # [512]
        wqk = np.ascontiguousarray(
            Wqkv[:, wqk_cols].reshape(8, 128, 4, 128).transpose(1, 0, 2, 3))
        wv = np.ascontiguousarray(
            Wqkv[:, vcols].reshape(8, 128, 256).transpose(1, 0, 2))
        orows = np.concatenate([np.arange(h * 64, h * 64 + 64) for h in heads])
        wo = np.ascontiguousarray(Wo[orows].reshape(2, 128, 1024))
        in_maps.append({
            "xT": xT_b[b],
            "wqk": wqk,
            "wv": wv,
            "wo": wo,
            "bqk": np.ascontiguousarray(bqkv[wqk_cols].reshape(4, 128).T),
            "bv": np.ascontiguousarray(bqkv[vcols].reshape(1, 256)),
            "mv8": mv8_b[b],
        })
    return in_maps


def kernel(x, Wqkv, bqkv, Wo, bo, padding_mask, num_heads, window_size):
    assert int(num_heads) == H and int(window_size) == W
    x = np.asarray(x, dtype=np.float32)
    Wqkv = np.asarray(Wqkv, dtype=np.float32)
    bqkv = np.asarray(bqkv, dtype=np.float32)
    Wo = np.asarray(Wo, dtype=np.float32)
    bo = np.asarray(bo, dtype=np.float32)
    pm = np.asarray(padding_mask).astype(bool)
    assert x.shape == (B, S, E)

    fast = bool(pm.all()) and not np.any(bqkv)
    if fast:
        if "fast" not in _cache:
            _cache["fast"] = _build_fast(depth=3, tp_delay=1, d_delay=2,
                                         warm_n=8, osb_eng='sync',
                                         pd_pool='split', xq_eng='act',
                                         mask_eng='gpsimd', b_slack=2,
                                         b_prol=1, pv_first=2)
        nc = _cache["fast"]
        in_maps = _prep_fast(x, Wqkv, Wo)
    else:
        vbias = bool(np.any(bqkv.reshape(H, 3, HD)[:, 2, :] != 0.0))
        key = ("nc", vbias, False)
        if key not in _cache:
            _cache[key] = _build_generic(vbias=vbias, paired=False)
        nc = _cache[key]
        in_maps = _prep_generic(x, Wqkv, bqkv, Wo, pm)

    res = run_bass_kernel_spmd(nc, in_maps, list(range(8)))
    kernel._last_results = res

    out = np.empty((B, S, E), dtype=np.float32)
    for b in range(B):
        acc = res.results[4 * b]["out"].astype(np.float32)
        for g in range(1, 4):
            acc = acc + res.results[4 * b + g]["out"].astype(np.float32)
        out[b] = acc + bo
    return out



# revision 53
# speedup vs baseline: 1.1453x; 1.0458x over previous
"""Sliding-window multi-head attention (Longformer-style band attention) for
Trainium2, distributed over 8 NeuronCores.

Sharding: data-parallel over batch (B=2) x tensor-parallel over heads
(16 heads -> 4 groups of 4). Core c handles batch c//4, heads
[4*(c%4), 4*(c%4)+4). Each core computes the QKV projection for its head
group, band attention over 128-key tiles, and a partial output projection;
the host sums the 4 partials per batch and adds bo.

Fast path (all-ones padding mask, zero qkv bias): bf16 datapath end to end.
Scores are computed transposed ([key 128, query 128] tiles, 5 key tiles per
128-query block), exp'd on the scalar engine into bf16 probabilities,
triangular band masks applied on gpsimd, and PV accumulated as [query, 65]
with an appended ones column giving the softmax denominator for free.
Normalization is a per-partition reciprocal+scale on the vector engine; the
normalized context pair is PE-transposed and staged for the output
projection, which writes bf16 partials DMA'd from SBUF. QKV projection work
items are interleaved into the attention loop to keep the tensor engine
saturated, with scratch warm-up matmuls absorbing the PE clock ramp at
startup.

Generic path (padding masks / nonzero qkv bias) falls back to an f32r
implementation of the same blocking.
"""
import sys
import numpy as np
import ml_dtypes

try:
    import concourse.bass as bass
except ImportError:
    sys.path.insert(0, "/opt/trn_rl_repo")
    import concourse.bass as bass
import concourse.mybir as mybir
import concourse.tile as tile
from concourse import bacc
from concourse.bass_utils import run_bass_kernel_spmd

dt = mybir.dt
bf16 = ml_dtypes.bfloat16

B, S, E, H, W = 2, 4096, 1024, 16, 512
HD = E // H          # 64
NH_CORE = 4
w = W // 2           # 256
NT = S // 128        # 32 key tiles of 128
NQT = S // 128       # 32 query tiles of 128
NBC = S // 512       # 8 qkv token chunks of 512
NCC = S // 256       # generic path: 16 query chunks of 256
NEG = -9e15

_cache = {}


def _build_fast(depth=2, b_prol=2, mask_eng='dve', qkcopy_eng='dve',
                d_delay=1, cx_bufs=2, big_bufs=2, tp_delay=1, warm_n=0,
                osb_eng='gpsimd', pd_pool='big', xq_eng='sync', b_slack=8,
                d_hold=0, tp_pool='cx', pv_first=0, st_bufs=2,
                osb_copy='mixed', v_eng='dve', d_release=99, tail_q=99,
                tail_d=99, b_order='seq', b_cap=0):
    nc = bacc.Bacc("TRN2", target_bir_lowering=False, debug=False,
                   num_devices=8)

    # fp8 DoubleRow with host-side error compensation: x = x8 + dx8 and
    # W = W8 + dW8 (each fp8e4); three product chains x8W8 + x8dW8 + dx8W8
    # restore bf16-grade accuracy at 0.75x the bf16 PE cost (DoubleRow
    # contracts 256 rows per instruction at 0.5 cycles/row).
    # Layouts: [partition p, t (256-row ktile), s (main/residual), i
    # (DoubleRow pair), cols] with contraction index c = 256t + 128i + p.
    XT = nc.dram_tensor("xT", [8, 128, 4, 2, 2, 512], dt.float8e4,
                        kind="ExternalInput")
    WQK = nc.dram_tensor("wqk", [128, 4, 2, 2, 512], dt.float8e4,
                         kind="ExternalInput")
    WV = nc.dram_tensor("wv", [128, 4, 2, 2, 256], dt.float8e4,
                        kind="ExternalInput")
    WO = nc.dram_tensor("wo", [2, 128, 1024], dt.bfloat16,
                        kind="ExternalInput")
    OUT = nc.dram_tensor("out", [S, E], dt.bfloat16, kind="ExternalOutput")

    p_i = np.arange(128)[:, None]
    c_i = np.arange(128)[None, :]
    lo = (p_i >= c_i).astype(bf16)   # tile g==qt-2: valid kr >= qr
    up = (p_i <= c_i).astype(bf16)   # tile g==qt+2: valid kr <= qr
    MASKS = nc.inline_tensor(np.ascontiguousarray(
        np.stack([lo, up], axis=1)), name="trimasks")   # [128, 2, 128]
    IDENT = nc.inline_tensor(np.eye(128, dtype=bf16), name="ident")

    with tile.TileContext(nc) as tc:
        with tc.tile_pool(name="const", bufs=1) as cpool, \
             tc.tile_pool(name="qkTp", bufs=1) as qkpool, \
             tc.tile_pool(name="vaugp", bufs=1) as vpool, \
             tc.tile_pool(name="ctxTp", bufs=1) as ctpool, \
             tc.tile_pool(name="xq", bufs=4) as xqpool, \
             tc.tile_pool(name="pt", bufs=7) as ptpool, \
             tc.tile_pool(name="recp", bufs=4) as recpool, \
             tc.tile_pool(name="cnp", bufs=4) as cnpool, \
             tc.tile_pool(name="osbp", bufs=3) as opool, \
             tc.tile_pool(name="stp", bufs=st_bufs, space="PSUM") as sapool, \
             tc.tile_pool(name="cxp", bufs=cx_bufs, space="PSUM") as cxpool, \
             tc.tile_pool(name="bigp", bufs=big_bufs, space="PSUM") as bigpool:

            # ---- constants / weights ----
            wqk = cpool.tile([128, 4, 2, 2, 512], dt.float8e4)
            wv = cpool.tile([128, 4, 2, 2, 256], dt.float8e4)
            wo = cpool.tile([128, 2, 1024], dt.bfloat16)
            masks = cpool.tile([128, 2, 128], dt.bfloat16)
            ident = cpool.tile([128, 128], dt.bfloat16)
            # t-slice granularity so the first QKV matmuls start early
            # (subtile deps gate each accumulation step on its own slice);
            # scalar-engine HWDGE triggers: cheap and off the SP queue.
            # wqk/xq0 slices interleaved so slice pairs land together.
            xq0 = xqpool.tile([128, 4, 2, 2, 512], dt.float8e4, tag="xq",
                              name="xq")
            for kh in range(2):
                ks = slice(kh * 2, kh * 2 + 2)
                nc.scalar.dma_start(out=wqk[:, ks], in_=WQK[:, ks])
                nc.sync.dma_start(out=xq0[:, ks], in_=XT[0, :, ks])
            nc.scalar.dma_start(out=wv, in_=WV[:, :, :, :, :])
            nc.scalar.dma_start(out=masks, in_=MASKS[:, :, :])
            nc.scalar.dma_start(out=ident, in_=IDENT[:, :])
            nc.scalar.dma_start(out=wo[:, 0, :], in_=WO[0, :, :])
            nc.scalar.dma_start(out=wo[:, 1, :], in_=WO[1, :, :])

            # PE warmup: scratch matmuls absorb the p-state ramp while the
            # first input DMAs are still streaming in.
            if warm_n:
                wsrc = cpool.tile([128, 512], dt.bfloat16)
                nc.vector.memset(wsrc, 0.0)
                wdst = bigpool.tile([128, 512], dt.float32, tag="big",
                                    name="wdst")
                for i in range(warm_n):
                    nc.tensor.matmul(wdst, wsrc[:, 0:128], wsrc)

            # ---- persistent intermediates ----
            # q/k stored as fp8 DoubleRow slot pairs (scores run in fp8-DR
            # at half the bf16 PE cost). q: slot0 A = fp8(8q), slot1
            # B = fp8(8q - A); k: both slots fp8(-8k). The DR slot sum
            # k_n*A + k_n*B = k_n*8q cancels q's quantization error exactly;
            # only k's single-fp8 error remains. Score psum = -64*qk, undone
            # by a negative exp scale.
            qkT = [qkpool.tile([128, 2, S], dt.float8e4, name=f"qkT{cb}")
                   for cb in range(2)]          # q head pairs: (A, B) slots
            qkT += [qkpool.tile([128, S], dt.float8e4, name=f"qkT{cb}")
                    for cb in range(2, 4)]      # k: single fp8(-8k) copy
            vaug = vpool.tile([128, NT, NH_CORE, 65], dt.bfloat16)
            with nc.allow_low_precision(reason="ones col"):
                nc.vector.memset(vaug[:, :, :, 64], 1.0)
            ctxT = [ctpool.tile([128, S], dt.bfloat16, name=f"ctxT{p}")
                    for p in range(2)]

            # ---------------- phase B: QKV projection ----------------
            # (sw, sx) product chains: x8·W8 + dx8·W8 + x8·dW8, grouped so
            # the x8-only passes run first (dx8 streams in behind x8)
            PASSES = (((0, 0), (1, 0)), ((0, 1),))     # qk: (sw, sx)
            VPASSES = (((0, 0), (0, 1)), ((1, 0),))    # v: (sx, sw)
            DR = mybir.MatmulPerfMode.DoubleRow
            QS = 8.0 / WSCALE   # psum (q*WSCALE) -> stored 8q / -8k

            def qk_store(cb, sl, pg):
                with nc.allow_low_precision(reason="fp8 score operands"):
                    if cb < 2:   # q: slot A, then residual B = 8q - A
                        d0 = qkT[cb][:, 0, sl]
                        nc.vector.tensor_scalar_mul(d0, pg, QS)
                        nc.vector.ln_bwd_dx(qkT[cb][:, 1, sl], pg, d0,
                                            1.0 / QS, 0.0, scale=QS)
                    else:        # k: single fp8(-8k); matmul reads it twice
                        nc.vector.tensor_scalar_mul(qkT[cb][:, sl], pg, -QS)

            def make_xq(s0):
                xq = xqpool.tile([128, 4, 2, 2, 512], dt.float8e4, tag="xq",
                                 name="xq")
                xeng = nc.scalar if xq_eng == 'act' else nc.sync
                xeng.dma_start(out=xq, in_=XT[s0])
                return xq

            def b_items():
                pre = [xq0, make_xq(1)]
                for s0 in range(NBC):
                    xq = pre[0]
                    pre = pre[1:]
                    if s0 + 2 < NBC:
                        pre.append(make_xq(s0 + 2))  # prefetch 2 ahead

                    # chunk 0: two-pass accumulation so the first matmuls
                    # only need the first half of wqk/xq0 (still streaming)
                    if s0 == 0:
                        pgs = {}

                        def qk_half(cb, kh):
                            if kh == 0:
                                pgs[cb] = bigpool.tile(
                                    [128, 512], dt.float32, tag="big",
                                    name="pg")
                            pg = pgs[cb]
                            mm = kh * 6
                            for chains in PASSES:
                                for t in (kh * 2, kh * 2 + 1):
                                    for sw, sx in chains:
                                        nc.tensor.matmul(
                                            pg,
                                            wqk[:, t, sw, :,
                                                cb * 128:(cb + 1) * 128],
                                            xq[:, t, sx, :, :],
                                            start=(mm == 0), stop=(mm == 11),
                                            perf_mode=DR)
                                        mm += 1
                            if kh == 1:
                                qk_store(cb, slice(0, 512), pg)

                        # pairwise interleave: at most 2 open psum groups
                        # (ring=2), first items need only the first halves
                        for cb0 in (0, 2):
                            yield (lambda cb=cb0: qk_half(cb, 0))
                            yield (lambda cb=cb0 + 1: qk_half(cb, 0))
                            yield (lambda cb=cb0: qk_half(cb, 1))
                            yield (lambda cb=cb0 + 1: qk_half(cb, 1))

                        def v_item0(ts):
                            pv = bigpool.tile([128, 4, 64], dt.float32,
                                              tag="big", name="pv")
                            mm = 0
                            for chains in VPASSES:
                                for t in range(4):
                                    for sx, sw in chains:
                                        nc.tensor.matmul(
                                            pv,
                                            xq[:, t, sx, :,
                                               ts * 128:(ts + 1) * 128],
                                            wv[:, t, sw, :, :],
                                            start=(mm == 0), stop=(mm == 11),
                                            perf_mode=DR)
                                        mm += 1
                            veng = (nc.gpsimd if v_eng == 'pool'
                                    else nc.vector)
                            with nc.allow_low_precision(reason="bf16"):
                                veng.tensor_scalar_mul(
                                    vaug[:, ts, :, 0:64], pv, 1.0 / WSCALE)
                        for ts in range(4):
                            yield (lambda ts=ts: v_item0(ts))
                        continue

                    def qk_item(s0=s0, xq=xq, cb=0):
                        pg = bigpool.tile([128, 512], dt.float32, tag="big",
                                          name="pg")
                        mm = 0
                        for chains in PASSES:
                            for t in range(4):
                                for sw, sx in chains:
                                    nc.tensor.matmul(
                                        pg,
                                        wqk[:, t, sw, :,
                                            cb * 128:(cb + 1) * 128],
                                        xq[:, t, sx, :, :],
                                        start=(mm == 0), stop=(mm == 11),
                                        perf_mode=DR)
                                    mm += 1
                        qk_store(cb, slice(s0 * 512, (s0 + 1) * 512), pg)
                    def v_item(s0=s0, xq=xq, ts=0):
                        pv = bigpool.tile([128, 4, 64], dt.float32,
                                          tag="big", name="pv")
                        mm = 0
                        for chains in VPASSES:
                            for t in range(4):
                                for sx, sw in chains:
                                    nc.tensor.matmul(
                                        pv,
                                        xq[:, t, sx, :,
                                           ts * 128:(ts + 1) * 128],
                                        wv[:, t, sw, :, :],
                                        start=(mm == 0), stop=(mm == 11),
                                        perf_mode=DR)
                                    mm += 1
                        st = s0 * 4 + ts
                        veng = nc.gpsimd if v_eng == 'pool' else nc.vector
                        with nc.allow_low_precision(reason="bf16"):
                            veng.tensor_scalar_mul(
                                vaug[:, st, :, 0:64], pv, 1.0 / WSCALE)
                    # interleave qk/v items: v stores are one DVE op, so a
                    # (qk, v) cadence keeps the shared pg/pv psum ring from
                    # stalling on the 2-op qk store chain
                    if b_order == 'interleave':
                        for j in range(4):
                            yield (lambda s0=s0, xq=xq, cb=j:
                                   qk_item(s0, xq, cb))
                            yield (lambda s0=s0, xq=xq, ts=j:
                                   v_item(s0, xq, ts))
                    else:
                        for cb in range(4):
                            yield (lambda s0=s0, xq=xq, cb=cb:
                                   qk_item(s0, xq, cb))
                        for ts in range(4):
                            yield (lambda s0=s0, xq=xq, ts=ts:
                                   v_item(s0, xq, ts))

            b_gen = b_items()
            b_total = 12 + (NBC - 1) * 8   # chunk 0 split into 12 items
            b_emitted = 0

            def emit_b(n):
                done = 0
                for _ in range(n):
                    item = next(b_gen, None)
                    if item is None:
                        break
                    item()
                    done += 1
                return done

            # ---------------- phase C: band attention ----------------
            from collections import deque
            pending = deque()
            _dq = deque()
            _held = []

            cur_stp = [None]

            def score_unit(h, qt):
                pr, po = h // 2, (h % 2) * 64
                gs = [g for g in range(qt - 2, qt + 3) if 0 <= g < NT]
                nA = len(gs)
                stp = sapool.tile([128, 5, 128], dt.float32, tag="stp",
                                  name="stp")
                cur_stp[0] = stp
                for j in range(nA):
                    g = gs[j]
                    # stationary k read twice via a stride-0 slot dim: the
                    # DR slot sum k.(A+B) = k.8q cancels q's fp8 error
                    kap = qkT[2 + pr][po:po + 64, g * 128:(g + 1) * 128]
                    k2 = bass.AP(tensor=kap.tensor, offset=kap.offset,
                                 ap=[kap.ap[0], [0, 2]] + list(kap.ap[1:]))
                    nc.tensor.matmul(
                        stp[:, j, :], k2,
                        qkT[pr][po:po + 64, :, qt * 128:(qt + 1) * 128],
                        perf_mode=DR)
                ptA = ptpool.tile([128, 5, 128], dt.bfloat16, tag="pt",
                                  name="ptA")
                # psum holds -64*qk; negative scale restores exp(qk/8)
                nc.scalar.activation(ptA[:, 0:nA, :], stp[:, 0:nA, :],
                                     mybir.ActivationFunctionType.Exp,
                                     scale=-1.0 / (64.0 * np.sqrt(HD)))
                lo = gs[0] == qt - 2
                up = gs[-1] == qt + 2
                m_eng = 'dve1' if qt >= tail_q else mask_eng
                with nc.allow_low_precision(reason="bf16"):
                    if m_eng == 'dve1' and lo and up:
                        # both triangles in one strided op (slices 0 and 4)
                        nc.vector.tensor_mul(ptA[:, 0:5:4, :],
                                             ptA[:, 0:5:4, :], masks)
                    else:
                        meng = (nc.vector if m_eng in ('dve', 'dve1')
                                else nc.gpsimd)
                        if lo:
                            meng.tensor_mul(ptA[:, 0, :], ptA[:, 0, :],
                                            masks[:, 0, :])
                        if up:
                            meng.tensor_mul(ptA[:, nA - 1, :],
                                            ptA[:, nA - 1, :], masks[:, 1, :])
                return (gs, nA, ptA)

            cn_ref = [None, None]  # per parity: pending pair ctxn tile
            _tq = deque()          # deferred ctxT transpose: (h, qt, ctxn2)

            def pv_unit(h, qt, gs, nA, ptA):
                pr = h // 2
                ctx = cxpool.tile([128, 65], dt.float32, tag="cx",
                                  name="ctx")
                n = len(gs)
                # masked slices (0 and n-1) go last: their mask ops on the
                # mask engine get the longest lead time
                order = list(range(1, n - 1)) + [n - 1, 0] if n > 2 \
                    else list(range(n))
                for i, j in enumerate(order):
                    nc.tensor.matmul(ctx, ptA[:, j, :], vaug[:, gs[j], h, :],
                                     start=(i == 0), stop=(i == n - 1))
                rec = recpool.tile([128, 1], dt.float32, tag="rec",
                                   name="rec")
                nc.vector.reciprocal(rec, ctx[:, 64:65])
                if h % 2 == 0:
                    cn_ref[pr] = cnpool.tile([128, 2, 64], dt.bfloat16,
                                             tag="cn", name="ctxn2")
                ctxn2 = cn_ref[pr]
                with nc.allow_low_precision(reason="bf16"):
                    nc.vector.tensor_scalar_mul(ctxn2[:, h % 2, :],
                                                ctx[:, 0:64], rec)
                _tq.append((h, qt, ctxn2))

            def tp_unit(h, qt, ctxn2):
                # PE-transpose a head pair's normalized context in one shot:
                # ctxn2 [128 q, 128 pairdims] -> tp [128 pairdims, 128 q].
                # Scratch = slice 5 of the score tile in flight (never used
                # for scores), viewed as bf16.
                if h % 2 == 1:
                    pr = h // 2
                    if tp_pool == 'big':
                        tp = bigpool.tile([128, 128], dt.bfloat16,
                                          tag="big", name="tp")
                    else:
                        tp = cxpool.tile([128, 128], dt.bfloat16, tag="cx",
                                         name="tp")
                    nc.tensor.transpose(tp, ctxn2, ident)
                    with nc.allow_low_precision(reason="bf16"):
                        nc.vector.tensor_copy(
                            ctxT[pr][:, qt * 128:(qt + 1) * 128], tp)
                if h == NH_CORE - 1:
                    _dq.append(qt)

            def emit_d(qt, split_dma=False, tail=False):
                osb = opool.tile([128, 1024], dt.bfloat16, tag="osb",
                                 name="osb")
                deng = nc.gpsimd if osb_eng == 'gpsimd' else nc.sync
                for nn in range(2):
                    use_cx = (pd_pool == 'cx' or
                              (pd_pool == 'split' and nn == 0))
                    if use_cx:
                        pD = cxpool.tile([128, 512], dt.float32, tag="cx",
                                         name="pD")
                    else:
                        pD = bigpool.tile([128, 512], dt.float32, tag="big",
                                          name="pD")
                    for p in range(2):
                        nc.tensor.matmul(
                            pD, ctxT[p][:, qt * 128:(qt + 1) * 128],
                            wo[:, p, nn * 512:(nn + 1) * 512],
                            start=(p == 0), stop=(p == 1))
                    with nc.allow_low_precision(reason="bf16 partials"):
                        if osb_copy == 'pool':
                            nc.gpsimd.tensor_copy(
                                osb[:, nn * 512:(nn + 1) * 512], pD)
                        elif nn == 0 and not tail:
                            nc.scalar.copy(
                                osb[:, nn * 512:(nn + 1) * 512], pD)
                        else:
                            nc.vector.tensor_copy(
                                osb[:, nn * 512:(nn + 1) * 512], pD)
                    if split_dma:
                        deng.dma_start(
                            out=OUT[qt * 128:(qt + 1) * 128,
                                    nn * 512:(nn + 1) * 512],
                            in_=osb[:, nn * 512:(nn + 1) * 512])
                if not split_dma:
                    deng.dma_start(out=OUT[qt * 128:(qt + 1) * 128, :],
                                   in_=osb)

            # pacing: unit qt needs qkT/vaug through token (qt+2)*128+128,
            # i.e. chunks 0..ceil((qt*128+384)/512)-1 done.
            b_emitted += emit_b(8 * b_prol)
            for qt in range(NQT):
                # scores of qt need chunks covering tokens to (qt+3)*128-1,
                # i.e. chunks 0..(qt+2)//4 done; b_slack items of margin.
                need = min(b_total, 12 + 8 * ((qt + 2) // 4) + b_slack)
                # heads in order (0,2,1,3): staggers the two pair-chains
                for h in (0, 2, 1, 3):
                    dd = d_delay if qt < NQT - 2 else 0
                    while _dq and len(_dq) > dd:
                        dqt = _dq.popleft()
                        # park a few mid-sequence o-proj blocks: they become
                        # dependency-free PE work overlapping the final
                        # attention drain
                        if d_hold and len(_held) < d_hold and 16 <= dqt < 28:
                            _held.append(dqt)
                        else:
                            emit_d(dqt, split_dma=(dqt >= NQT - 2),
                                   tail=(dqt >= tail_d))
                    if d_hold and qt >= d_release and _held:
                        emit_d(_held.pop(0), split_dma=True, tail=True)
                    want = need - b_emitted
                    if want > 0:
                        per = max(1, (want + (NH_CORE - h) - 1)
                                  // (NH_CORE - h))
                        if b_cap:
                            per = min(per, b_cap)
                        b_emitted += emit_b(per)
                    if pv_first == 2 and len(_tq) > tp_delay:
                        tp_unit(*_tq.popleft())
                    if pv_first == 1 and len(pending) >= depth:
                        pv_unit(*pending.popleft())
                        pending.append((h, qt) + score_unit(h, qt))
                    else:
                        pending.append((h, qt) + score_unit(h, qt))
                        if len(pending) > depth:
                            pv_unit(*pending.popleft())
                    if pv_first != 2 and len(_tq) > tp_delay:
                        tp_unit(*_tq.popleft())
            while pending:
                pv_unit(*pending.popleft())
                if len(_tq) > 1:
                    tp_unit(*_tq.popleft())
            while _tq:
                tp_unit(*_tq.popleft())
                while _dq:
                    emit_d(_dq.popleft())
            b_emitted += emit_b(b_total)
            while _dq:
                emit_d(_dq.popleft(), split_dma=True)
            for dqt in _held:
                emit_d(dqt, split_dma=True)

    nc.compile()
    return nc


f8 = ml_dtypes.float8_e4m3


def _split8(a):
    """a (f32) -> (a8, da8) fp8e4 with a ~= a8 + da8 (compensated split)."""
    a8 = a.astype(f8)
    d8 = (a - a8.astype(np.float32)).astype(f8)
    return a8, d8


WSCALE = 128.0  # lifts W (and its residual) out of e4m3's subnormal range


def _pack_w8(wcols, ncol):
    """[1024, ncol] f32 -> [128, 4t, 2s, 2i, ncol] fp8 with contraction
    index c = 256t + 128i + p. Weights are pre-scaled by WSCALE; the
    psum->sbuf copy divides it back out."""
    w8, dw8 = _split8(wcols * WSCALE)
    ws = np.stack([w8, dw8])                     # [s, 1024, ncol]
    ws = ws.reshape(2, 4, 2, 128, ncol)          # [s, t, i, p, col]
    return np.ascontiguousarray(ws.transpose(3, 1, 0, 2, 4))


def _prep_fast(x, Wqkv, Wo):
    """Per-core input maps (compensated fp8 QKV operands, bf16 Wo)."""
    xT_b = []
    for b in range(B):
        xt = np.ascontiguousarray(x[b].T)              # [E, S] f32
        x8, dx8 = _split8(xt)
        xs = np.stack([x8, dx8])                       # [s, E, S]
        xs = xs.reshape(2, 4, 2, 128, 8, 512)          # [s, t, i, p, s0, tok]
        xT_b.append(np.ascontiguousarray(xs.transpose(4, 3, 1, 0, 2, 5)))
        # xT_b[b][s0, p, t, s, i, tok] = xs[s, 256t+128i+p, 512*s0+tok]
    in_maps = []
    for c in range(8):
        b, hg = c // 4, c % 4
        heads = range(4 * hg, 4 * hg + 4)
        qcols = np.concatenate([np.arange(h * 192, h * 192 + 64)
                                for h in heads])
        kcols = qcols + 64
        vcols = qcols + 128
        wqk_cols = np.concatenate([qcols, kcols])           # [512]
        wqk = _pack_w8(Wqkv[:, wqk_cols], 512)
        wv = _pack_w8(Wqkv[:, vcols], 256)
        orows = np.concatenate([np.arange(h * 64, h * 64 + 64)
                                for h in heads])
        wo = np.ascontiguousarray(Wo[orows].reshape(2, 128, 1024)).astype(bf16)
        in_maps.append({"xT": xT_b[b], "wqk": wqk, "wv": wv, "wo": wo})
    return in_maps



def _build_generic(vbias=True, st_bufs=2, po_bufs=1, bc_bufs=1, cx_bufs=2,
           mask_eng='dve', bccopy_eng='act', pt_bufs=8,
           osbcopy_eng='dve', bcast_via='pe', paired=True, depth=1,
           fuse_b=True, pb_bufs=2, b_lead=3, b_prol=2,
           norm_src='sbuf', ctxcopy_eng='act'):
    if fuse_b:
        pt_bufs = min(pt_bufs, 6)
    _nb = 2 if fuse_b else 3
    nc = bacc.Bacc("TRN2", target_bir_lowering=False, debug=False, num_devices=8)

    XT = nc.dram_tensor("xT", [128, 16, 8, 256], dt.float32r, kind="ExternalInput")
    WQK = nc.dram_tensor("wqk", [128, 8, 4, 128], dt.float32r, kind="ExternalInput")
    WV = nc.dram_tensor("wv", [128, 8, 256], dt.float32r, kind="ExternalInput")
    WO = nc.dram_tensor("wo", [2, 128, 1024], dt.float32r, kind="ExternalInput")
    BQK = nc.dram_tensor("bqk", [128, 4], dt.float32, kind="ExternalInput")
    BV = nc.dram_tensor("bv", [1, 256], dt.float32, kind="ExternalInput")
    MV8 = nc.dram_tensor("mv8", [128, 32], dt.float32, kind="ExternalInput")
    OUT = nc.dram_tensor("out", [S, E], dt.float32, kind="ExternalOutput")

    # constant 0/1 triangular band masks for u in {-2,-1,2,3}
    p_i = np.arange(128)[:, None]
    r_i = np.arange(256)[None, :]
    mask_np = {}
    for u in (-2, -1, 2, 3):
        mask_np[u] = ((u * 128 + p_i - r_i >= -w) & (u * 128 + p_i - r_i <= w)
                      ).astype(np.float32)
    MASKS = nc.inline_tensor(
        np.ascontiguousarray(
            np.stack([mask_np[u] for u in (-2, -1, 2, 3)]).transpose(1, 0, 2)),
        name="trimasks")
    ONES = nc.inline_tensor(np.ones((1, 128), dtype=np.float32), name="onesrow")

    with tile.TileContext(nc) as tc:
        with tc.tile_pool(name="const", bufs=1) as cpool, \
             tc.tile_pool(name="qkT", bufs=1) as qkpool, \
             tc.tile_pool(name="vaug", bufs=1) as vpool, \
             tc.tile_pool(name="ctxT", bufs=1) as ctxpool:

            wo = [cpool.tile([128, 1024], dt.float32r, name=f"wo{p}") for p in range(2)]
            bqk = cpool.tile([128, 4], dt.float32)
            nc.gpsimd.dma_start(out=bqk, in_=BQK[:, :])
            bv_f = cpool.tile([1, 256], dt.float32)
            nc.gpsimd.dma_start(out=bv_f, in_=BV[:, :])
            mv8 = cpool.tile([128, 32], dt.float32)
            nc.gpsimd.dma_start(out=mv8, in_=MV8[:, :])
            masks = cpool.tile([128, 4, 256], dt.float32)
            mask_idx = {-2: 0, -1: 1, 2: 2, 3: 3}
            ones_f = cpool.tile([1, 128], dt.float32)
            nc.gpsimd.dma_start(out=ones_f, in_=ONES[:, :])
            ones_r = cpool.tile([1, 128], dt.float32r)
            bv_r = cpool.tile([1, 256], dt.float32r)
            with nc.allow_low_precision(reason="f32r matmul pipeline"):
                nc.vector.tensor_copy(ones_r, ones_f)
                nc.vector.tensor_copy(bv_r, bv_f)

            # persistent intermediates
            qkT = [qkpool.tile([128, S], dt.float32r, name=f"qkT{cb}")
                   for cb in range(4)]  # 0,1: q pairs; 2,3: k pairs
            vaug = [vpool.tile([128, NT, 65], dt.float32r, name=f"vaug{h}")
                    for h in range(NH_CORE)]
            ones32 = cpool.tile([128, NT], dt.float32)
            nc.vector.memset(ones32, 1.0)
            for h in range(NH_CORE):
                with nc.allow_low_precision(reason="f32r"):
                    nc.vector.tensor_copy(vaug[h][:, :, 64], ones32)
            ctxT = [ctxpool.tile([128, S], dt.float32r, name=f"ctxT{p}")
                    for p in range(2)]

            # ---------------- Phase B: QKV projection ----------------
            # Emitted either up front (fuse_b=False) or as fine-grained work
            # items interleaved into the attention loop's idle PE slots.
            bwpool = ctx_pools = None
            import contextlib
            _bstack = contextlib.ExitStack()
            bwpool = _bstack.enter_context(tc.tile_pool(name="bw", bufs=1))
            xqpool = _bstack.enter_context(
                tc.tile_pool(name="xq", bufs=(2 if fuse_b else 3)))
            pbpool = _bstack.enter_context(
                tc.tile_pool(name="pb", bufs=(pb_bufs if fuse_b else 8),
                             space="PSUM"))
            wqk = bwpool.tile([128, 8, 4, 128], dt.float32r)
            wv = bwpool.tile([128, 8, 256], dt.float32r)
            xq0 = [xqpool.tile([128, 4, 256], dt.float32r, tag=f"xq{i}",
                               name="xq") for i in range(2)]
            for i in range(2):
                nc.sync.dma_start(out=xq0[i], in_=XT[:, 0, i * 4:(i + 1) * 4, :])
            for kt in range(8):
                nc.sync.dma_start(out=wqk[:, kt, :, :], in_=WQK[:, kt, :, :])
            nc.sync.dma_start(out=wv[:, 0:4, :], in_=WV[:, 0:4, :])
            nc.sync.dma_start(out=wv[:, 4:8, :], in_=WV[:, 4:8, :])

            def b_items():
                for s0 in range(16):  # 256-token chunks of S
                    if s0 == 0:
                        xq = xq0
                    else:
                        xq = [xqpool.tile([128, 4, 256], dt.float32r,
                                          tag=f"xq{i}", name="xq")
                              for i in range(2)]
                        for i in range(2):
                            nc.sync.dma_start(
                                out=xq[i], in_=XT[:, s0, i * 4:(i + 1) * 4, :])

                    def qk_item(s0=s0, xq=xq, cb=0):
                        pg = pbpool.tile([128, 256], dt.float32, tag="pb",
                                         name="pqk")
                        for k8 in range(8):
                            nc.tensor.matmul(pg, wqk[:, k8, cb, :],
                                             xq[k8 // 4][:, k8 % 4, :],
                                             start=(k8 == 0), stop=(k8 == 7))
                        nc.scalar.activation(
                            qkT[cb][:, s0 * 256:(s0 + 1) * 256], pg,
                            mybir.ActivationFunctionType.Identity,
                            bias=bqk[:, cb:cb + 1])
                    for cb in range(4):
                        yield (lambda s0=s0, xq=xq, cb=cb: qk_item(s0, xq, cb))

                    def v_item(s0=s0, xq=xq, hf=0):
                        pv = pbpool.tile([128, 256], dt.float32, tag="pb",
                                         name="pv")
                        for k8 in range(8):
                            nc.tensor.matmul(
                                pv,
                                xq[k8 // 4][:, k8 % 4, hf * 128:(hf + 1) * 128],
                                wv[:, k8, :], start=(k8 == 0),
                                stop=(k8 == 7 and not vbias))
                        if vbias:
                            nc.tensor.matmul(pv, ones_r, bv_r,
                                             start=False, stop=True)
                        st = s0 * 2 + hf
                        for h in range(NH_CORE):
                            with nc.allow_low_precision(reason="f32r"):
                                nc.vector.tensor_copy(
                                    vaug[h][:, st, 0:64],
                                    pv[:, h * 64:(h + 1) * 64])
                    for hf in range(2):
                        yield (lambda s0=s0, xq=xq, hf=hf: v_item(s0, xq, hf))

            b_gen = b_items()
            b_total = 16 * 6
            b_emitted = 0

            def emit_b(n):
                emitted = 0
                for _ in range(n):
                    item = next(b_gen, None)
                    if item is None:
                        break
                    item()
                    emitted += 1
                return emitted

            if not fuse_b:
                b_emitted += emit_b(b_total)
                _bstack.close()

            nc.gpsimd.dma_start(out=masks, in_=MASKS[:, :, :])
            for p in range(2):
                nc.gpsimd.dma_start(out=wo[p], in_=WO[p, :, :])
            # ------- Phase C: band attention, with output projection folded in -------
            import contextlib
            _cstack = contextlib.ExitStack()
            with _cstack:
                stpool = _cstack.enter_context(
                    tc.tile_pool(name="stp", bufs=st_bufs, space="PSUM"))
                cxpool = _cstack.enter_context(
                    tc.tile_pool(name="ctxp", bufs=cx_bufs, space="PSUM"))
                if bcast_via == 'pe':
                    bcpool = _cstack.enter_context(
                        tc.tile_pool(name="bcp", bufs=bc_bufs, space="PSUM"))
                else:
                    drpool = _cstack.enter_context(
                        tc.tile_pool(name="dr", bufs=4, space="DRAM"))
                popool = _cstack.enter_context(
                    tc.tile_pool(name="po", bufs=po_bufs, space="PSUM"))
                ptpool = _cstack.enter_context(
                    tc.tile_pool(name="pt", bufs=pt_bufs))
                bcsb = _cstack.enter_context(tc.tile_pool(name="bcs", bufs=_nb))
                opool = _cstack.enter_context(tc.tile_pool(name="osb", bufs=2))
                rcpool = _cstack.enter_context(tc.tile_pool(name="rcp", bufs=_nb))

                def score_stage(h, cc):
                    # returns list of (gts, pt, jslices) where pt holds exp'd
                    # probabilities for the key tiles in gts
                    pr, po = h // 2, (h % 2) * 64
                    out = []
                    if paired:
                        # all-ones padding: exp has no per-key bias, so key
                        # tiles are processed in aligned pairs (one psum bank,
                        # one exp, one mask-mul per pair)
                        for ub in (-2, 0, 2):
                            gts = [2 * cc + ub, 2 * cc + ub + 1]
                            if gts[0] < 0 or gts[1] >= NT:
                                continue
                            stp = stpool.tile([128, 2, 256], dt.float32,
                                              tag="st", name="stp")
                            for j, gt in enumerate(gts):
                                nc.tensor.matmul(
                                    stp[:, j, :],
                                    qkT[2 + pr][po:po + 64,
                                                gt * 128:(gt + 1) * 128],
                                    qkT[pr][po:po + 64,
                                            cc * 256:(cc + 1) * 256])
                            pt = ptpool.tile([128, 2, 256], dt.float32r,
                                             tag="pt", name="pt")
                            nc.scalar.activation(
                                pt, stp, mybir.ActivationFunctionType.Exp,
                                scale=1.0 / np.sqrt(HD))
                            if ub != 0:
                                mi = 0 if ub == -2 else 2
                                with nc.allow_low_precision(reason="f32r"):
                                    eng = (nc.gpsimd if mask_eng == 'gpsimd'
                                           else nc.vector)
                                    eng.tensor_mul(pt, pt,
                                                   masks[:, mi:mi + 2, :])
                            out.append((gts, pt))
                        return out
                    for u in range(-2, 4):
                        gt = 2 * cc + u
                        if not 0 <= gt < NT:
                            continue
                        stp = stpool.tile([128, 256], dt.float32, tag="st",
                                          name="stp")
                        nc.tensor.matmul(
                            stp,
                            qkT[2 + pr][po:po + 64, gt * 128:(gt + 1) * 128],
                            qkT[pr][po:po + 64, cc * 256:(cc + 1) * 256])
                        pt = ptpool.tile([128, 256], dt.float32r, tag="pt",
                                         name="pt")
                        nc.scalar.activation(pt, stp,
                                             mybir.ActivationFunctionType.Exp,
                                             bias=mv8[:, gt:gt + 1],
                                             scale=1.0 / np.sqrt(HD))
                        if u in mask_idx:
                            with nc.allow_low_precision(reason="f32r"):
                                eng = (nc.gpsimd if mask_eng == 'gpsimd'
                                       else nc.vector)
                                eng.tensor_mul(pt, pt,
                                               masks[:, mask_idx[u], :])
                        out.append(([gt], pt))
                    return out

                def pv_stage(h, cc, pts):
                    if _dq:
                        emit_d(_dq.popleft())
                    pr, po = h // 2, (h % 2) * 64
                    ctx = cxpool.tile([65, 256], dt.float32, tag="cx",
                                      name="ctx")
                    nmm = sum(len(gts) for gts, _ in pts)
                    j = 0
                    for gts, pt in pts:
                        for jj, gt in enumerate(gts):
                            rhs = pt[:, jj, :] if len(gts) > 1 else pt
                            nc.tensor.matmul(ctx, vaug[h][:, gt, :], rhs,
                                             start=(j == 0),
                                             stop=(j == nmm - 1))
                            j += 1
                    if norm_src == 'sbuf':
                        # copy ctx out of PSUM first: frees the cx slot early
                        # and the final multiply reads bc straight from PSUM
                        cxs = bcsb.tile([65, 256], dt.float32, tag="bcs",
                                        name="cxs")
                        if ctxcopy_eng == 'act':
                            nc.scalar.copy(cxs, ctx)
                        else:
                            nc.vector.tensor_copy(cxs, ctx)
                        ctx = cxs
                    rec = rcpool.tile([1, 256], dt.float32r, tag="rc",
                                      name="rec")
                    with nc.allow_low_precision(reason="f32r"):
                        nc.vector.reciprocal(rec, ctx[64:65, :])
                    bcs = None
                    if norm_src != 'sbuf':
                        bcs = bcsb.tile([64, 256], dt.float32, tag="bcs",
                                        name="bcs")
                    if bcast_via == 'dma':
                        drec = drpool.tile([1, 256], dt.float32r, tag="dr",
                                           name="drec")
                        nc.sync.dma_start(out=drec, in_=rec)
                        dbc = bass.AP(tensor=drec.tensor, offset=drec.offset,
                                      ap=[[0, 64]] + drec.ap[1:])
                        nc.sync.dma_start(out=bcs.bitcast(dt.float32r), in_=dbc)
                    else:
                        bc = bcpool.tile([64, 256], dt.float32, tag="bc",
                                         name="bc")
                        nc.tensor.matmul(bc, ones_r[:, 0:64], rec)
                        if norm_src == 'sbuf':
                            bcs = bc
                        elif bccopy_eng == 'act':
                            nc.scalar.copy(bcs, bc)
                        else:
                            nc.vector.tensor_copy(bcs, bc)
                    with nc.allow_low_precision(reason="f32r"):
                        nc.vector.tensor_mul(
                            ctxT[pr][po:po + 64, cc * 256:(cc + 1) * 256],
                            ctx[0:64, :], bcs)
                    if h == NH_CORE - 1:
                        _dq.append(2 * cc)
                        _dq.append(2 * cc + 1)

                def emit_d(qt):
                    osb = opool.tile([128, 1024], dt.float32, tag="osb",
                                     name="osb")
                    for nn in range(2):
                        pD = popool.tile([128, 512], dt.float32, tag="po",
                                         name="pD")
                        for p in range(2):
                            nc.tensor.matmul(pD,
                                             ctxT[p][:, qt * 128:(qt + 1) * 128],
                                             wo[p][:, nn * 512:(nn + 1) * 512],
                                             start=(p == 0), stop=(p == 1))
                        if osbcopy_eng == 'act':
                            nc.scalar.copy(osb[:, nn * 512:(nn + 1) * 512], pD)
                        else:
                            nc.vector.tensor_copy(osb[:, nn * 512:(nn + 1) * 512], pD)
                    nc.gpsimd.dma_start(out=OUT[qt * 128:(qt + 1) * 128, :],
                                        in_=osb)

                from collections import deque
                pending = deque()
                _dq = deque()
                if fuse_b:
                    # prologue: cover key tiles for the first two query chunks
                    b_emitted += emit_b(6 * b_prol)
                step = 0
                for cc in range(NCC):
                    for h in range(NH_CORE):
                        if fuse_b:
                            # pace remaining B so chunk cc+2 is done before
                            # attention chunk cc+1 starts
                            target = min(b_total, 6 * (cc + b_lead))
                            want = target - b_emitted
                            per = max(1, (want + (NH_CORE - h) - 1)
                                      // (NH_CORE - h))
                            if want > 0:
                                b_emitted += emit_b(per)
                        pts = score_stage(h, cc)
                        pending.append((h, cc, pts))
                        if len(pending) > depth:
                            pv_stage(*pending.popleft())
                        step += 1
                while pending:
                    pv_stage(*pending.popleft())
                while _dq:
                    emit_d(_dq.popleft())
                if fuse_b:
                    b_emitted += emit_b(b_total)

            _bstack.close()

    nc.compile()
    return nc



def _prep_generic(x, Wqkv, bqkv, Wo, pm):
    in_maps = []
    xT_b = []
    for b in range(B):
        xt = np.ascontiguousarray(x[b].T)                      # [E, S]
        xT_b.append(np.ascontiguousarray(
            xt.reshape(8, 128, 16, 256).transpose(1, 2, 0, 3)))
    mv8_b = []
    for b in range(B):
        # mv8[p, t] = (0 if valid else NEG)/8 for key index t*128+p
        mv = np.where(pm[b], 0.0, NEG).astype(np.float32) / 8.0
        mv8_b.append(np.ascontiguousarray(mv.reshape(32, 128).T))

    for c in range(8):
        b, hg = c // 4, c % 4
        heads = range(4 * hg, 4 * hg + 4)
        qcols = np.concatenate([np.arange(h * 192, h * 192 + 64) for h in heads])
        kcols = qcols + 64
        vcols = qcols + 128
        wqk_cols = np.concatenate([qcols, kcols])               # [512]
        wqk = np.ascontiguousarray(
            Wqkv[:, wqk_cols].reshape(8, 128, 4, 128).transpose(1, 0, 2, 3))
        wv = np.ascontiguousarray(
            Wqkv[:, vcols].reshape(8, 128, 256).transpose(1, 0, 2))
        orows = np.concatenate([np.arange(h * 64, h * 64 + 64) for h in heads])
        wo = np.ascontiguousarray(Wo[orows].reshape(2, 128, 1024))
        in_maps.append({
            "xT": xT_b[b],
            "wqk": wqk,
            "wv": wv,
            "wo": wo,
            "bqk": np.ascontiguousarray(bqkv[wqk_cols].reshape(4, 128).T),
            "bv": np.ascontiguousarray(bqkv[vcols].reshape(1, 256)),
            "mv8": mv8_b[b],
        })
    return in_maps


def kernel(x, Wqkv, bqkv, Wo, bo, padding_mask, num_heads, window_size):
    assert int(num_heads) == H and int(window_size) == W
    x = np.asarray(x, dtype=np.float32)
    Wqkv = np.asarray(Wqkv, dtype=np.float32)
    bqkv = np.asarray(bqkv, dtype=np.float32)
    Wo = np.asarray(Wo, dtype=np.float32)
    bo = np.asarray(bo, dtype=np.float32)
    pm = np.asarray(padding_mask).astype(bool)
    assert x.shape == (B, S, E)

    fast = bool(pm.all()) and not np.any(bqkv)
    if fast:
        if "fast" not in _cache:
            _cache["fast"] = _build_fast(depth=3, tp_delay=1, d_delay=2,
                                         warm_n=8, osb_eng='sync',
                                         pd_pool='split', xq_eng='sync',
                                         mask_eng='gpsimd', b_slack=2,
                                         b_prol=1, pv_first=2)
        nc = _cache["fast"]
        in_maps = _prep_fast(x, Wqkv, Wo)
    else:
        vbias = bool(np.any(bqkv.reshape(H, 3, HD)[:, 2, :] != 0.0))
        key = ("nc", vbias, False)
        if key not in _cache:
            _cache[key] = _build_generic(vbias=vbias, paired=False)
        nc = _cache[key]
        in_maps = _prep_generic(x, Wqkv, bqkv, Wo, pm)

    res = run_bass_kernel_spmd(nc, in_maps, list(range(8)))
    kernel._last_results = res

    out = np.empty((B, S, E), dtype=np.float32)
    for b in range(B):
        acc = res.results[4 * b]["out"].astype(np.float32)
        for g in range(1, 4):
            acc = acc + res.results[4 * b + g]["out"].astype(np.float32)
        out[b] = acc + bo
    return out



# revision 56
# speedup vs baseline: 1.1457x; 1.0004x over previous
"""Sliding-window multi-head attention (Longformer-style band attention) for
Trainium2, distributed over 8 NeuronCores.

Sharding: data-parallel over batch (B=2) x tensor-parallel over heads
(16 heads -> 4 groups of 4). Core c handles batch c//4, heads
[4*(c%4), 4*(c%4)+4). Each core computes the QKV projection for its head
group, band attention over 128-key tiles, and a partial output projection;
the host sums the 4 partials per batch and adds bo.

Fast path (all-ones padding mask, zero qkv bias): mixed fp8/bf16 datapath
tuned against the TRN2 cost model, where fp8e4 DoubleRow matmuls process two
128-row contraction slices per instruction at 0.5 cycles/row (4x bf16 for
deep contractions).

- QKV projection: error-compensated fp8 DoubleRow. Host splits x = x8 + dx8
  and W*128 = W8 + dW8 (the *128 lifts W and its residual out of e4m3's
  subnormal range; the psum->sbuf copy divides it back). Three product
  chains x8W8 + dx8W8 + x8dW8 run at 0.75x the bf16 PE cost with bf16-grade
  accuracy.
- Scores: half-compensated fp8 DoubleRow at 0.5x bf16 PE cost. q is stored
  as slot pair A = fp8(8q), B = fp8(8q - A) (one tensor_scalar + one
  LN_BWD_DX custom-DVE op); k as a single fp8(-8k) copy that the matmul's
  stationary operand reads twice via a stride-0 slot dim. The DoubleRow
  slot sum k.(A+B) = k.8q cancels q's quantization error exactly; only k's
  single-fp8 error (~1.3% end to end) remains. Score psum = -64*qk, undone
  by a negative exp scale.
- exp on the scalar engine into bf16 probabilities, triangular band masks
  on gpsimd (vector in the post-QKV tail, where gpsimd saturates), PV in
  bf16 as [query, 65] with an appended ones column giving the softmax
  denominator for free. PV/o-proj stay bf16: with only ~2e-2 error budget,
  single-sided fp8 fails there and full compensation costs the same as
  bf16.
- Normalization, PE-transpose and the bf16 output projection as before;
  QKV work items are paced into the attention loop to keep PE saturated,
  warm-up matmuls absorb the PE clock ramp.

Generic path (padding masks / nonzero qkv bias) falls back to an f32r
implementation of the same blocking.
"""
import sys
import numpy as np
import ml_dtypes

try:
    import concourse.bass as bass
except ImportError:
    sys.path.insert(0, "/opt/trn_rl_repo")
    import concourse.bass as bass
import concourse.mybir as mybir
import concourse.tile as tile
from concourse import bacc
from concourse.bass_utils import run_bass_kernel_spmd

dt = mybir.dt
bf16 = ml_dtypes.bfloat16

B, S, E, H, W = 2, 4096, 1024, 16, 512
HD = E // H          # 64
NH_CORE = 4
w = W // 2           # 256
NT = S // 128        # 32 key tiles of 128
NQT = S // 128       # 32 query tiles of 128
NBC = S // 512       # 8 qkv token chunks of 512
NCC = S // 256       # generic path: 16 query chunks of 256
NEG = -9e15

_cache = {}


def _build_fast(depth=2, b_prol=2, mask_eng='dve', qkcopy_eng='dve',
                d_delay=1, cx_bufs=2, big_bufs=2, tp_delay=1, warm_n=0,
                osb_eng='gpsimd', pd_pool='big', xq_eng='sync', b_slack=8,
                d_hold=0, tp_pool='cx', pv_first=0, st_bufs=2,
                osb_copy='mixed', v_eng='dve', d_release=99, tail_q=99,
                tail_d=99, b_order='seq', b_cap=0):
    nc = bacc.Bacc("TRN2", target_bir_lowering=False, debug=False,
                   num_devices=8)

    # fp8 DoubleRow with host-side error compensation: x = x8 + dx8 and
    # W = W8 + dW8 (each fp8e4); three product chains x8W8 + x8dW8 + dx8W8
    # restore bf16-grade accuracy at 0.75x the bf16 PE cost (DoubleRow
    # contracts 256 rows per instruction at 0.5 cycles/row).
    # Layouts: [partition p, t (256-row ktile), s (main/residual), i
    # (DoubleRow pair), cols] with contraction index c = 256t + 128i + p.
    XT = nc.dram_tensor("xT", [8, 128, 4, 2, 2, 512], dt.float8e4,
                        kind="ExternalInput")
    WQK = nc.dram_tensor("wqk", [128, 4, 2, 2, 512], dt.float8e4,
                         kind="ExternalInput")
    WV = nc.dram_tensor("wv", [128, 4, 2, 2, 256], dt.float8e4,
                        kind="ExternalInput")
    WO = nc.dram_tensor("wo", [2, 128, 1024], dt.bfloat16,
                        kind="ExternalInput")
    OUT = nc.dram_tensor("out", [S, E], dt.bfloat16, kind="ExternalOutput")

    p_i = np.arange(128)[:, None]
    c_i = np.arange(128)[None, :]
    lo = (p_i >= c_i).astype(bf16)   # tile g==qt-2: valid kr >= qr
    up = (p_i <= c_i).astype(bf16)   # tile g==qt+2: valid kr <= qr
    MASKS = nc.inline_tensor(np.ascontiguousarray(
        np.stack([lo, up], axis=1)), name="trimasks")   # [128, 2, 128]
    IDENT = nc.inline_tensor(np.eye(128, dtype=bf16), name="ident")

    with tile.TileContext(nc) as tc:
        with tc.tile_pool(name="const", bufs=1) as cpool, \
             tc.tile_pool(name="qkTp", bufs=1) as qkpool, \
             tc.tile_pool(name="vaugp", bufs=1) as vpool, \
             tc.tile_pool(name="ctxTp", bufs=1) as ctpool, \
             tc.tile_pool(name="xq", bufs=4) as xqpool, \
             tc.tile_pool(name="pt", bufs=7) as ptpool, \
             tc.tile_pool(name="recp", bufs=4) as recpool, \
             tc.tile_pool(name="cnp", bufs=4) as cnpool, \
             tc.tile_pool(name="osbp", bufs=3) as opool, \
             tc.tile_pool(name="stp", bufs=st_bufs, space="PSUM") as sapool, \
             tc.tile_pool(name="cxp", bufs=cx_bufs, space="PSUM") as cxpool, \
             tc.tile_pool(name="bigp", bufs=big_bufs, space="PSUM") as bigpool:

            # ---- constants / weights ----
            wqk = cpool.tile([128, 4, 2, 2, 512], dt.float8e4)
            wv = cpool.tile([128, 4, 2, 2, 256], dt.float8e4)
            wo = cpool.tile([128, 2, 1024], dt.bfloat16)
            masks = cpool.tile([128, 2, 128], dt.bfloat16)
            ident = cpool.tile([128, 128], dt.bfloat16)
            # t-slice granularity so the first QKV matmuls start early
            # (subtile deps gate each accumulation step on its own slice);
            # scalar-engine HWDGE triggers: cheap and off the SP queue.
            # wqk/xq0 slices interleaved so slice pairs land together.
            xq0 = xqpool.tile([128, 4, 2, 2, 512], dt.float8e4, tag="xq",
                              name="xq")
            for kh in range(2):
                ks = slice(kh * 2, kh * 2 + 2)
                nc.scalar.dma_start(out=wqk[:, ks], in_=WQK[:, ks])
                nc.sync.dma_start(out=xq0[:, ks], in_=XT[0, :, ks])
            nc.scalar.dma_start(out=wv, in_=WV[:, :, :, :, :])
            nc.scalar.dma_start(out=masks, in_=MASKS[:, :, :])
            nc.scalar.dma_start(out=ident, in_=IDENT[:, :])
            nc.scalar.dma_start(out=wo[:, 0, :], in_=WO[0, :, :])
            nc.scalar.dma_start(out=wo[:, 1, :], in_=WO[1, :, :])

            # PE warmup: scratch matmuls absorb the p-state ramp while the
            # first input DMAs are still streaming in.
            if warm_n:
                wsrc = cpool.tile([128, 512], dt.bfloat16)
                nc.vector.memset(wsrc, 0.0)
                wdst = bigpool.tile([128, 512], dt.float32, tag="big",
                                    name="wdst")
                for i in range(warm_n):
                    nc.tensor.matmul(wdst, wsrc[:, 0:128], wsrc)

            # ---- persistent intermediates ----
            # q/k stored as fp8 DoubleRow slot pairs (scores run in fp8-DR
            # at half the bf16 PE cost). q: slot0 A = fp8(8q), slot1
            # B = fp8(8q - A); k: both slots fp8(-8k). The DR slot sum
            # k_n*A + k_n*B = k_n*8q cancels q's quantization error exactly;
            # only k's single-fp8 error remains. Score psum = -64*qk, undone
            # by a negative exp scale.
            qkT = [qkpool.tile([128, 2, S], dt.float8e4, name=f"qkT{cb}")
                   for cb in range(2)]          # q head pairs: (A, B) slots
            qkT += [qkpool.tile([128, S], dt.float8e4, name=f"qkT{cb}")
                    for cb in range(2, 4)]      # k: single fp8(-8k) copy
            vaug = vpool.tile([128, NT, NH_CORE, 65], dt.bfloat16)
            with nc.allow_low_precision(reason="ones col"):
                nc.vector.memset(vaug[:, :, :, 64], 1.0)
            ctxT = [ctpool.tile([128, S], dt.bfloat16, name=f"ctxT{p}")
                    for p in range(2)]

            # ---------------- phase B: QKV projection ----------------
            # (sw, sx) product chains: x8·W8 + dx8·W8 + x8·dW8, grouped so
            # the x8-only passes run first (dx8 streams in behind x8)
            PASSES = (((0, 0), (1, 0)), ((0, 1),))     # qk: (sw, sx)
            VPASSES = (((0, 0), (0, 1)), ((1, 0),))    # v: (sx, sw)
            DR = mybir.MatmulPerfMode.DoubleRow
            QS = 8.0 / WSCALE   # psum (q*WSCALE) -> stored 8q / -8k

            def qk_store(cb, sl, pg):
                with nc.allow_low_precision(reason="fp8 score operands"):
                    if cb < 2:   # q: slot A, then residual B = 8q - A
                        d0 = qkT[cb][:, 0, sl]
                        nc.vector.tensor_scalar_mul(d0, pg, QS)
                        nc.vector.ln_bwd_dx(qkT[cb][:, 1, sl], pg, d0,
                                            1.0 / QS, 0.0, scale=QS)
                    else:        # k: single fp8(-8k); matmul reads it twice
                        nc.vector.tensor_scalar_mul(qkT[cb][:, sl], pg, -QS)

            def make_xq(s0):
                xq = xqpool.tile([128, 4, 2, 2, 512], dt.float8e4, tag="xq",
                                 name="xq")
                xeng = nc.scalar if xq_eng == 'act' else nc.sync
                xeng.dma_start(out=xq, in_=XT[s0])
                return xq

            def b_items():
                pre = [xq0, make_xq(1)]
                for s0 in range(NBC):
                    xq = pre[0]
                    pre = pre[1:]
                    if s0 + 2 < NBC:
                        pre.append(make_xq(s0 + 2))  # prefetch 2 ahead

                    # chunk 0: two-pass accumulation so the first matmuls
                    # only need the first half of wqk/xq0 (still streaming)
                    if s0 == 0:
                        pgs = {}

                        def qk_half(cb, kh):
                            if kh == 0:
                                pgs[cb] = bigpool.tile(
                                    [128, 512], dt.float32, tag="big",
                                    name="pg")
                            pg = pgs[cb]
                            mm = kh * 6
                            for chains in PASSES:
                                for t in (kh * 2, kh * 2 + 1):
                                    for sw, sx in chains:
                                        nc.tensor.matmul(
                                            pg,
                                            wqk[:, t, sw, :,
                                                cb * 128:(cb + 1) * 128],
                                            xq[:, t, sx, :, :],
                                            start=(mm == 0), stop=(mm == 11),
                                            perf_mode=DR)
                                        mm += 1
                            if kh == 1:
                                qk_store(cb, slice(0, 512), pg)

                        # pairwise interleave: at most 2 open psum groups
                        # (ring=2), first items need only the first halves
                        for cb0 in (0, 2):
                            yield (lambda cb=cb0: qk_half(cb, 0))
                            yield (lambda cb=cb0 + 1: qk_half(cb, 0))
                            yield (lambda cb=cb0: qk_half(cb, 1))
                            yield (lambda cb=cb0 + 1: qk_half(cb, 1))

                        def v_item0(ts):
                            pv = bigpool.tile([128, 4, 64], dt.float32,
                                              tag="big", name="pv")
                            mm = 0
                            for chains in VPASSES:
                                for t in range(4):
                                    for sx, sw in chains:
                                        nc.tensor.matmul(
                                            pv,
                                            xq[:, t, sx, :,
                                               ts * 128:(ts + 1) * 128],
                                            wv[:, t, sw, :, :],
                                            start=(mm == 0), stop=(mm == 11),
                                            perf_mode=DR)
                                        mm += 1
                            veng = (nc.gpsimd if v_eng == 'pool'
                                    else nc.vector)
                            with nc.allow_low_precision(reason="bf16"):
                                veng.tensor_scalar_mul(
                                    vaug[:, ts, :, 0:64], pv, 1.0 / WSCALE)
                        for ts in range(4):
                            yield (lambda ts=ts: v_item0(ts))
                        continue

                    def qk_item(s0=s0, xq=xq, cb=0):
                        pg = bigpool.tile([128, 512], dt.float32, tag="big",
                                          name="pg")
                        mm = 0
                        for chains in PASSES:
                            for t in range(4):
                                for sw, sx in chains:
                                    nc.tensor.matmul(
                                        pg,
                                        wqk[:, t, sw, :,
                                            cb * 128:(cb + 1) * 128],
                                        xq[:, t, sx, :, :],
                                        start=(mm == 0), stop=(mm == 11),
                                        perf_mode=DR)
                                    mm += 1
                        qk_store(cb, slice(s0 * 512, (s0 + 1) * 512), pg)
                    def v_item(s0=s0, xq=xq, ts=0):
                        pv = bigpool.tile([128, 4, 64], dt.float32,
                                          tag="big", name="pv")
                        mm = 0
                        for chains in VPASSES:
                            for t in range(4):
                                for sx, sw in chains:
                                    nc.tensor.matmul(
                                        pv,
                                        xq[:, t, sx, :,
                                           ts * 128:(ts + 1) * 128],
                                        wv[:, t, sw, :, :],
                                        start=(mm == 0), stop=(mm == 11),
                                        perf_mode=DR)
                                    mm += 1
                        st = s0 * 4 + ts
                        veng = nc.gpsimd if v_eng == 'pool' else nc.vector
                        with nc.allow_low_precision(reason="bf16"):
                            veng.tensor_scalar_mul(
                                vaug[:, st, :, 0:64], pv, 1.0 / WSCALE)
                    # interleave qk/v items: v stores are one DVE op, so a
                    # (qk, v) cadence keeps the shared pg/pv psum ring from
                    # stalling on the 2-op qk store chain
                    if b_order == 'interleave':
                        for j in range(4):
                            yield (lambda s0=s0, xq=xq, cb=j:
                                   qk_item(s0, xq, cb))
                            yield (lambda s0=s0, xq=xq, ts=j:
                                   v_item(s0, xq, ts))
                    else:
                        for cb in range(4):
                            yield (lambda s0=s0, xq=xq, cb=cb:
                                   qk_item(s0, xq, cb))
                        for ts in range(4):
                            yield (lambda s0=s0, xq=xq, ts=ts:
                                   v_item(s0, xq, ts))

            b_gen = b_items()
            b_total = 12 + (NBC - 1) * 8   # chunk 0 split into 12 items
            b_emitted = 0

            def emit_b(n):
                done = 0
                for _ in range(n):
                    item = next(b_gen, None)
                    if item is None:
                        break
                    item()
                    done += 1
                return done

            # ---------------- phase C: band attention ----------------
            from collections import deque
            pending = deque()
            _dq = deque()
            _held = []

            cur_stp = [None]

            def score_unit(h, qt):
                pr, po = h // 2, (h % 2) * 64
                gs = [g for g in range(qt - 2, qt + 3) if 0 <= g < NT]
                nA = len(gs)
                stp = sapool.tile([128, 5, 128], dt.float32, tag="stp",
                                  name="stp")
                cur_stp[0] = stp
                for j in range(nA):
                    g = gs[j]
                    # stationary k read twice via a stride-0 slot dim: the
                    # DR slot sum k.(A+B) = k.8q cancels q's fp8 error
                    kap = qkT[2 + pr][po:po + 64, g * 128:(g + 1) * 128]
                    k2 = bass.AP(tensor=kap.tensor, offset=kap.offset,
                                 ap=[kap.ap[0], [0, 2]] + list(kap.ap[1:]))
                    nc.tensor.matmul(
                        stp[:, j, :], k2,
                        qkT[pr][po:po + 64, :, qt * 128:(qt + 1) * 128],
                        perf_mode=DR)
                ptA = ptpool.tile([128, 5, 128], dt.bfloat16, tag="pt",
                                  name="ptA")
                # psum holds -64*qk; negative scale restores exp(qk/8)
                nc.scalar.activation(ptA[:, 0:nA, :], stp[:, 0:nA, :],
                                     mybir.ActivationFunctionType.Exp,
                                     scale=-1.0 / (64.0 * np.sqrt(HD)))
                lo = gs[0] == qt - 2
                up = gs[-1] == qt + 2
                m_eng = 'dve1' if qt >= tail_q else mask_eng
                with nc.allow_low_precision(reason="bf16"):
                    if m_eng == 'dve1' and lo and up:
                        # both triangles in one strided op (slices 0 and 4)
                        nc.vector.tensor_mul(ptA[:, 0:5:4, :],
                                             ptA[:, 0:5:4, :], masks)
                    else:
                        meng = (nc.vector if m_eng in ('dve', 'dve1')
                                else nc.gpsimd)
                        if lo:
                            meng.tensor_mul(ptA[:, 0, :], ptA[:, 0, :],
                                            masks[:, 0, :])
                        if up:
                            meng.tensor_mul(ptA[:, nA - 1, :],
                                            ptA[:, nA - 1, :], masks[:, 1, :])
                return (gs, nA, ptA)

            cn_ref = [None, None]  # per parity: pending pair ctxn tile
            _tq = deque()          # deferred ctxT transpose: (h, qt, ctxn2)

            def pv_unit(h, qt, gs, nA, ptA):
                pr = h // 2
                ctx = cxpool.tile([128, 65], dt.float32, tag="cx",
                                  name="ctx")
                n = len(gs)
                # masked slices (0 and n-1) go last: their mask ops on the
                # mask engine get the longest lead time
                order = list(range(1, n - 1)) + [n - 1, 0] if n > 2 \
                    else list(range(n))
                for i, j in enumerate(order):
                    nc.tensor.matmul(ctx, ptA[:, j, :], vaug[:, gs[j], h, :],
                                     start=(i == 0), stop=(i == n - 1))
                rec = recpool.tile([128, 1], dt.float32, tag="rec",
                                   name="rec")
                nc.vector.reciprocal(rec, ctx[:, 64:65])
                if h % 2 == 0:
                    cn_ref[pr] = cnpool.tile([128, 2, 64], dt.bfloat16,
                                             tag="cn", name="ctxn2")
                ctxn2 = cn_ref[pr]
                with nc.allow_low_precision(reason="bf16"):
                    nc.vector.tensor_scalar_mul(ctxn2[:, h % 2, :],
                                                ctx[:, 0:64], rec)
                _tq.append((h, qt, ctxn2))

            def tp_unit(h, qt, ctxn2):
                # PE-transpose a head pair's normalized context in one shot:
                # ctxn2 [128 q, 128 pairdims] -> tp [128 pairdims, 128 q].
                # Scratch = slice 5 of the score tile in flight (never used
                # for scores), viewed as bf16.
                if h % 2 == 1:
                    pr = h // 2
                    if tp_pool == 'big':
                        tp = bigpool.tile([128, 128], dt.bfloat16,
                                          tag="big", name="tp")
                    else:
                        tp = cxpool.tile([128, 128], dt.bfloat16, tag="cx",
                                         name="tp")
                    nc.tensor.transpose(tp, ctxn2, ident)
                    with nc.allow_low_precision(reason="bf16"):
                        nc.vector.tensor_copy(
                            ctxT[pr][:, qt * 128:(qt + 1) * 128], tp)
                if h == NH_CORE - 1:
                    _dq.append(qt)

            def emit_d(qt, split_dma=False, tail=False):
                osb = opool.tile([128, 1024], dt.bfloat16, tag="osb",
                                 name="osb")
                deng = nc.gpsimd if osb_eng == 'gpsimd' else nc.sync
                for nn in range(2):
                    use_cx = (pd_pool == 'cx' or
                              (pd_pool == 'split' and nn == 0))
                    if use_cx:
                        pD = cxpool.tile([128, 512], dt.float32, tag="cx",
                                         name="pD")
                    else:
                        pD = bigpool.tile([128, 512], dt.float32, tag="big",
                                          name="pD")
                    for p in range(2):
                        nc.tensor.matmul(
                            pD, ctxT[p][:, qt * 128:(qt + 1) * 128],
                            wo[:, p, nn * 512:(nn + 1) * 512],
                            start=(p == 0), stop=(p == 1))
                    dst = osb[:, nn * 512:(nn + 1) * 512]
                    use_act = (nn == 0) if osb_copy != 'swap' else (nn == 1)
                    with nc.allow_low_precision(reason="bf16 partials"):
                        if osb_copy == 'pool':
                            nc.gpsimd.tensor_copy(dst, pD)
                        elif osb_copy == 'dve2' or tail or not use_act:
                            nc.vector.tensor_copy(dst, pD)
                        else:
                            nc.scalar.copy(dst, pD)
                    if split_dma:
                        deng.dma_start(
                            out=OUT[qt * 128:(qt + 1) * 128,
                                    nn * 512:(nn + 1) * 512],
                            in_=osb[:, nn * 512:(nn + 1) * 512])
                if not split_dma:
                    deng.dma_start(out=OUT[qt * 128:(qt + 1) * 128, :],
                                   in_=osb)

            # pacing: unit qt needs qkT/vaug through token (qt+2)*128+128,
            # i.e. chunks 0..ceil((qt*128+384)/512)-1 done.
            b_emitted += emit_b(8 * b_prol)
            for qt in range(NQT):
                # scores of qt need chunks covering tokens to (qt+3)*128-1,
                # i.e. chunks 0..(qt+2)//4 done; b_slack items of margin.
                need = min(b_total, 12 + 8 * ((qt + 2) // 4) + b_slack)
                # heads in order (0,2,1,3): staggers the two pair-chains
                for h in (0, 2, 1, 3):
                    dd = d_delay if qt < NQT - 2 else 0
                    while _dq and len(_dq) > dd:
                        dqt = _dq.popleft()
                        # park a few mid-sequence o-proj blocks: they become
                        # dependency-free PE work overlapping the final
                        # attention drain
                        if d_hold and len(_held) < d_hold and 16 <= dqt < 28:
                            _held.append(dqt)
                        else:
                            emit_d(dqt, split_dma=(dqt >= NQT - 2),
                                   tail=(dqt >= tail_d))
                    if d_hold and qt >= d_release and _held:
                        emit_d(_held.pop(0), split_dma=True, tail=True)
                    want = need - b_emitted
                    if want > 0:
                        per = max(1, (want + (NH_CORE - h) - 1)
                                  // (NH_CORE - h))
                        if b_cap:
                            per = min(per, b_cap)
                        b_emitted += emit_b(per)
                    if pv_first == 2 and len(_tq) > tp_delay:
                        tp_unit(*_tq.popleft())
                    if pv_first == 1 and len(pending) >= depth:
                        pv_unit(*pending.popleft())
                        pending.append((h, qt) + score_unit(h, qt))
                    else:
                        pending.append((h, qt) + score_unit(h, qt))
                        if len(pending) > depth:
                            pv_unit(*pending.popleft())
                    if pv_first != 2 and len(_tq) > tp_delay:
                        tp_unit(*_tq.popleft())
            while pending:
                pv_unit(*pending.popleft())
                if len(_tq) > 1:
                    tp_unit(*_tq.popleft())
            while _tq:
                tp_unit(*_tq.popleft())
                while _dq:
                    emit_d(_dq.popleft())
            b_emitted += emit_b(b_total)
            while _dq:
                emit_d(_dq.popleft(), split_dma=True)
            for dqt in _held:
                emit_d(dqt, split_dma=True)

    nc.compile()
    return nc


f8 = ml_dtypes.float8_e4m3


def _split8(a):
    """a (f32) -> (a8, da8) fp8e4 with a ~= a8 + da8 (compensated split)."""
    a8 = a.astype(f8)
    d8 = (a - a8.astype(np.float32)).astype(f8)
    return a8, d8


WSCALE = 128.0  # lifts W (and its residual) out of e4m3's subnormal range


def _pack_w8(wcols, ncol):
    """[1024, ncol] f32 -> [128, 4t, 2s, 2i, ncol] fp8 with contraction
    index c = 256t + 128i + p. Weights are pre-scaled by WSCALE; the
    psum->sbuf copy divides it back out."""
    w8, dw8 = _split8(wcols * WSCALE)
    ws = np.stack([w8, dw8])                     # [s, 1024, ncol]
    ws = ws.reshape(2, 4, 2, 128, ncol)          # [s, t, i, p, col]
    return np.ascontiguousarray(ws.transpose(3, 1, 0, 2, 4))


def _prep_fast(x, Wqkv, Wo):
    """Per-core input maps (compensated fp8 QKV operands, bf16 Wo)."""
    xT_b = []
    for b in range(B):
        xt = np.ascontiguousarray(x[b].T)              # [E, S] f32
        x8, dx8 = _split8(xt)
        xs = np.stack([x8, dx8])                       # [s, E, S]
        xs = xs.reshape(2, 4, 2, 128, 8, 512)          # [s, t, i, p, s0, tok]
        xT_b.append(np.ascontiguousarray(xs.transpose(4, 3, 1, 0, 2, 5)))
        # xT_b[b][s0, p, t, s, i, tok] = xs[s, 256t+128i+p, 512*s0+tok]
    in_maps = []
    for c in range(8):
        b, hg = c // 4, c % 4
        heads = range(4 * hg, 4 * hg + 4)
        qcols = np.concatenate([np.arange(h * 192, h * 192 + 64)
                                for h in heads])
        kcols = qcols + 64
        vcols = qcols + 128
        wqk_cols = np.concatenate([qcols, kcols])           # [512]
        wqk = _pack_w8(Wqkv[:, wqk_cols], 512)
        wv = _pack_w8(Wqkv[:, vcols], 256)
        orows = np.concatenate([np.arange(h * 64, h * 64 + 64)
                                for h in heads])
        wo = np.ascontiguousarray(Wo[orows].reshape(2, 128, 1024)).astype(bf16)
        in_maps.append({"xT": xT_b[b], "wqk": wqk, "wv": wv, "wo": wo})
    return in_maps



def _build_generic(vbias=True, st_bufs=2, po_bufs=1, bc_bufs=1, cx_bufs=2,
           mask_eng='dve', bccopy_eng='act', pt_bufs=8,
           osbcopy_eng='dve', bcast_via='pe', paired=True, depth=1,
           fuse_b=True, pb_bufs=2, b_lead=3, b_prol=2,
           norm_src='sbuf', ctxcopy_eng='act'):
    if fuse_b:
        pt_bufs = min(pt_bufs, 6)
    _nb = 2 if fuse_b else 3
    nc = bacc.Bacc("TRN2", target_bir_lowering=False, debug=False, num_devices=8)

    XT = nc.dram_tensor("xT", [128, 16, 8, 256], dt.float32r, kind="ExternalInput")
    WQK = nc.dram_tensor("wqk", [128, 8, 4, 128], dt.float32r, kind="ExternalInput")
    WV = nc.dram_tensor("wv", [128, 8, 256], dt.float32r, kind="ExternalInput")
    WO = nc.dram_tensor("wo", [2, 128, 1024], dt.float32r, kind="ExternalInput")
    BQK = nc.dram_tensor("bqk", [128, 4], dt.float32, kind="ExternalInput")
    BV = nc.dram_tensor("bv", [1, 256], dt.float32, kind="ExternalInput")
    MV8 = nc.dram_tensor("mv8", [128, 32], dt.float32, kind="ExternalInput")
    OUT = nc.dram_tensor("out", [S, E], dt.float32, kind="ExternalOutput")

    # constant 0/1 triangular band masks for u in {-2,-1,2,3}
    p_i = np.arange(128)[:, None]
    r_i = np.arange(256)[None, :]
    mask_np = {}
    for u in (-2, -1, 2, 3):
        mask_np[u] = ((u * 128 + p_i - r_i >= -w) & (u * 128 + p_i - r_i <= w)
                      ).astype(np.float32)
    MASKS = nc.inline_tensor(
        np.ascontiguousarray(
            np.stack([mask_np[u] for u in (-2, -1, 2, 3)]).transpose(1, 0, 2)),
        name="trimasks")
    ONES = nc.inline_tensor(np.ones((1, 128), dtype=np.float32), name="onesrow")

    with tile.TileContext(nc) as tc:
        with tc.tile_pool(name="const", bufs=1) as cpool, \
             tc.tile_pool(name="qkT", bufs=1) as qkpool, \
             tc.tile_pool(name="vaug", bufs=1) as vpool, \
             tc.tile_pool(name="ctxT", bufs=1) as ctxpool:

            wo = [cpool.tile([128, 1024], dt.float32r, name=f"wo{p}") for p in range(2)]
            bqk = cpool.tile([128, 4], dt.float32)
            nc.gpsimd.dma_start(out=bqk, in_=BQK[:, :])
            bv_f = cpool.tile([1, 256], dt.float32)
            nc.gpsimd.dma_start(out=bv_f, in_=BV[:, :])
            mv8 = cpool.tile([128, 32], dt.float32)
            nc.gpsimd.dma_start(out=mv8, in_=MV8[:, :])
            masks = cpool.tile([128, 4, 256], dt.float32)
            mask_idx = {-2: 0, -1: 1, 2: 2, 3: 3}
            ones_f = cpool.tile([1, 128], dt.float32)
            nc.gpsimd.dma_start(out=ones_f, in_=ONES[:, :])
            ones_r = cpool.tile([1, 128], dt.float32r)
            bv_r = cpool.tile([1, 256], dt.float32r)
            with nc.allow_low_precision(reason="f32r matmul pipeline"):
                nc.vector.tensor_copy(ones_r, ones_f)
                nc.vector.tensor_copy(bv_r, bv_f)

            # persistent intermediates
            qkT = [qkpool.tile([128, S], dt.float32r, name=f"qkT{cb}")
                   for cb in range(4)]  # 0,1: q pairs; 2,3: k pairs
            vaug = [vpool.tile([128, NT, 65], dt.float32r, name=f"vaug{h}")
                    for h in range(NH_CORE)]
            ones32 = cpool.tile([128, NT], dt.float32)
            nc.vector.memset(ones32, 1.0)
            for h in range(NH_CORE):
                with nc.allow_low_precision(reason="f32r"):
                    nc.vector.tensor_copy(vaug[h][:, :, 64], ones32)
            ctxT = [ctxpool.tile([128, S], dt.float32r, name=f"ctxT{p}")
                    for p in range(2)]

            # ---------------- Phase B: QKV projection ----------------
            # Emitted either up front (fuse_b=False) or as fine-grained work
            # items interleaved into the attention loop's idle PE slots.
            bwpool = ctx_pools = None
            import contextlib
            _bstack = contextlib.ExitStack()
            bwpool = _bstack.enter_context(tc.tile_pool(name="bw", bufs=1))
            xqpool = _bstack.enter_context(
                tc.tile_pool(name="xq", bufs=(2 if fuse_b else 3)))
            pbpool = _bstack.enter_context(
                tc.tile_pool(name="pb", bufs=(pb_bufs if fuse_b else 8),
                             space="PSUM"))
            wqk = bwpool.tile([128, 8, 4, 128], dt.float32r)
            wv = bwpool.tile([128, 8, 256], dt.float32r)
            xq0 = [xqpool.tile([128, 4, 256], dt.float32r, tag=f"xq{i}",
                               name="xq") for i in range(2)]
            for i in range(2):
                nc.sync.dma_start(out=xq0[i], in_=XT[:, 0, i * 4:(i + 1) * 4, :])
            for kt in range(8):
                nc.sync.dma_start(out=wqk[:, kt, :, :], in_=WQK[:, kt, :, :])
            nc.sync.dma_start(out=wv[:, 0:4, :], in_=WV[:, 0:4, :])
            nc.sync.dma_start(out=wv[:, 4:8, :], in_=WV[:, 4:8, :])

            def b_items():
                for s0 in range(16):  # 256-token chunks of S
                    if s0 == 0:
                        xq = xq0
                    else:
                        xq = [xqpool.tile([128, 4, 256], dt.float32r,
                                          tag=f"xq{i}", name="xq")
                              for i in range(2)]
                        for i in range(2):
                            nc.sync.dma_start(
                                out=xq[i], in_=XT[:, s0, i * 4:(i + 1) * 4, :])

                    def qk_item(s0=s0, xq=xq, cb=0):
                        pg = pbpool.tile([128, 256], dt.float32, tag="pb",
                                         name="pqk")
                        for k8 in range(8):
                            nc.tensor.matmul(pg, wqk[:, k8, cb, :],
                                             xq[k8 // 4][:, k8 % 4, :],
                                             start=(k8 == 0), stop=(k8 == 7))
                        nc.scalar.activation(
                            qkT[cb][:, s0 * 256:(s0 + 1) * 256], pg,
                            mybir.ActivationFunctionType.Identity,
                            bias=bqk[:, cb:cb + 1])
                    for cb in range(4):
                        yield (lambda s0=s0, xq=xq, cb=cb: qk_item(s0, xq, cb))

                    def v_item(s0=s0, xq=xq, hf=0):
                        pv = pbpool.tile([128, 256], dt.float32, tag="pb",
                                         name="pv")
                        for k8 in range(8):
                            nc.tensor.matmul(
                                pv,
                                xq[k8 // 4][:, k8 % 4, hf * 128:(hf + 1) * 128],
                                wv[:, k8, :], start=(k8 == 0),
                                stop=(k8 == 7 and not vbias))
                        if vbias:
                            nc.tensor.matmul(pv, ones_r, bv_r,
                                             start=False, stop=True)
                        st = s0 * 2 + hf
                        for h in range(NH_CORE):
                            with nc.allow_low_precision(reason="f32r"):
                                nc.vector.tensor_copy(
                                    vaug[h][:, st, 0:64],
                                    pv[:, h * 64:(h + 1) * 64])
                    for hf in range(2):
                        yield (lambda s0=s0, xq=xq, hf=hf: v_item(s0, xq, hf))

            b_gen = b_items()
            b_total = 16 * 6
            b_emitted = 0

            def emit_b(n):
                emitted = 0
                for _ in range(n):
                    item = next(b_gen, None)
                    if item is None:
                        break
                    item()
                    emitted += 1
                return emitted

            if not fuse_b:
                b_emitted += emit_b(b_total)
                _bstack.close()

            nc.gpsimd.dma_start(out=masks, in_=MASKS[:, :, :])
            for p in range(2):
                nc.gpsimd.dma_start(out=wo[p], in_=WO[p, :, :])
            # ------- Phase C: band attention, with output projection folded in -------
            import contextlib
            _cstack = contextlib.ExitStack()
            with _cstack:
                stpool = _cstack.enter_context(
                    tc.tile_pool(name="stp", bufs=st_bufs, space="PSUM"))
                cxpool = _cstack.enter_context(
                    tc.tile_pool(name="ctxp", bufs=cx_bufs, space="PSUM"))
                if bcast_via == 'pe':
                    bcpool = _cstack.enter_context(
                        tc.tile_pool(name="bcp", bufs=bc_bufs, space="PSUM"))
                else:
                    drpool = _cstack.enter_context(
                        tc.tile_pool(name="dr", bufs=4, space="DRAM"))
                popool = _cstack.enter_context(
                    tc.tile_pool(name="po", bufs=po_bufs, space="PSUM"))
                ptpool = _cstack.enter_context(
                    tc.tile_pool(name="pt", bufs=pt_bufs))
                bcsb = _cstack.enter_context(tc.tile_pool(name="bcs", bufs=_nb))
                opool = _cstack.enter_context(tc.tile_pool(name="osb", bufs=2))
                rcpool = _cstack.enter_context(tc.tile_pool(name="rcp", bufs=_nb))

                def score_stage(h, cc):
                    # returns list of (gts, pt, jslices) where pt holds exp'd
                    # probabilities for the key tiles in gts
                    pr, po = h // 2, (h % 2) * 64
                    out = []
                    if paired:
                        # all-ones padding: exp has no per-key bias, so key
                        # tiles are processed in aligned pairs (one psum bank,
                        # one exp, one mask-mul per pair)
                        for ub in (-2, 0, 2):
                            gts = [2 * cc + ub, 2 * cc + ub + 1]
                            if gts[0] < 0 or gts[1] >= NT:
                                continue
                            stp = stpool.tile([128, 2, 256], dt.float32,
                                              tag="st", name="stp")
                            for j, gt in enumerate(gts):
                                nc.tensor.matmul(
                                    stp[:, j, :],
                                    qkT[2 + pr][po:po + 64,
                                                gt * 128:(gt + 1) * 128],
                                    qkT[pr][po:po + 64,
                                            cc * 256:(cc + 1) * 256])
                            pt = ptpool.tile([128, 2, 256], dt.float32r,
                                             tag="pt", name="pt")
                            nc.scalar.activation(
                                pt, stp, mybir.ActivationFunctionType.Exp,
                                scale=1.0 / np.sqrt(HD))
                            if ub != 0:
                                mi = 0 if ub == -2 else 2
                                with nc.allow_low_precision(reason="f32r"):
                                    eng = (nc.gpsimd if mask_eng == 'gpsimd'
                                           else nc.vector)
                                    eng.tensor_mul(pt, pt,
                                                   masks[:, mi:mi + 2, :])
                            out.append((gts, pt))
                        return out
                    for u in range(-2, 4):
                        gt = 2 * cc + u
                        if not 0 <= gt < NT:
                            continue
                        stp = stpool.tile([128, 256], dt.float32, tag="st",
                                          name="stp")
                        nc.tensor.matmul(
                            stp,
                            qkT[2 + pr][po:po + 64, gt * 128:(gt + 1) * 128],
                            qkT[pr][po:po + 64, cc * 256:(cc + 1) * 256])
                        pt = ptpool.tile([128, 256], dt.float32r, tag="pt",
                                         name="pt")
                        nc.scalar.activation(pt, stp,
                                             mybir.ActivationFunctionType.Exp,
                                             bias=mv8[:, gt:gt + 1],
                                             scale=1.0 / np.sqrt(HD))
                        if u in mask_idx:
                            with nc.allow_low_precision(reason="f32r"):
                                eng = (nc.gpsimd if mask_eng == 'gpsimd'
                                       else nc.vector)
                                eng.tensor_mul(pt, pt,
                                               masks[:, mask_idx[u], :])
                        out.append(([gt], pt))
                    return out

                def pv_stage(h, cc, pts):
                    if _dq:
                        emit_d(_dq.popleft())
                    pr, po = h // 2, (h % 2) * 64
                    ctx = cxpool.tile([65, 256], dt.float32, tag="cx",
                                      name="ctx")
                    nmm = sum(len(gts) for gts, _ in pts)
                    j = 0
                    for gts, pt in pts:
                        for jj, gt in enumerate(gts):
                            rhs = pt[:, jj, :] if len(gts) > 1 else pt
                            nc.tensor.matmul(ctx, vaug[h][:, gt, :], rhs,
                                             start=(j == 0),
                                             stop=(j == nmm - 1))
                            j += 1
                    if norm_src == 'sbuf':
                        # copy ctx out of PSUM first: frees the cx slot early
                        # and the final multiply reads bc straight from PSUM
                        cxs = bcsb.tile([65, 256], dt.float32, tag="bcs",
                                        name="cxs")
                        if ctxcopy_eng == 'act':
                            nc.scalar.copy(cxs, ctx)
                        else:
                            nc.vector.tensor_copy(cxs, ctx)
                        ctx = cxs
                    rec = rcpool.tile([1, 256], dt.float32r, tag="rc",
                                      name="rec")
                    with nc.allow_low_precision(reason="f32r"):
                        nc.vector.reciprocal(rec, ctx[64:65, :])
                    bcs = None
                    if norm_src != 'sbuf':
                        bcs = bcsb.tile([64, 256], dt.float32, tag="bcs",
                                        name="bcs")
                    if bcast_via == 'dma':
                        drec = drpool.tile([1, 256], dt.float32r, tag="dr",
                                           name="drec")
                        nc.sync.dma_start(out=drec, in_=rec)
                        dbc = bass.AP(tensor=drec.tensor, offset=drec.offset,
                                      ap=[[0, 64]] + drec.ap[1:])
                        nc.sync.dma_start(out=bcs.bitcast(dt.float32r), in_=dbc)
                    else:
                        bc = bcpool.tile([64, 256], dt.float32, tag="bc",
                                         name="bc")
                        nc.tensor.matmul(bc, ones_r[:, 0:64], rec)
                        if norm_src == 'sbuf':
                            bcs = bc
                        elif bccopy_eng == 'act':
                            nc.scalar.copy(bcs, bc)
                        else:
                            nc.vector.tensor_copy(bcs, bc)
                    with nc.allow_low_precision(reason="f32r"):
                        nc.vector.tensor_mul(
                            ctxT[pr][po:po + 64, cc * 256:(cc + 1) * 256],
                            ctx[0:64, :], bcs)
                    if h == NH_CORE - 1:
                        _dq.append(2 * cc)
                        _dq.append(2 * cc + 1)

                def emit_d(qt):
                    osb = opool.tile([128, 1024], dt.float32, tag="osb",
                                     name="osb")
                    for nn in range(2):
                        pD = popool.tile([128, 512], dt.float32, tag="po",
                                         name="pD")
                        for p in range(2):
                            nc.tensor.matmul(pD,
                                             ctxT[p][:, qt * 128:(qt + 1) * 128],
                                             wo[p][:, nn * 512:(nn + 1) * 512],
                                             start=(p == 0), stop=(p == 1))
                        if osbcopy_eng == 'act':
                            nc.scalar.copy(osb[:, nn * 512:(nn + 1) * 512], pD)
                        else:
                            nc.vector.tensor_copy(osb[:, nn * 512:(nn + 1) * 512], pD)
                    nc.gpsimd.dma_start(out=OUT[qt * 128:(qt + 1) * 128, :],
                                        in_=osb)

                from collections import deque
                pending = deque()
                _dq = deque()
                if fuse_b:
                    # prologue: cover key tiles for the first two query chunks
                    b_emitted += emit_b(6 * b_prol)
                step = 0
                for cc in range(NCC):
                    for h in range(NH_CORE):
                        if fuse_b:
                            # pace remaining B so chunk cc+2 is done before
                            # attention chunk cc+1 starts
                            target = min(b_total, 6 * (cc + b_lead))
                            want = target - b_emitted
                            per = max(1, (want + (NH_CORE - h) - 1)
                                      // (NH_CORE - h))
                            if want > 0:
                                b_emitted += emit_b(per)
                        pts = score_stage(h, cc)
                        pending.append((h, cc, pts))
                        if len(pending) > depth:
                            pv_stage(*pending.popleft())
                        step += 1
                while pending:
                    pv_stage(*pending.popleft())
                while _dq:
                    emit_d(_dq.popleft())
                if fuse_b:
                    b_emitted += emit_b(b_total)

            _bstack.close()

    nc.compile()
    return nc



def _prep_generic(x, Wqkv, bqkv, Wo, pm):
    in_maps = []
    xT_b = []
    for b in range(B):
        xt = np.ascontiguousarray(x[b].T)                      # [E, S]
        xT_b.append(np.ascontiguousarray(
            xt.reshape(8, 128, 16, 256).transpose(1, 2, 0, 3)))
    mv8_b = []
    for b in range(B):
        # mv8[p, t] = (0 if valid else NEG)/8 for key index t*128+p
        mv = np.where(pm[b], 0.0, NEG).astype(np.float32) / 8.0
        mv8_b.append(np.ascontiguousarray(mv.reshape(32, 128).T))

    for c in range(8):
        b, hg = c // 4, c % 4
        heads = range(4 * hg, 4 * hg + 4)
        qcols = np.concatenate([np.arange(h * 192, h * 192 + 64) for h in heads])
        kcols = qcols + 64
        vcols = qcols + 128
        wqk_cols = np.concatenate([qcols, kcols])               # [512]
        wqk = np.ascontiguousarray(
            Wqkv[:, wqk_cols].reshape(8, 128, 4, 128).transpose(1, 0, 2, 3))
        wv = np.ascontiguousarray(
            Wqkv[:, vcols].reshape(8, 128, 256).transpose(1, 0, 2))
        orows = np.concatenate([np.arange(h * 64, h * 64 + 64) for h in heads])
        wo = np.ascontiguousarray(Wo[orows].reshape(2, 128, 1024))
        in_maps.append({
            "xT": xT_b[b],
            "wqk": wqk,
            "wv": wv,
            "wo": wo,
            "bqk": np.ascontiguousarray(bqkv[wqk_cols].reshape(4, 128).T),
            "bv": np.ascontiguousarray(bqkv[vcols].reshape(1, 256)),
            "mv8": mv8_b[b],
        })
    return in_maps


def kernel(x, Wqkv, bqkv, Wo, bo, padding_mask, num_heads, window_size):
    assert int(num_heads) == H and int(window_size) == W
    x = np.asarray(x, dtype=np.float32)
    Wqkv = np.asarray(Wqkv, dtype=np.float32)
    bqkv = np.asarray(bqkv, dtype=np.float32)
    Wo = np.asarray(Wo, dtype=np.float32)
    bo = np.asarray(bo, dtype=np.float32)
    pm = np.asarray(padding_mask).astype(bool)
    assert x.shape == (B, S, E)

    fast = bool(pm.all()) and not np.any(bqkv)
    if fast:
        if "fast" not in _cache:
            _cache["fast"] = _build_fast(depth=3, tp_delay=1, d_delay=2,
                                         warm_n=8, osb_eng='sync',
                                         pd_pool='split', xq_eng='sync',
                                         mask_eng='gpsimd', b_slack=2,
                                         b_prol=1, pv_first=2, tail_q=26)
        nc = _cache["fast"]
        in_maps = _prep_fast(x, Wqkv, Wo)
    else:
        vbias = bool(np.any(bqkv.reshape(H, 3, HD)[:, 2, :] != 0.0))
        key = ("nc", vbias, False)
        if key not in _cache:
            _cache[key] = _build_generic(vbias=vbias, paired=False)
        nc = _cache[key]
        in_maps = _prep_generic(x, Wqkv, bqkv, Wo, pm)

    res = run_bass_kernel_spmd(nc, in_maps, list(range(8)))
    kernel._last_results = res

    out = np.empty((B, S, E), dtype=np.float32)
    for b in range(B):
        acc = res.results[4 * b]["out"].astype(np.float32)
        for g in range(1, 4):
            acc = acc + res.results[4 * b + g]["out"].astype(np.float32)
        out[b] = acc + bo
    return out



# revision 61
# speedup vs baseline: 1.1459x; 1.0002x over previous
"""Sliding-window multi-head attention (Longformer-style band attention) for
Trainium2, distributed over 8 NeuronCores.

Sharding: data-parallel over batch (B=2) x tensor-parallel over heads
(16 heads -> 4 groups of 4). Core c handles batch c//4, heads
[4*(c%4), 4*(c%4)+4). Each core computes the QKV projection for its head
group, band attention over 128-key tiles, and a partial output projection;
the host sums the 4 partials per batch and adds bo.

Fast path (all-ones padding mask, zero qkv bias): mixed fp8/bf16 datapath
tuned against the TRN2 cost model, where fp8e4 DoubleRow matmuls process two
128-row contraction slices per instruction at 0.5 cycles/row (4x bf16 for
deep contractions).

- QKV projection: error-compensated fp8 DoubleRow. Host splits x = x8 + dx8
  and W*128 = W8 + dW8 (the *128 lifts W and its residual out of e4m3's
  subnormal range; the psum->sbuf copy divides it back). Three product
  chains x8W8 + dx8W8 + x8dW8 run at 0.75x the bf16 PE cost with bf16-grade
  accuracy.
- Scores: half-compensated fp8 DoubleRow at 0.5x bf16 PE cost. q is stored
  as slot pair A = fp8(8q), B = fp8(8q - A) (one tensor_scalar + one
  LN_BWD_DX custom-DVE op); k as a single fp8(-8k) copy that the matmul's
  stationary operand reads twice via a stride-0 slot dim. The DoubleRow
  slot sum k.(A+B) = k.8q cancels q's quantization error exactly; only k's
  single-fp8 error (~1.3% end to end) remains. Score psum = -64*qk, undone
  by a negative exp scale.
- exp on the scalar engine into bf16 probabilities, triangular band masks
  on gpsimd (vector in the post-QKV tail, where gpsimd saturates), PV in
  bf16 as [query, 65] with an appended ones column giving the softmax
  denominator for free. PV/o-proj stay bf16: with only ~2e-2 error budget,
  single-sided fp8 fails there and full compensation costs the same as
  bf16.
- Normalization, PE-transpose and the bf16 output projection as before;
  QKV work items are paced into the attention loop to keep PE saturated,
  warm-up matmuls absorb the PE clock ramp.

Generic path (padding masks / nonzero qkv bias) falls back to an f32r
implementation of the same blocking.
"""
import sys
import numpy as np
import ml_dtypes

try:
    import concourse.bass as bass
except ImportError:
    sys.path.insert(0, "/opt/trn_rl_repo")
    import concourse.bass as bass
import concourse.mybir as mybir
import concourse.tile as tile
from concourse import bacc
from concourse.bass_utils import run_bass_kernel_spmd

dt = mybir.dt
bf16 = ml_dtypes.bfloat16

B, S, E, H, W = 2, 4096, 1024, 16, 512
HD = E // H          # 64
NH_CORE = 4
w = W // 2           # 256
NT = S // 128        # 32 key tiles of 128
NQT = S // 128       # 32 query tiles of 128
NBC = S // 512       # 8 qkv token chunks of 512
NCC = S // 256       # generic path: 16 query chunks of 256
NEG = -9e15

_cache = {}


def _build_fast(depth=2, b_prol=2, mask_eng='dve', qkcopy_eng='dve',
                d_delay=1, cx_bufs=2, big_bufs=2, tp_delay=1, warm_n=0,
                osb_eng='gpsimd', pd_pool='big', xq_eng='sync', b_slack=8,
                d_hold=0, tp_pool='cx', pv_first=0, st_bufs=2,
                osb_copy='mixed', v_eng='dve', d_release=99, tail_q=99,
                tail_d=99, b_order='seq', b_cap=0, tp_mode='pe',
                tp_eng='sync'):
    nc = bacc.Bacc("TRN2", target_bir_lowering=False, debug=False,
                   num_devices=8)

    # fp8 DoubleRow with host-side error compensation: x = x8 + dx8 and
    # W = W8 + dW8 (each fp8e4); three product chains x8W8 + x8dW8 + dx8W8
    # restore bf16-grade accuracy at 0.75x the bf16 PE cost (DoubleRow
    # contracts 256 rows per instruction at 0.5 cycles/row).
    # Layouts: [partition p, t (256-row ktile), s (main/residual), i
    # (DoubleRow pair), cols] with contraction index c = 256t + 128i + p.
    XT = nc.dram_tensor("xT", [8, 128, 4, 2, 2, 512], dt.float8e4,
                        kind="ExternalInput")
    WQK = nc.dram_tensor("wqk", [128, 4, 2, 2, 512], dt.float8e4,
                         kind="ExternalInput")
    WV = nc.dram_tensor("wv", [128, 4, 2, 2, 256], dt.float8e4,
                        kind="ExternalInput")
    WO = nc.dram_tensor("wo", [2, 128, 1024], dt.bfloat16,
                        kind="ExternalInput")
    OUT = nc.dram_tensor("out", [S, E], dt.bfloat16, kind="ExternalOutput")

    p_i = np.arange(128)[:, None]
    c_i = np.arange(128)[None, :]
    lo = (p_i >= c_i).astype(bf16)   # tile g==qt-2: valid kr >= qr
    up = (p_i <= c_i).astype(bf16)   # tile g==qt+2: valid kr <= qr
    MASKS = nc.inline_tensor(np.ascontiguousarray(
        np.stack([lo, up], axis=1)), name="trimasks")   # [128, 2, 128]
    IDENT = nc.inline_tensor(np.eye(128, dtype=bf16), name="ident")

    with tile.TileContext(nc) as tc:
        with tc.tile_pool(name="const", bufs=1) as cpool, \
             tc.tile_pool(name="qkTp", bufs=1) as qkpool, \
             tc.tile_pool(name="vaugp", bufs=1) as vpool, \
             tc.tile_pool(name="ctxTp", bufs=1) as ctpool, \
             tc.tile_pool(name="xq", bufs=4) as xqpool, \
             tc.tile_pool(name="pt", bufs=7) as ptpool, \
             tc.tile_pool(name="recp", bufs=4) as recpool, \
             tc.tile_pool(name="cnp", bufs=4) as cnpool, \
             tc.tile_pool(name="osbp", bufs=3) as opool, \
             tc.tile_pool(name="stp", bufs=st_bufs, space="PSUM") as sapool, \
             tc.tile_pool(name="cxp", bufs=cx_bufs, space="PSUM") as cxpool, \
             tc.tile_pool(name="bigp", bufs=big_bufs, space="PSUM") as bigpool:

            # ---- constants / weights ----
            wqk = cpool.tile([128, 4, 2, 2, 512], dt.float8e4)
            wv = cpool.tile([128, 4, 2, 2, 256], dt.float8e4)
            wo = cpool.tile([128, 2, 1024], dt.bfloat16)
            masks = cpool.tile([128, 2, 128], dt.bfloat16)
            ident = cpool.tile([128, 128], dt.bfloat16)
            # t-slice granularity so the first QKV matmuls start early
            # (subtile deps gate each accumulation step on its own slice);
            # scalar-engine HWDGE triggers: cheap and off the SP queue.
            # wqk/xq0 slices interleaved so slice pairs land together.
            xq0 = xqpool.tile([128, 4, 2, 2, 512], dt.float8e4, tag="xq",
                              name="xq")
            for kh in range(2):
                ks = slice(kh * 2, kh * 2 + 2)
                nc.scalar.dma_start(out=wqk[:, ks], in_=WQK[:, ks])
                nc.sync.dma_start(out=xq0[:, ks], in_=XT[0, :, ks])
            nc.scalar.dma_start(out=wv, in_=WV[:, :, :, :, :])
            # masks/ident/wo are issued after the b-item prologue (below) so
            # the serial DMA pipe delivers the chunk-1/2 x prefetches first;
            # they aren't read until the first score/transpose/o-proj units.

            # PE warmup: scratch matmuls absorb the p-state ramp while the
            # first input DMAs are still streaming in.
            if warm_n:
                wsrc = cpool.tile([128, 512], dt.bfloat16)
                nc.vector.memset(wsrc, 0.0)
                wdst = bigpool.tile([128, 512], dt.float32, tag="big",
                                    name="wdst")
                for i in range(warm_n):
                    nc.tensor.matmul(wdst, wsrc[:, 0:128], wsrc)

            # ---- persistent intermediates ----
            # q/k stored as fp8 DoubleRow slot pairs (scores run in fp8-DR
            # at half the bf16 PE cost). q: slot0 A = fp8(8q), slot1
            # B = fp8(8q - A); k: both slots fp8(-8k). The DR slot sum
            # k_n*A + k_n*B = k_n*8q cancels q's quantization error exactly;
            # only k's single-fp8 error remains. Score psum = -64*qk, undone
            # by a negative exp scale.
            qkT = [qkpool.tile([128, 2, S], dt.float8e4, name=f"qkT{cb}")
                   for cb in range(2)]          # q head pairs: (A, B) slots
            qkT += [qkpool.tile([128, S], dt.float8e4, name=f"qkT{cb}")
                    for cb in range(2, 4)]      # k: single fp8(-8k) copy
            vaug = vpool.tile([128, NT, NH_CORE, 65], dt.bfloat16)
            with nc.allow_low_precision(reason="ones col"):
                nc.vector.memset(vaug[:, :, :, 64], 1.0)
            ctxT = [ctpool.tile([128, S], dt.bfloat16, name=f"ctxT{p}")
                    for p in range(2)]

            # ---------------- phase B: QKV projection ----------------
            # (sw, sx) product chains: x8·W8 + dx8·W8 + x8·dW8, grouped so
            # the x8-only passes run first (dx8 streams in behind x8)
            PASSES = (((0, 0), (1, 0)), ((0, 1),))     # qk: (sw, sx)
            VPASSES = (((0, 0), (0, 1)), ((1, 0),))    # v: (sx, sw)
            DR = mybir.MatmulPerfMode.DoubleRow
            QS = 8.0 / WSCALE   # psum (q*WSCALE) -> stored 8q / -8k

            def qk_store(cb, sl, pg):
                with nc.allow_low_precision(reason="fp8 score operands"):
                    if cb < 2:   # q: slot A, then residual B = 8q - A
                        d0 = qkT[cb][:, 0, sl]
                        nc.vector.tensor_scalar_mul(d0, pg, QS)
                        nc.vector.ln_bwd_dx(qkT[cb][:, 1, sl], pg, d0,
                                            1.0 / QS, 0.0, scale=QS)
                    else:        # k: single fp8(-8k); matmul reads it twice
                        nc.vector.tensor_scalar_mul(qkT[cb][:, sl], pg, -QS)

            def make_xq(s0):
                xq = xqpool.tile([128, 4, 2, 2, 512], dt.float8e4, tag="xq",
                                 name="xq")
                xeng = nc.scalar if xq_eng == 'act' else nc.sync
                xeng.dma_start(out=xq, in_=XT[s0])
                return xq

            def b_items():
                pre = [xq0, make_xq(1)]
                for s0 in range(NBC):
                    xq = pre[0]
                    pre = pre[1:]
                    if s0 + 2 < NBC:
                        pre.append(make_xq(s0 + 2))  # prefetch 2 ahead

                    # chunk 0: two-pass accumulation so the first matmuls
                    # only need the first half of wqk/xq0 (still streaming)
                    if s0 == 0:
                        pgs = {}

                        def qk_half(cb, kh):
                            if kh == 0:
                                pgs[cb] = bigpool.tile(
                                    [128, 512], dt.float32, tag="big",
                                    name="pg")
                            pg = pgs[cb]
                            mm = kh * 6
                            for chains in PASSES:
                                for t in (kh * 2, kh * 2 + 1):
                                    for sw, sx in chains:
                                        nc.tensor.matmul(
                                            pg,
                                            wqk[:, t, sw, :,
                                                cb * 128:(cb + 1) * 128],
                                            xq[:, t, sx, :, :],
                                            start=(mm == 0), stop=(mm == 11),
                                            perf_mode=DR)
                                        mm += 1
                            if kh == 1:
                                qk_store(cb, slice(0, 512), pg)

                        # pairwise interleave: at most 2 open psum groups
                        # (ring=2), first items need only the first halves
                        for cb0 in (0, 2):
                            yield (lambda cb=cb0: qk_half(cb, 0))
                            yield (lambda cb=cb0 + 1: qk_half(cb, 0))
                            yield (lambda cb=cb0: qk_half(cb, 1))
                            yield (lambda cb=cb0 + 1: qk_half(cb, 1))

                        def v_item0(ts):
                            pv = bigpool.tile([128, 4, 64], dt.float32,
                                              tag="big", name="pv")
                            mm = 0
                            for chains in VPASSES:
                                for t in range(4):
                                    for sx, sw in chains:
                                        nc.tensor.matmul(
                                            pv,
                                            xq[:, t, sx, :,
                                               ts * 128:(ts + 1) * 128],
                                            wv[:, t, sw, :, :],
                                            start=(mm == 0), stop=(mm == 11),
                                            perf_mode=DR)
                                        mm += 1
                            veng = (nc.gpsimd if v_eng == 'pool'
                                    else nc.vector)
                            with nc.allow_low_precision(reason="bf16"):
                                veng.tensor_scalar_mul(
                                    vaug[:, ts, :, 0:64], pv, 1.0 / WSCALE)
                        for ts in range(4):
                            yield (lambda ts=ts: v_item0(ts))
                        continue

                    def qk_item(s0=s0, xq=xq, cb=0):
                        pg = bigpool.tile([128, 512], dt.float32, tag="big",
                                          name="pg")
                        mm = 0
                        for chains in PASSES:
                            for t in range(4):
                                for sw, sx in chains:
                                    nc.tensor.matmul(
                                        pg,
                                        wqk[:, t, sw, :,
                                            cb * 128:(cb + 1) * 128],
                                        xq[:, t, sx, :, :],
                                        start=(mm == 0), stop=(mm == 11),
                                        perf_mode=DR)
                                    mm += 1
                        qk_store(cb, slice(s0 * 512, (s0 + 1) * 512), pg)
                    def v_item(s0=s0, xq=xq, ts=0):
                        pv = bigpool.tile([128, 4, 64], dt.float32,
                                          tag="big", name="pv")
                        mm = 0
                        for chains in VPASSES:
                            for t in range(4):
                                for sx, sw in chains:
                                    nc.tensor.matmul(
                                        pv,
                                        xq[:, t, sx, :,
                                           ts * 128:(ts + 1) * 128],
                                        wv[:, t, sw, :, :],
                                        start=(mm == 0), stop=(mm == 11),
                                        perf_mode=DR)
                                    mm += 1
                        st = s0 * 4 + ts
                        veng = nc.gpsimd if v_eng == 'pool' else nc.vector
                        with nc.allow_low_precision(reason="bf16"):
                            veng.tensor_scalar_mul(
                                vaug[:, st, :, 0:64], pv, 1.0 / WSCALE)
                    # interleave qk/v items: v stores are one DVE op, so a
                    # (qk, v) cadence keeps the shared pg/pv psum ring from
                    # stalling on the 2-op qk store chain
                    if b_order == 'interleave':
                        for j in range(4):
                            yield (lambda s0=s0, xq=xq, cb=j:
                                   qk_item(s0, xq, cb))
                            yield (lambda s0=s0, xq=xq, ts=j:
                                   v_item(s0, xq, ts))
                    else:
                        for cb in range(4):
                            yield (lambda s0=s0, xq=xq, cb=cb:
                                   qk_item(s0, xq, cb))
                        for ts in range(4):
                            yield (lambda s0=s0, xq=xq, ts=ts:
                                   v_item(s0, xq, ts))

            b_gen = b_items()
            b_total = 12 + (NBC - 1) * 8   # chunk 0 split into 12 items
            b_emitted = 0

            def emit_b(n):
                done = 0
                for _ in range(n):
                    item = next(b_gen, None)
                    if item is None:
                        break
                    item()
                    done += 1
                return done

            # ---------------- phase C: band attention ----------------
            from collections import deque
            pending = deque()
            _dq = deque()
            _held = []

            cur_stp = [None]

            def score_unit(h, qt):
                pr, po = h // 2, (h % 2) * 64
                gs = [g for g in range(qt - 2, qt + 3) if 0 <= g < NT]
                nA = len(gs)
                stp = sapool.tile([128, 5, 128], dt.float32, tag="stp",
                                  name="stp")
                cur_stp[0] = stp
                for j in range(nA):
                    g = gs[j]
                    # stationary k read twice via a stride-0 slot dim: the
                    # DR slot sum k.(A+B) = k.8q cancels q's fp8 error
                    kap = qkT[2 + pr][po:po + 64, g * 128:(g + 1) * 128]
                    k2 = bass.AP(tensor=kap.tensor, offset=kap.offset,
                                 ap=[kap.ap[0], [0, 2]] + list(kap.ap[1:]))
                    nc.tensor.matmul(
                        stp[:, j, :], k2,
                        qkT[pr][po:po + 64, :, qt * 128:(qt + 1) * 128],
                        perf_mode=DR)
                ptA = ptpool.tile([128, 5, 128], dt.bfloat16, tag="pt",
                                  name="ptA")
                # psum holds -64*qk; negative scale restores exp(qk/8)
                nc.scalar.activation(ptA[:, 0:nA, :], stp[:, 0:nA, :],
                                     mybir.ActivationFunctionType.Exp,
                                     scale=-1.0 / (64.0 * np.sqrt(HD)))
                lo = gs[0] == qt - 2
                up = gs[-1] == qt + 2
                m_eng = 'dve1' if qt >= tail_q else mask_eng
                with nc.allow_low_precision(reason="bf16"):
                    if m_eng == 'dve1' and lo and up:
                        # both triangles in one strided op (slices 0 and 4)
                        nc.vector.tensor_mul(ptA[:, 0:5:4, :],
                                             ptA[:, 0:5:4, :], masks)
                    else:
                        meng = (nc.vector if m_eng in ('dve', 'dve1')
                                else nc.gpsimd)
                        if lo:
                            meng.tensor_mul(ptA[:, 0, :], ptA[:, 0, :],
                                            masks[:, 0, :])
                        if up:
                            meng.tensor_mul(ptA[:, nA - 1, :],
                                            ptA[:, nA - 1, :], masks[:, 1, :])
                return (gs, nA, ptA)

            cn_ref = [None, None]  # per parity: pending pair ctxn tile
            _tq = deque()          # deferred ctxT transpose: (h, qt, ctxn2)

            def pv_unit(h, qt, gs, nA, ptA):
                pr = h // 2
                ctx = cxpool.tile([128, 65], dt.float32, tag="cx",
                                  name="ctx")
                n = len(gs)
                # masked slices (0 and n-1) go last: their mask ops on the
                # mask engine get the longest lead time
                order = list(range(1, n - 1)) + [n - 1, 0] if n > 2 \
                    else list(range(n))
                for i, j in enumerate(order):
                    nc.tensor.matmul(ctx, ptA[:, j, :], vaug[:, gs[j], h, :],
                                     start=(i == 0), stop=(i == n - 1))
                rec = recpool.tile([128, 1], dt.float32, tag="rec",
                                   name="rec")
                nc.vector.reciprocal(rec, ctx[:, 64:65])
                if h % 2 == 0:
                    cn_ref[pr] = cnpool.tile([128, 2, 64], dt.bfloat16,
                                             tag="cn", name="ctxn2")
                ctxn2 = cn_ref[pr]
                with nc.allow_low_precision(reason="bf16"):
                    nc.vector.tensor_scalar_mul(ctxn2[:, h % 2, :],
                                                ctx[:, 0:64], rec)
                _tq.append((h, qt, ctxn2))

            def tp_unit(h, qt, ctxn2):
                # Transpose a head pair's normalized context in one shot:
                # ctxn2 [128 q, 128 pairdims] -> ctxT [128 pairdims, 128 q].
                if h % 2 == 1:
                    pr = h // 2
                    dst = ctxT[pr][:, qt * 128:(qt + 1) * 128]
                    if tp_mode == 'dma':
                        # XBAR DMA transpose: SBUF->SBUF, off both PE and DVE
                        teng = nc.scalar if tp_eng == 'act' else nc.sync
                        teng.dma_start(out=dst, in_=ctxn2, transpose=True)
                    else:
                        if tp_pool == 'big':
                            tp = bigpool.tile([128, 128], dt.bfloat16,
                                              tag="big", name="tp")
                        else:
                            tp = cxpool.tile([128, 128], dt.bfloat16,
                                             tag="cx", name="tp")
                        nc.tensor.transpose(tp, ctxn2, ident)
                        with nc.allow_low_precision(reason="bf16"):
                            nc.vector.tensor_copy(dst, tp)
                if h == NH_CORE - 1:
                    _dq.append(qt)

            def emit_d(qt, split_dma=False, tail=False):
                osb = opool.tile([128, 1024], dt.bfloat16, tag="osb",
                                 name="osb")
                deng = nc.gpsimd if osb_eng == 'gpsimd' else nc.sync
                for nn in range(2):
                    use_cx = (pd_pool == 'cx' or
                              (pd_pool == 'split' and nn == 0))
                    if use_cx:
                        pD = cxpool.tile([128, 512], dt.float32, tag="cx",
                                         name="pD")
                    else:
                        pD = bigpool.tile([128, 512], dt.float32, tag="big",
                                          name="pD")
                    for p in range(2):
                        nc.tensor.matmul(
                            pD, ctxT[p][:, qt * 128:(qt + 1) * 128],
                            wo[:, p, nn * 512:(nn + 1) * 512],
                            start=(p == 0), stop=(p == 1))
                    dst = osb[:, nn * 512:(nn + 1) * 512]
                    use_act = (nn == 0) if osb_copy != 'swap' else (nn == 1)
                    with nc.allow_low_precision(reason="bf16 partials"):
                        if osb_copy == 'pool':
                            nc.gpsimd.tensor_copy(dst, pD)
                        elif osb_copy == 'dve2' or tail or not use_act:
                            nc.vector.tensor_copy(dst, pD)
                        else:
                            nc.scalar.copy(dst, pD)
                    if split_dma:
                        deng.dma_start(
                            out=OUT[qt * 128:(qt + 1) * 128,
                                    nn * 512:(nn + 1) * 512],
                            in_=osb[:, nn * 512:(nn + 1) * 512])
                if not split_dma:
                    deng.dma_start(out=OUT[qt * 128:(qt + 1) * 128, :],
                                   in_=osb)

            # pacing: unit qt needs qkT/vaug through token (qt+2)*128+128,
            # i.e. chunks 0..ceil((qt*128+384)/512)-1 done.
            b_emitted += emit_b(8 * b_prol)
            nc.scalar.dma_start(out=masks, in_=MASKS[:, :, :])
            nc.scalar.dma_start(out=ident, in_=IDENT[:, :])
            nc.scalar.dma_start(out=wo[:, 0, :], in_=WO[0, :, :])
            nc.scalar.dma_start(out=wo[:, 1, :], in_=WO[1, :, :])
            for qt in range(NQT):
                # scores of qt need chunks covering tokens to (qt+3)*128-1,
                # i.e. chunks 0..(qt+2)//4 done; b_slack items of margin.
                need = min(b_total, 12 + 8 * ((qt + 2) // 4) + b_slack)
                # heads in order (0,2,1,3): staggers the two pair-chains
                for h in (0, 2, 1, 3):
                    dd = d_delay if qt < NQT - 2 else 0
                    while _dq and len(_dq) > dd:
                        dqt = _dq.popleft()
                        # park a few mid-sequence o-proj blocks: they become
                        # dependency-free PE work overlapping the final
                        # attention drain
                        if d_hold and len(_held) < d_hold and 16 <= dqt < 28:
                            _held.append(dqt)
                        else:
                            emit_d(dqt, split_dma=(dqt >= NQT - 2),
                                   tail=(dqt >= tail_d))
                    if d_hold and qt >= d_release and _held:
                        emit_d(_held.pop(0), split_dma=True, tail=True)
                    want = need - b_emitted
                    if want > 0:
                        per = max(1, (want + (NH_CORE - h) - 1)
                                  // (NH_CORE - h))
                        if b_cap:
                            per = min(per, b_cap)
                        b_emitted += emit_b(per)
                    if pv_first == 2 and len(_tq) > tp_delay:
                        tp_unit(*_tq.popleft())
                    if pv_first == 1 and len(pending) >= depth:
                        pv_unit(*pending.popleft())
                        pending.append((h, qt) + score_unit(h, qt))
                    else:
                        pending.append((h, qt) + score_unit(h, qt))
                        if len(pending) > depth:
                            pv_unit(*pending.popleft())
                    if pv_first != 2 and len(_tq) > tp_delay:
                        tp_unit(*_tq.popleft())
            while pending:
                pv_unit(*pending.popleft())
                if len(_tq) > 1:
                    tp_unit(*_tq.popleft())
            while _tq:
                tp_unit(*_tq.popleft())
                while _dq:
                    emit_d(_dq.popleft())
            b_emitted += emit_b(b_total)
            while _dq:
                emit_d(_dq.popleft(), split_dma=True)
            for dqt in _held:
                emit_d(dqt, split_dma=True)

    nc.compile()
    return nc


f8 = ml_dtypes.float8_e4m3


def _split8(a):
    """a (f32) -> (a8, da8) fp8e4 with a ~= a8 + da8 (compensated split)."""
    a8 = a.astype(f8)
    d8 = (a - a8.astype(np.float32)).astype(f8)
    return a8, d8


WSCALE = 128.0  # lifts W (and its residual) out of e4m3's subnormal range


def _pack_w8(wcols, ncol):
    """[1024, ncol] f32 -> [128, 4t, 2s, 2i, ncol] fp8 with contraction
    index c = 256t + 128i + p. Weights are pre-scaled by WSCALE; the
    psum->sbuf copy divides it back out."""
    w8, dw8 = _split8(wcols * WSCALE)
    ws = np.stack([w8, dw8])                     # [s, 1024, ncol]
    ws = ws.reshape(2, 4, 2, 128, ncol)          # [s, t, i, p, col]
    return np.ascontiguousarray(ws.transpose(3, 1, 0, 2, 4))


def _prep_fast(x, Wqkv, Wo):
    """Per-core input maps (compensated fp8 QKV operands, bf16 Wo)."""
    xT_b = []
    for b in range(B):
        xt = np.ascontiguousarray(x[b].T)              # [E, S] f32
        x8, dx8 = _split8(xt)
        xs = np.stack([x8, dx8])                       # [s, E, S]
        xs = xs.reshape(2, 4, 2, 128, 8, 512)          # [s, t, i, p, s0, tok]
        xT_b.append(np.ascontiguousarray(xs.transpose(4, 3, 1, 0, 2, 5)))
        # xT_b[b][s0, p, t, s, i, tok] = xs[s, 256t+128i+p, 512*s0+tok]
    in_maps = []
    for c in range(8):
        b, hg = c // 4, c % 4
        heads = range(4 * hg, 4 * hg + 4)
        qcols = np.concatenate([np.arange(h * 192, h * 192 + 64)
                                for h in heads])
        kcols = qcols + 64
        vcols = qcols + 128
        wqk_cols = np.concatenate([qcols, kcols])           # [512]
        wqk = _pack_w8(Wqkv[:, wqk_cols], 512)
        wv = _pack_w8(Wqkv[:, vcols], 256)
        orows = np.concatenate([np.arange(h * 64, h * 64 + 64)
                                for h in heads])
        wo = np.ascontiguousarray(Wo[orows].reshape(2, 128, 1024)).astype(bf16)
        in_maps.append({"xT": xT_b[b], "wqk": wqk, "wv": wv, "wo": wo})
    return in_maps



def _build_generic(vbias=True, st_bufs=2, po_bufs=1, bc_bufs=1, cx_bufs=2,
           mask_eng='dve', bccopy_eng='act', pt_bufs=8,
           osbcopy_eng='dve', bcast_via='pe', paired=True, depth=1,
           fuse_b=True, pb_bufs=2, b_lead=3, b_prol=2,
           norm_src='sbuf', ctxcopy_eng='act'):
    if fuse_b:
        pt_bufs = min(pt_bufs, 6)
    _nb = 2 if fuse_b else 3
    nc = bacc.Bacc("TRN2", target_bir_lowering=False, debug=False, num_devices=8)

    XT = nc.dram_tensor("xT", [128, 16, 8, 256], dt.float32r, kind="ExternalInput")
    WQK = nc.dram_tensor("wqk", [128, 8, 4, 128], dt.float32r, kind="ExternalInput")
    WV = nc.dram_tensor("wv", [128, 8, 256], dt.float32r, kind="ExternalInput")
    WO = nc.dram_tensor("wo", [2, 128, 1024], dt.float32r, kind="ExternalInput")
    BQK = nc.dram_tensor("bqk", [128, 4], dt.float32, kind="ExternalInput")
    BV = nc.dram_tensor("bv", [1, 256], dt.float32, kind="ExternalInput")
    MV8 = nc.dram_tensor("mv8", [128, 32], dt.float32, kind="ExternalInput")
    OUT = nc.dram_tensor("out", [S, E], dt.float32, kind="ExternalOutput")

    # constant 0/1 triangular band masks for u in {-2,-1,2,3}
    p_i = np.arange(128)[:, None]
    r_i = np.arange(256)[None, :]
    mask_np = {}
    for u in (-2, -1, 2, 3):
        mask_np[u] = ((u * 128 + p_i - r_i >= -w) & (u * 128 + p_i - r_i <= w)
                      ).astype(np.float32)
    MASKS = nc.inline_tensor(
        np.ascontiguousarray(
            np.stack([mask_np[u] for u in (-2, -1, 2, 3)]).transpose(1, 0, 2)),
        name="trimasks")
    ONES = nc.inline_tensor(np.ones((1, 128), dtype=np.float32), name="onesrow")

    with tile.TileContext(nc) as tc:
        with tc.tile_pool(name="const", bufs=1) as cpool, \
             tc.tile_pool(name="qkT", bufs=1) as qkpool, \
             tc.tile_pool(name="vaug", bufs=1) as vpool, \
             tc.tile_pool(name="ctxT", bufs=1) as ctxpool:

            wo = [cpool.tile([128, 1024], dt.float32r, name=f"wo{p}") for p in range(2)]
            bqk = cpool.tile([128, 4], dt.float32)
            nc.gpsimd.dma_start(out=bqk, in_=BQK[:, :])
            bv_f = cpool.tile([1, 256], dt.float32)
            nc.gpsimd.dma_start(out=bv_f, in_=BV[:, :])
            mv8 = cpool.tile([128, 32], dt.float32)
            nc.gpsimd.dma_start(out=mv8, in_=MV8[:, :])
            masks = cpool.tile([128, 4, 256], dt.float32)
            mask_idx = {-2: 0, -1: 1, 2: 2, 3: 3}
            ones_f = cpool.tile([1, 128], dt.float32)
            nc.gpsimd.dma_start(out=ones_f, in_=ONES[:, :])
            ones_r = cpool.tile([1, 128], dt.float32r)
            bv_r = cpool.tile([1, 256], dt.float32r)
            with nc.allow_low_precision(reason="f32r matmul pipeline"):
                nc.vector.tensor_copy(ones_r, ones_f)
                nc.vector.tensor_copy(bv_r, bv_f)

            # persistent intermediates
            qkT = [qkpool.tile([128, S], dt.float32r, name=f"qkT{cb}")
                   for cb in range(4)]  # 0,1: q pairs; 2,3: k pairs
            vaug = [vpool.tile([128, NT, 65], dt.float32r, name=f"vaug{h}")
                    for h in range(NH_CORE)]
            ones32 = cpool.tile([128, NT], dt.float32)
            nc.vector.memset(ones32, 1.0)
            for h in range(NH_CORE):
                with nc.allow_low_precision(reason="f32r"):
                    nc.vector.tensor_copy(vaug[h][:, :, 64], ones32)
            ctxT = [ctxpool.tile([128, S], dt.float32r, name=f"ctxT{p}")
                    for p in range(2)]

            # ---------------- Phase B: QKV projection ----------------
            # Emitted either up front (fuse_b=False) or as fine-grained work
            # items interleaved into the attention loop's idle PE slots.
            bwpool = ctx_pools = None
            import contextlib
            _bstack = contextlib.ExitStack()
            bwpool = _bstack.enter_context(tc.tile_pool(name="bw", bufs=1))
            xqpool = _bstack.enter_context(
                tc.tile_pool(name="xq", bufs=(2 if fuse_b else 3)))
            pbpool = _bstack.enter_context(
                tc.tile_pool(name="pb", bufs=(pb_bufs if fuse_b else 8),
                             space="PSUM"))
            wqk = bwpool.tile([128, 8, 4, 128], dt.float32r)
            wv = bwpool.tile([128, 8, 256], dt.float32r)
            xq0 = [xqpool.tile([128, 4, 256], dt.float32r, tag=f"xq{i}",
                               name="xq") for i in range(2)]
            for i in range(2):
                nc.sync.dma_start(out=xq0[i], in_=XT[:, 0, i * 4:(i + 1) * 4, :])
            for kt in range(8):
                nc.sync.dma_start(out=wqk[:, kt, :, :], in_=WQK[:, kt, :, :])
            nc.sync.dma_start(out=wv[:, 0:4, :], in_=WV[:, 0:4, :])
            nc.sync.dma_start(out=wv[:, 4:8, :], in_=WV[:, 4:8, :])

            def b_items():
                for s0 in range(16):  # 256-token chunks of S
                    if s0 == 0:
                        xq = xq0
                    else:
                        xq = [xqpool.tile([128, 4, 256], dt.float32r,
                                          tag=f"xq{i}", name="xq")
                              for i in range(2)]
                        for i in range(2):
                            nc.sync.dma_start(
                                out=xq[i], in_=XT[:, s0, i * 4:(i + 1) * 4, :])

                    def qk_item(s0=s0, xq=xq, cb=0):
                        pg = pbpool.tile([128, 256], dt.float32, tag="pb",
                                         name="pqk")
                        for k8 in range(8):
                            nc.tensor.matmul(pg, wqk[:, k8, cb, :],
                                             xq[k8 // 4][:, k8 % 4, :],
                                             start=(k8 == 0), stop=(k8 == 7))
                        nc.scalar.activation(
                            qkT[cb][:, s0 * 256:(s0 + 1) * 256], pg,
                            mybir.ActivationFunctionType.Identity,
                            bias=bqk[:, cb:cb + 1])
                    for cb in range(4):
                        yield (lambda s0=s0, xq=xq, cb=cb: qk_item(s0, xq, cb))

                    def v_item(s0=s0, xq=xq, hf=0):
                        pv = pbpool.tile([128, 256], dt.float32, tag="pb",
                                         name="pv")
                        for k8 in range(8):
                            nc.tensor.matmul(
                                pv,
                                xq[k8 // 4][:, k8 % 4, hf * 128:(hf + 1) * 128],
                                wv[:, k8, :], start=(k8 == 0),
                                stop=(k8 == 7 and not vbias))
                        if vbias:
                            nc.tensor.matmul(pv, ones_r, bv_r,
                                             start=False, stop=True)
                        st = s0 * 2 + hf
                        for h in range(NH_CORE):
                            with nc.allow_low_precision(reason="f32r"):
                                nc.vector.tensor_copy(
                                    vaug[h][:, st, 0:64],
                                    pv[:, h * 64:(h + 1) * 64])
                    for hf in range(2):
                        yield (lambda s0=s0, xq=xq, hf=hf: v_item(s0, xq, hf))

            b_gen = b_items()
            b_total = 16 * 6
            b_emitted = 0

            def emit_b(n):
                emitted = 0
                for _ in range(n):
                    item = next(b_gen, None)
                    if item is None:
                        break
                    item()
                    emitted += 1
                return emitted

            if not fuse_b:
                b_emitted += emit_b(b_total)
                _bstack.close()

            nc.gpsimd.dma_start(out=masks, in_=MASKS[:, :, :])
            for p in range(2):
                nc.gpsimd.dma_start(out=wo[p], in_=WO[p, :, :])
            # ------- Phase C: band attention, with output projection folded in -------
            import contextlib
            _cstack = contextlib.ExitStack()
            with _cstack:
                stpool = _cstack.enter_context(
                    tc.tile_pool(name="stp", bufs=st_bufs, space="PSUM"))
                cxpool = _cstack.enter_context(
                    tc.tile_pool(name="ctxp", bufs=cx_bufs, space="PSUM"))
                if bcast_via == 'pe':
                    bcpool = _cstack.enter_context(
                        tc.tile_pool(name="bcp", bufs=bc_bufs, space="PSUM"))
                else:
                    drpool = _cstack.enter_context(
                        tc.tile_pool(name="dr", bufs=4, space="DRAM"))
                popool = _cstack.enter_context(
                    tc.tile_pool(name="po", bufs=po_bufs, space="PSUM"))
                ptpool = _cstack.enter_context(
                    tc.tile_pool(name="pt", bufs=pt_bufs))
                bcsb = _cstack.enter_context(tc.tile_pool(name="bcs", bufs=_nb))
                opool = _cstack.enter_context(tc.tile_pool(name="osb", bufs=2))
                rcpool = _cstack.enter_context(tc.tile_pool(name="rcp", bufs=_nb))

                def score_stage(h, cc):
                    # returns list of (gts, pt, jslices) where pt holds exp'd
                    # probabilities for the key tiles in gts
                    pr, po = h // 2, (h % 2) * 64
                    out = []
                    if paired:
                        # all-ones padding: exp has no per-key bias, so key
                        # tiles are processed in aligned pairs (one psum bank,
                        # one exp, one mask-mul per pair)
                        for ub in (-2, 0, 2):
                            gts = [2 * cc + ub, 2 * cc + ub + 1]
                            if gts[0] < 0 or gts[1] >= NT:
                                continue
                            stp = stpool.tile([128, 2, 256], dt.float32,
                                              tag="st", name="stp")
                            for j, gt in enumerate(gts):
                                nc.tensor.matmul(
                                    stp[:, j, :],
                                    qkT[2 + pr][po:po + 64,
                                                gt * 128:(gt + 1) * 128],
                                    qkT[pr][po:po + 64,
                                            cc * 256:(cc + 1) * 256])
                            pt = ptpool.tile([128, 2, 256], dt.float32r,
                                             tag="pt", name="pt")
                            nc.scalar.activation(
                                pt, stp, mybir.ActivationFunctionType.Exp,
                                scale=1.0 / np.sqrt(HD))
                            if ub != 0:
                                mi = 0 if ub == -2 else 2
                                with nc.allow_low_precision(reason="f32r"):
                                    eng = (nc.gpsimd if mask_eng == 'gpsimd'
                                           else nc.vector)
                                    eng.tensor_mul(pt, pt,
                                                   masks[:, mi:mi + 2, :])
                            out.append((gts, pt))
                        return out
                    for u in range(-2, 4):
                        gt = 2 * cc + u
                        if not 0 <= gt < NT:
                            continue
                        stp = stpool.tile([128, 256], dt.float32, tag="st",
                                          name="stp")
                        nc.tensor.matmul(
                            stp,
                            qkT[2 + pr][po:po + 64, gt * 128:(gt + 1) * 128],
                            qkT[pr][po:po + 64, cc * 256:(cc + 1) * 256])
                        pt = ptpool.tile([128, 256], dt.float32r, tag="pt",
                                         name="pt")
                        nc.scalar.activation(pt, stp,
                                             mybir.ActivationFunctionType.Exp,
                                             bias=mv8[:, gt:gt + 1],
                                             scale=1.0 / np.sqrt(HD))
                        if u in mask_idx:
                            with nc.allow_low_precision(reason="f32r"):
                                eng = (nc.gpsimd if mask_eng == 'gpsimd'
                                       else nc.vector)
                                eng.tensor_mul(pt, pt,
                                               masks[:, mask_idx[u], :])
                        out.append(([gt], pt))
                    return out

                def pv_stage(h, cc, pts):
                    if _dq:
                        emit_d(_dq.popleft())
                    pr, po = h // 2, (h % 2) * 64
                    ctx = cxpool.tile([65, 256], dt.float32, tag="cx",
                                      name="ctx")
                    nmm = sum(len(gts) for gts, _ in pts)
                    j = 0
                    for gts, pt in pts:
                        for jj, gt in enumerate(gts):
                            rhs = pt[:, jj, :] if len(gts) > 1 else pt
                            nc.tensor.matmul(ctx, vaug[h][:, gt, :], rhs,
                                             start=(j == 0),
                                             stop=(j == nmm - 1))
                            j += 1
                    if norm_src == 'sbuf':
                        # copy ctx out of PSUM first: frees the cx slot early
                        # and the final multiply reads bc straight from PSUM
                        cxs = bcsb.tile([65, 256], dt.float32, tag="bcs",
                                        name="cxs")
                        if ctxcopy_eng == 'act':
                            nc.scalar.copy(cxs, ctx)
                        else:
                            nc.vector.tensor_copy(cxs, ctx)
                        ctx = cxs
                    rec = rcpool.tile([1, 256], dt.float32r, tag="rc",
                                      name="rec")
                    with nc.allow_low_precision(reason="f32r"):
                        nc.vector.reciprocal(rec, ctx[64:65, :])
                    bcs = None
                    if norm_src != 'sbuf':
                        bcs = bcsb.tile([64, 256], dt.float32, tag="bcs",
                                        name="bcs")
                    if bcast_via == 'dma':
                        drec = drpool.tile([1, 256], dt.float32r, tag="dr",
                                           name="drec")
                        nc.sync.dma_start(out=drec, in_=rec)
                        dbc = bass.AP(tensor=drec.tensor, offset=drec.offset,
                                      ap=[[0, 64]] + drec.ap[1:])
                        nc.sync.dma_start(out=bcs.bitcast(dt.float32r), in_=dbc)
                    else:
                        bc = bcpool.tile([64, 256], dt.float32, tag="bc",
                                         name="bc")
                        nc.tensor.matmul(bc, ones_r[:, 0:64], rec)
                        if norm_src == 'sbuf':
                            bcs = bc
                        elif bccopy_eng == 'act':
                            nc.scalar.copy(bcs, bc)
                        else:
                            nc.vector.tensor_copy(bcs, bc)
                    with nc.allow_low_precision(reason="f32r"):
                        nc.vector.tensor_mul(
                            ctxT[pr][po:po + 64, cc * 256:(cc + 1) * 256],
                            ctx[0:64, :], bcs)
                    if h == NH_CORE - 1:
                        _dq.append(2 * cc)
                        _dq.append(2 * cc + 1)

                def emit_d(qt):
                    osb = opool.tile([128, 1024], dt.float32, tag="osb",
                                     name="osb")
                    for nn in range(2):
                        pD = popool.tile([128, 512], dt.float32, tag="po",
                                         name="pD")
                        for p in range(2):
                            nc.tensor.matmul(pD,
                                             ctxT[p][:, qt * 128:(qt + 1) * 128],
                                             wo[p][:, nn * 512:(nn + 1) * 512],
                                             start=(p == 0), stop=(p == 1))
                        if osbcopy_eng == 'act':
                            nc.scalar.copy(osb[:, nn * 512:(nn + 1) * 512], pD)
                        else:
                            nc.vector.tensor_copy(osb[:, nn * 512:(nn + 1) * 512], pD)
                    nc.gpsimd.dma_start(out=OUT[qt * 128:(qt + 1) * 128, :],
                                        in_=osb)

                from collections import deque
                pending = deque()
                _dq = deque()
                if fuse_b:
                    # prologue: cover key tiles for the first two query chunks
                    b_emitted += emit_b(6 * b_prol)
                step = 0
                for cc in range(NCC):
                    for h in range(NH_CORE):
                        if fuse_b:
                            # pace remaining B so chunk cc+2 is done before
                            # attention chunk cc+1 starts
                            target = min(b_total, 6 * (cc + b_lead))
                            want = target - b_emitted
                            per = max(1, (want + (NH_CORE - h) - 1)
                                      // (NH_CORE - h))
                            if want > 0:
                                b_emitted += emit_b(per)
                        pts = score_stage(h, cc)
                        pending.append((h, cc, pts))
                        if len(pending) > depth:
                            pv_stage(*pending.popleft())
                        step += 1
                while pending:
                    pv_stage(*pending.popleft())
                while _dq:
                    emit_d(_dq.popleft())
                if fuse_b:
                    b_emitted += emit_b(b_total)

            _bstack.close()

    nc.compile()
    return nc



def _prep_generic(x, Wqkv, bqkv, Wo, pm):
    in_maps = []
    xT_b = []
    for b in range(B):
        xt = np.ascontiguousarray(x[b].T)                      # [E, S]
        xT_b.append(np.ascontiguousarray(
            xt.reshape(8, 128, 16, 256).transpose(1, 2, 0, 3)))
    mv8_b = []
    for b in range(B):
        # mv8[p, t] = (0 if valid else NEG)/8 for key index t*128+p
        mv = np.where(pm[b], 0.0, NEG).astype(np.float32) / 8.0
        mv8_b.append(np.ascontiguousarray(mv.reshape(32, 128).T))

    for c in range(8):
        b, hg = c // 4, c % 4
        heads = range(4 * hg, 4 * hg + 4)
        qcols = np.concatenate([np.arange(h * 192, h * 192 + 64) for h in heads])
        kcols = qcols + 64
        vcols = qcols + 128
        wqk_cols = np.concatenate([qcols, kcols])               # [512]
        wqk = np.ascontiguousarray(
            Wqkv[:, wqk_cols].reshape(8, 128, 4, 128).transpose(1, 0, 2, 3))
        wv = np.ascontiguousarray(
            Wqkv[:, vcols].reshape(8, 128, 256).transpose(1, 0, 2))
        orows = np.concatenate([np.arange(h * 64, h * 64 + 64) for h in heads])
        wo = np.ascontiguousarray(Wo[orows].reshape(2, 128, 1024))
        in_maps.append({
            "xT": xT_b[b],
            "wqk": wqk,
            "wv": wv,
            "wo": wo,
            "bqk": np.ascontiguousarray(bqkv[wqk_cols].reshape(4, 128).T),
            "bv": np.ascontiguousarray(bqkv[vcols].reshape(1, 256)),
            "mv8": mv8_b[b],
        })
    return in_maps


def kernel(x, Wqkv, bqkv, Wo, bo, padding_mask, num_heads, window_size):
    assert int(num_heads) == H and int(window_size) == W
    x = np.asarray(x, dtype=np.float32)
    Wqkv = np.asarray(Wqkv, dtype=np.float32)
    bqkv = np.asarray(bqkv, dtype=np.float32)
    Wo = np.asarray(Wo, dtype=np.float32)
    bo = np.asarray(bo, dtype=np.float32)
    pm = np.asarray(padding_mask).astype(bool)
    assert x.shape == (B, S, E)

    fast = bool(pm.all()) and not np.any(bqkv)
    if fast:
        if "fast" not in _cache:
            _cache["fast"] = _build_fast(depth=3, tp_delay=1, d_delay=2,
                                         warm_n=8, osb_eng='sync',
                                         pd_pool='split', xq_eng='sync',
                                         mask_eng='gpsimd', b_slack=1,
                                         b_prol=1, pv_first=2, tail_q=26)
        nc = _cache["fast"]
        in_maps = _prep_fast(x, Wqkv, Wo)
    else:
        vbias = bool(np.any(bqkv.reshape(H, 3, HD)[:, 2, :] != 0.0))
        key = ("nc", vbias, False)
        if key not in _cache:
            _cache[key] = _build_generic(vbias=vbias, paired=False)
        nc = _cache[key]
        in_maps = _prep_generic(x, Wqkv, bqkv, Wo, pm)

    res = run_bass_kernel_spmd(nc, in_maps, list(range(8)))
    kernel._last_results = res

    out = np.empty((B, S, E), dtype=np.float32)
    for b in range(B):
        acc = res.results[4 * b]["out"].astype(np.float32)
        for g in range(1, 4):
            acc = acc + res.results[4 * b + g]["out"].astype(np.float32)
        out[b] = acc + bo
    return out



# revision 65
# speedup vs baseline: 1.1522x; 1.0055x over previous
"""Sliding-window multi-head attention (Longformer-style band attention) for
Trainium2, distributed over 8 NeuronCores.

Sharding: data-parallel over batch (B=2) x tensor-parallel over heads
(16 heads -> 4 groups of 4). Core c handles batch c//4, heads
[4*(c%4), 4*(c%4)+4). Each core computes the QKV projection for its head
group, band attention over 128-key tiles, and a partial output projection;
the host sums the 4 partials per batch and adds bo.

Fast path (all-ones padding mask, zero qkv bias): mixed fp8/bf16 datapath
tuned against the TRN2 cost model, where fp8e4 DoubleRow matmuls process two
128-row contraction slices per instruction at 0.5 cycles/row (4x bf16 for
deep contractions).

- QKV projection: error-compensated fp8 DoubleRow. Host splits x = x8 + dx8
  and W*128 = W8 + dW8 (the *128 lifts W and its residual out of e4m3's
  subnormal range; the psum->sbuf copy divides it back). Three product
  chains x8W8 + dx8W8 + x8dW8 run at 0.75x the bf16 PE cost with bf16-grade
  accuracy.
- Scores: half-compensated fp8 DoubleRow at 0.5x bf16 PE cost. q is stored
  as slot pair A = fp8(8q), B = fp8(8q - A) (one tensor_scalar + one
  LN_BWD_DX custom-DVE op); k as a single fp8(-8k) copy that the matmul's
  stationary operand reads twice via a stride-0 slot dim. The DoubleRow
  slot sum k.(A+B) = k.8q cancels q's quantization error exactly; only k's
  single-fp8 error (~1.3% end to end) remains. Score psum = -64*qk, undone
  by a negative exp scale.
- exp on the scalar engine into bf16 probabilities, triangular band masks
  on gpsimd (vector in the post-QKV tail, where gpsimd saturates), PV in
  bf16 as [query, 65] with an appended ones column giving the softmax
  denominator for free. PV/o-proj stay bf16: with only ~2e-2 error budget,
  single-sided fp8 fails there and full compensation costs the same as
  bf16.
- Normalization, PE-transpose and the bf16 output projection as before;
  QKV work items are paced into the attention loop to keep PE saturated,
  warm-up matmuls absorb the PE clock ramp.

Generic path (padding masks / nonzero qkv bias) falls back to an f32r
implementation of the same blocking.
"""
import sys
import numpy as np
import ml_dtypes

try:
    import concourse.bass as bass
except ImportError:
    sys.path.insert(0, "/opt/trn_rl_repo")
    import concourse.bass as bass
import concourse.mybir as mybir
import concourse.tile as tile
from concourse import bacc
from concourse.bass_utils import run_bass_kernel_spmd

dt = mybir.dt
bf16 = ml_dtypes.bfloat16

B, S, E, H, W = 2, 4096, 1024, 16, 512
HD = E // H          # 64
NH_CORE = 4
w = W // 2           # 256
NT = S // 128        # 32 key tiles of 128
NQT = S // 128       # 32 query tiles of 128
NBC = S // 512       # 8 qkv token chunks of 512
NCC = S // 256       # generic path: 16 query chunks of 256
NEG = -9e15

_cache = {}


def _build_fast(depth=2, b_prol=2, mask_eng='dve', qkcopy_eng='dve',
                d_delay=1, cx_bufs=2, big_bufs=2, tp_delay=1, warm_n=0,
                osb_eng='gpsimd', pd_pool='big', xq_eng='sync', b_slack=8,
                d_hold=0, tp_pool='cx', pv_first=0, st_bufs=2,
                osb_copy='mixed', v_eng='dve', d_release=99, tail_q=99,
                tail_d=99, b_order='seq', b_cap=0, tp_mode='pe',
                tp_eng='sync', q_via='psum'):
    nc = bacc.Bacc("TRN2", target_bir_lowering=False, debug=False,
                   num_devices=8)

    # fp8 DoubleRow with host-side error compensation: x = x8 + dx8 and
    # W = W8 + dW8 (each fp8e4); three product chains x8W8 + x8dW8 + dx8W8
    # restore bf16-grade accuracy at 0.75x the bf16 PE cost (DoubleRow
    # contracts 256 rows per instruction at 0.5 cycles/row).
    # Layouts: [partition p, t (256-row ktile), s (main/residual), i
    # (DoubleRow pair), cols] with contraction index c = 256t + 128i + p.
    XT = nc.dram_tensor("xT", [8, 128, 4, 2, 2, 512], dt.float8e4,
                        kind="ExternalInput")
    WQK = nc.dram_tensor("wqk", [128, 4, 2, 2, 512], dt.float8e4,
                         kind="ExternalInput")
    WV = nc.dram_tensor("wv", [128, 4, 2, 2, 256], dt.float8e4,
                        kind="ExternalInput")
    WO = nc.dram_tensor("wo", [2, 128, 1024], dt.bfloat16,
                        kind="ExternalInput")
    OUT = nc.dram_tensor("out", [S, E], dt.bfloat16, kind="ExternalOutput")

    p_i = np.arange(128)[:, None]
    c_i = np.arange(128)[None, :]
    lo = (p_i >= c_i).astype(bf16)   # tile g==qt-2: valid kr >= qr
    up = (p_i <= c_i).astype(bf16)   # tile g==qt+2: valid kr <= qr
    MASKS = nc.inline_tensor(np.ascontiguousarray(
        np.stack([lo, up], axis=1)), name="trimasks")   # [128, 2, 128]
    IDENT = nc.inline_tensor(np.eye(128, dtype=bf16), name="ident")

    with tile.TileContext(nc) as tc:
        with tc.tile_pool(name="const", bufs=1) as cpool, \
             tc.tile_pool(name="qkTp", bufs=1) as qkpool, \
             tc.tile_pool(name="vaugp", bufs=1) as vpool, \
             tc.tile_pool(name="ctxTp", bufs=1) as ctpool, \
             tc.tile_pool(name="xq", bufs=4) as xqpool, \
             tc.tile_pool(name="qb", bufs=3) as qbpool, \
             tc.tile_pool(name="pt", bufs=7) as ptpool, \
             tc.tile_pool(name="recp", bufs=4) as recpool, \
             tc.tile_pool(name="cnp", bufs=4) as cnpool, \
             tc.tile_pool(name="osbp", bufs=3) as opool, \
             tc.tile_pool(name="stp", bufs=st_bufs, space="PSUM") as sapool, \
             tc.tile_pool(name="cxp", bufs=cx_bufs, space="PSUM") as cxpool, \
             tc.tile_pool(name="bigp", bufs=big_bufs, space="PSUM") as bigpool:

            # ---- constants / weights ----
            wqk = cpool.tile([128, 4, 2, 2, 512], dt.float8e4)
            wv = cpool.tile([128, 4, 2, 2, 256], dt.float8e4)
            wo = cpool.tile([128, 2, 1024], dt.bfloat16)
            masks = cpool.tile([128, 2, 128], dt.bfloat16)
            ident = cpool.tile([128, 128], dt.bfloat16)
            # t-slice granularity so the first QKV matmuls start early
            # (subtile deps gate each accumulation step on its own slice);
            # scalar-engine HWDGE triggers: cheap and off the SP queue.
            # wqk/xq0 slices interleaved so slice pairs land together.
            xq0 = xqpool.tile([128, 4, 2, 2, 512], dt.float8e4, tag="xq",
                              name="xq")
            for kh in range(2):
                ks = slice(kh * 2, kh * 2 + 2)
                nc.scalar.dma_start(out=wqk[:, ks], in_=WQK[:, ks])
                nc.sync.dma_start(out=xq0[:, ks], in_=XT[0, :, ks])
            nc.scalar.dma_start(out=wv, in_=WV[:, :, :, :, :])
            # masks/ident/wo are issued after the b-item prologue (below) so
            # the serial DMA pipe delivers the chunk-1/2 x prefetches first;
            # they aren't read until the first score/transpose/o-proj units.

            # PE warmup: scratch matmuls absorb the p-state ramp while the
            # first input DMAs are still streaming in.
            if warm_n:
                wsrc = cpool.tile([128, 512], dt.bfloat16)
                nc.vector.memset(wsrc, 0.0)
                wdst = bigpool.tile([128, 512], dt.float32, tag="big",
                                    name="wdst")
                for i in range(warm_n):
                    nc.tensor.matmul(wdst, wsrc[:, 0:128], wsrc)

            # ---- persistent intermediates ----
            # q/k stored as fp8 DoubleRow slot pairs (scores run in fp8-DR
            # at half the bf16 PE cost). q: slot0 A = fp8(8q), slot1
            # B = fp8(8q - A); k: both slots fp8(-8k). The DR slot sum
            # k_n*A + k_n*B = k_n*8q cancels q's quantization error exactly;
            # only k's single-fp8 error remains. Score psum = -64*qk, undone
            # by a negative exp scale.
            qkT = [qkpool.tile([128, 2, S], dt.float8e4, name=f"qkT{cb}")
                   for cb in range(2)]          # q head pairs: (A, B) slots
            qkT += [qkpool.tile([128, S], dt.float8e4, name=f"qkT{cb}")
                    for cb in range(2, 4)]      # k: single fp8(-8k) copy
            vaug = vpool.tile([128, NT, NH_CORE, 65], dt.bfloat16)
            with nc.allow_low_precision(reason="ones col"):
                nc.vector.memset(vaug[:, :, :, 64], 1.0)
            ctxT = [ctpool.tile([128, S], dt.bfloat16, name=f"ctxT{p}")
                    for p in range(2)]

            # ---------------- phase B: QKV projection ----------------
            # (sw, sx) product chains: x8·W8 + dx8·W8 + x8·dW8, grouped so
            # the x8-only passes run first (dx8 streams in behind x8)
            PASSES = (((0, 0), (1, 0)), ((0, 1),))     # qk: (sw, sx)
            VPASSES = (((0, 0), (0, 1)), ((1, 0),))    # v: (sx, sw)
            DR = mybir.MatmulPerfMode.DoubleRow
            QS = 8.0 / WSCALE   # psum (q*WSCALE) -> stored 8q / -8k

            def qk_store(cb, sl, pg):
                with nc.allow_low_precision(reason="fp8 score operands"):
                    if cb < 2:   # q: slot A, then residual B = 8q - A
                        d0 = qkT[cb][:, 0, sl]
                        if q_via == 'bf16':
                            # stage through SBUF bf16 so the psum buffer
                            # frees after one op, not the 2-op split chain
                            qbf = qbpool.tile([128, 512], dt.bfloat16,
                                              tag="qb", name="qbf")
                            nc.vector.tensor_copy(qbf, pg)
                            nc.vector.tensor_scalar_mul(d0, qbf, QS)
                            nc.vector.ln_bwd_dx(qkT[cb][:, 1, sl], qbf, d0,
                                                1.0 / QS, 0.0, scale=QS)
                        else:
                            nc.vector.tensor_scalar_mul(d0, pg, QS)
                            nc.vector.ln_bwd_dx(qkT[cb][:, 1, sl], pg, d0,
                                                1.0 / QS, 0.0, scale=QS)
                    else:        # k: single fp8(-8k); matmul reads it twice
                        nc.vector.tensor_scalar_mul(qkT[cb][:, sl], pg, -QS)

            def make_xq(s0):
                xq = xqpool.tile([128, 4, 2, 2, 512], dt.float8e4, tag="xq",
                                 name="xq")
                xeng = nc.scalar if xq_eng == 'act' else nc.sync
                xeng.dma_start(out=xq, in_=XT[s0])
                return xq

            def b_items():
                pre = [xq0, make_xq(1)]
                for s0 in range(NBC):
                    xq = pre[0]
                    pre = pre[1:]
                    if s0 + 2 < NBC:
                        pre.append(make_xq(s0 + 2))  # prefetch 2 ahead

                    # chunk 0: two-pass accumulation so the first matmuls
                    # only need the first half of wqk/xq0 (still streaming)
                    if s0 == 0:
                        pgs = {}

                        def qk_half(cb, kh):
                            if kh == 0:
                                pgs[cb] = bigpool.tile(
                                    [128, 512], dt.float32, tag="big",
                                    name="pg")
                            pg = pgs[cb]
                            mm = kh * 6
                            for chains in PASSES:
                                for t in (kh * 2, kh * 2 + 1):
                                    for sw, sx in chains:
                                        nc.tensor.matmul(
                                            pg,
                                            wqk[:, t, sw, :,
                                                cb * 128:(cb + 1) * 128],
                                            xq[:, t, sx, :, :],
                                            start=(mm == 0), stop=(mm == 11),
                                            perf_mode=DR)
                                        mm += 1
                            if kh == 1:
                                qk_store(cb, slice(0, 512), pg)

                        # pairwise interleave: at most 2 open psum groups
                        # (ring=2), first items need only the first halves
                        for cb0 in (0, 2):
                            yield (lambda cb=cb0: qk_half(cb, 0))
                            yield (lambda cb=cb0 + 1: qk_half(cb, 0))
                            yield (lambda cb=cb0: qk_half(cb, 1))
                            yield (lambda cb=cb0 + 1: qk_half(cb, 1))

                        def v_item0(ts):
                            pv = bigpool.tile([128, 4, 64], dt.float32,
                                              tag="big", name="pv")
                            mm = 0
                            for chains in VPASSES:
                                for t in range(4):
                                    for sx, sw in chains:
                                        nc.tensor.matmul(
                                            pv,
                                            xq[:, t, sx, :,
                                               ts * 128:(ts + 1) * 128],
                                            wv[:, t, sw, :, :],
                                            start=(mm == 0), stop=(mm == 11),
                                            perf_mode=DR)
                                        mm += 1
                            veng = (nc.gpsimd if v_eng == 'pool'
                                    else nc.vector)
                            with nc.allow_low_precision(reason="bf16"):
                                veng.tensor_scalar_mul(
                                    vaug[:, ts, :, 0:64], pv, 1.0 / WSCALE)
                        for ts in range(4):
                            yield (lambda ts=ts: v_item0(ts))
                        continue

                    def qk_item(s0=s0, xq=xq, cb=0):
                        pg = bigpool.tile([128, 512], dt.float32, tag="big",
                                          name="pg")
                        mm = 0
                        for chains in PASSES:
                            for t in range(4):
                                for sw, sx in chains:
                                    nc.tensor.matmul(
                                        pg,
                                        wqk[:, t, sw, :,
                                            cb * 128:(cb + 1) * 128],
                                        xq[:, t, sx, :, :],
                                        start=(mm == 0), stop=(mm == 11),
                                        perf_mode=DR)
                                    mm += 1
                        qk_store(cb, slice(s0 * 512, (s0 + 1) * 512), pg)
                    def v_item(s0=s0, xq=xq, ts=0):
                        pv = bigpool.tile([128, 4, 64], dt.float32,
                                          tag="big", name="pv")
                        mm = 0
                        for chains in VPASSES:
                            for t in range(4):
                                for sx, sw in chains:
                                    nc.tensor.matmul(
                                        pv,
                                        xq[:, t, sx, :,
                                           ts * 128:(ts + 1) * 128],
                                        wv[:, t, sw, :, :],
                                        start=(mm == 0), stop=(mm == 11),
                                        perf_mode=DR)
                                    mm += 1
                        st = s0 * 4 + ts
                        veng = nc.gpsimd if v_eng == 'pool' else nc.vector
                        with nc.allow_low_precision(reason="bf16"):
                            veng.tensor_scalar_mul(
                                vaug[:, st, :, 0:64], pv, 1.0 / WSCALE)
                    # interleave qk/v items: v stores are one DVE op, so a
                    # (qk, v) cadence keeps the shared pg/pv psum ring from
                    # stalling on the 2-op qk store chain
                    if b_order == 'interleave':
                        for j in range(4):
                            yield (lambda s0=s0, xq=xq, cb=j:
                                   qk_item(s0, xq, cb))
                            yield (lambda s0=s0, xq=xq, ts=j:
                                   v_item(s0, xq, ts))
                    else:
                        for cb in range(4):
                            yield (lambda s0=s0, xq=xq, cb=cb:
                                   qk_item(s0, xq, cb))
                        for ts in range(4):
                            yield (lambda s0=s0, xq=xq, ts=ts:
                                   v_item(s0, xq, ts))

            b_gen = b_items()
            b_total = 12 + (NBC - 1) * 8   # chunk 0 split into 12 items
            b_emitted = 0

            def emit_b(n):
                done = 0
                for _ in range(n):
                    item = next(b_gen, None)
                    if item is None:
                        break
                    item()
                    done += 1
                return done

            # ---------------- phase C: band attention ----------------
            from collections import deque
            pending = deque()
            _dq = deque()
            _held = []

            cur_stp = [None]

            def score_unit(h, qt):
                pr, po = h // 2, (h % 2) * 64
                gs = [g for g in range(qt - 2, qt + 3) if 0 <= g < NT]
                nA = len(gs)
                stp = sapool.tile([128, 5, 128], dt.float32, tag="stp",
                                  name="stp")
                cur_stp[0] = stp
                for j in range(nA):
                    g = gs[j]
                    # stationary k read twice via a stride-0 slot dim: the
                    # DR slot sum k.(A+B) = k.8q cancels q's fp8 error
                    kap = qkT[2 + pr][po:po + 64, g * 128:(g + 1) * 128]
                    k2 = bass.AP(tensor=kap.tensor, offset=kap.offset,
                                 ap=[kap.ap[0], [0, 2]] + list(kap.ap[1:]))
                    nc.tensor.matmul(
                        stp[:, j, :], k2,
                        qkT[pr][po:po + 64, :, qt * 128:(qt + 1) * 128],
                        perf_mode=DR)
                ptA = ptpool.tile([128, 5, 128], dt.bfloat16, tag="pt",
                                  name="ptA")
                # psum holds -64*qk; negative scale restores exp(qk/8)
                nc.scalar.activation(ptA[:, 0:nA, :], stp[:, 0:nA, :],
                                     mybir.ActivationFunctionType.Exp,
                                     scale=-1.0 / (64.0 * np.sqrt(HD)))
                lo = gs[0] == qt - 2
                up = gs[-1] == qt + 2
                m_eng = 'dve1' if qt >= tail_q else mask_eng
                with nc.allow_low_precision(reason="bf16"):
                    if m_eng == 'dve1' and lo and up:
                        # both triangles in one strided op (slices 0 and 4)
                        nc.vector.tensor_mul(ptA[:, 0:5:4, :],
                                             ptA[:, 0:5:4, :], masks)
                    else:
                        meng = (nc.vector if m_eng in ('dve', 'dve1')
                                else nc.gpsimd)
                        if lo:
                            meng.tensor_mul(ptA[:, 0, :], ptA[:, 0, :],
                                            masks[:, 0, :])
                        if up:
                            meng.tensor_mul(ptA[:, nA - 1, :],
                                            ptA[:, nA - 1, :], masks[:, 1, :])
                return (gs, nA, ptA)

            cn_ref = [None, None]  # per parity: pending pair ctxn tile
            _tq = deque()          # deferred ctxT transpose: (h, qt, ctxn2)

            def pv_unit(h, qt, gs, nA, ptA):
                pr = h // 2
                ctx = cxpool.tile([128, 65], dt.float32, tag="cx",
                                  name="ctx")
                n = len(gs)
                # masked slices (0 and n-1) go last: their mask ops on the
                # mask engine get the longest lead time
                order = list(range(1, n - 1)) + [n - 1, 0] if n > 2 \
                    else list(range(n))
                for i, j in enumerate(order):
                    nc.tensor.matmul(ctx, ptA[:, j, :], vaug[:, gs[j], h, :],
                                     start=(i == 0), stop=(i == n - 1))
                rec = recpool.tile([128, 1], dt.float32, tag="rec",
                                   name="rec")
                nc.vector.reciprocal(rec, ctx[:, 64:65])
                if h % 2 == 0:
                    cn_ref[pr] = cnpool.tile([128, 2, 64], dt.bfloat16,
                                             tag="cn", name="ctxn2")
                ctxn2 = cn_ref[pr]
                with nc.allow_low_precision(reason="bf16"):
                    nc.vector.tensor_scalar_mul(ctxn2[:, h % 2, :],
                                                ctx[:, 0:64], rec)
                _tq.append((h, qt, ctxn2))

            def tp_unit(h, qt, ctxn2):
                # Transpose a head pair's normalized context in one shot:
                # ctxn2 [128 q, 128 pairdims] -> ctxT [128 pairdims, 128 q].
                if h % 2 == 1:
                    pr = h // 2
                    dst = ctxT[pr][:, qt * 128:(qt + 1) * 128]
                    if tp_mode == 'dma':
                        # XBAR DMA transpose: SBUF->SBUF, off both PE and DVE
                        teng = nc.scalar if tp_eng == 'act' else nc.sync
                        teng.dma_start(out=dst, in_=ctxn2, transpose=True)
                    else:
                        if tp_pool == 'big':
                            tp = bigpool.tile([128, 128], dt.bfloat16,
                                              tag="big", name="tp")
                        else:
                            tp = cxpool.tile([128, 128], dt.bfloat16,
                                             tag="cx", name="tp")
                        nc.tensor.transpose(tp, ctxn2, ident)
                        with nc.allow_low_precision(reason="bf16"):
                            nc.vector.tensor_copy(dst, tp)
                if h == NH_CORE - 1:
                    _dq.append(qt)

            def emit_d(qt, split_dma=False, tail=False):
                osb = opool.tile([128, 1024], dt.bfloat16, tag="osb",
                                 name="osb")
                deng = nc.gpsimd if osb_eng == 'gpsimd' else nc.sync
                for nn in range(2):
                    use_cx = (pd_pool == 'cx' or
                              (pd_pool == 'split' and nn == 0))
                    if use_cx:
                        pD = cxpool.tile([128, 512], dt.float32, tag="cx",
                                         name="pD")
                    else:
                        pD = bigpool.tile([128, 512], dt.float32, tag="big",
                                          name="pD")
                    for p in range(2):
                        nc.tensor.matmul(
                            pD, ctxT[p][:, qt * 128:(qt + 1) * 128],
                            wo[:, p, nn * 512:(nn + 1) * 512],
                            start=(p == 0), stop=(p == 1))
                    dst = osb[:, nn * 512:(nn + 1) * 512]
                    use_act = (nn == 0) if osb_copy != 'swap' else (nn == 1)
                    with nc.allow_low_precision(reason="bf16 partials"):
                        if osb_copy == 'pool':
                            nc.gpsimd.tensor_copy(dst, pD)
                        elif osb_copy == 'dve2' or tail or not use_act:
                            nc.vector.tensor_copy(dst, pD)
                        else:
                            nc.scalar.copy(dst, pD)
                    if split_dma:
                        deng.dma_start(
                            out=OUT[qt * 128:(qt + 1) * 128,
                                    nn * 512:(nn + 1) * 512],
                            in_=osb[:, nn * 512:(nn + 1) * 512])
                if not split_dma:
                    deng.dma_start(out=OUT[qt * 128:(qt + 1) * 128, :],
                                   in_=osb)

            # pacing: unit qt needs qkT/vaug through token (qt+2)*128+128,
            # i.e. chunks 0..ceil((qt*128+384)/512)-1 done.
            b_emitted += emit_b(8 * b_prol)
            nc.scalar.dma_start(out=masks, in_=MASKS[:, :, :])
            nc.scalar.dma_start(out=ident, in_=IDENT[:, :])
            nc.scalar.dma_start(out=wo[:, 0, :], in_=WO[0, :, :])
            nc.scalar.dma_start(out=wo[:, 1, :], in_=WO[1, :, :])
            for qt in range(NQT):
                # scores of qt need chunks covering tokens to (qt+3)*128-1,
                # i.e. chunks 0..(qt+2)//4 done; b_slack items of margin.
                need = min(b_total, 12 + 8 * ((qt + 2) // 4) + b_slack)
                # heads in order (0,2,1,3): staggers the two pair-chains
                for h in (0, 2, 1, 3):
                    dd = d_delay if qt < NQT - 2 else 0
                    while _dq and len(_dq) > dd:
                        dqt = _dq.popleft()
                        # park a few mid-sequence o-proj blocks: they become
                        # dependency-free PE work overlapping the final
                        # attention drain
                        if d_hold and len(_held) < d_hold and 16 <= dqt < 28:
                            _held.append(dqt)
                        else:
                            emit_d(dqt, split_dma=(dqt >= NQT - 2),
                                   tail=(dqt >= tail_d))
                    if d_hold and qt >= d_release and _held:
                        emit_d(_held.pop(0), split_dma=True, tail=True)
                    want = need - b_emitted
                    if want > 0:
                        per = max(1, (want + (NH_CORE - h) - 1)
                                  // (NH_CORE - h))
                        if b_cap:
                            per = min(per, b_cap)
                        b_emitted += emit_b(per)
                    if pv_first == 2 and len(_tq) > tp_delay:
                        tp_unit(*_tq.popleft())
                    if pv_first == 1 and len(pending) >= depth:
                        pv_unit(*pending.popleft())
                        pending.append((h, qt) + score_unit(h, qt))
                    else:
                        pending.append((h, qt) + score_unit(h, qt))
                        if len(pending) > depth:
                            pv_unit(*pending.popleft())
                    if pv_first != 2 and len(_tq) > tp_delay:
                        tp_unit(*_tq.popleft())
            while pending:
                pv_unit(*pending.popleft())
                if len(_tq) > 1:
                    tp_unit(*_tq.popleft())
            while _tq:
                tp_unit(*_tq.popleft())
                while _dq:
                    emit_d(_dq.popleft())
            b_emitted += emit_b(b_total)
            while _dq:
                emit_d(_dq.popleft(), split_dma=True)
            for dqt in _held:
                emit_d(dqt, split_dma=True)

    nc.compile()
    return nc


f8 = ml_dtypes.float8_e4m3


def _split8(a):
    """a (f32) -> (a8, da8) fp8e4 with a ~= a8 + da8 (compensated split)."""
    a8 = a.astype(f8)
    d8 = (a - a8.astype(np.float32)).astype(f8)
    return a8, d8


WSCALE = 128.0  # lifts W (and its residual) out of e4m3's subnormal range


def _pack_w8(wcols, ncol):
    """[1024, ncol] f32 -> [128, 4t, 2s, 2i, ncol] fp8 with contraction
    index c = 256t + 128i + p. Weights are pre-scaled by WSCALE; the
    psum->sbuf copy divides it back out."""
    w8, dw8 = _split8(wcols * WSCALE)
    ws = np.stack([w8, dw8])                     # [s, 1024, ncol]
    ws = ws.reshape(2, 4, 2, 128, ncol)          # [s, t, i, p, col]
    return np.ascontiguousarray(ws.transpose(3, 1, 0, 2, 4))


def _prep_fast(x, Wqkv, Wo):
    """Per-core input maps (compensated fp8 QKV operands, bf16 Wo)."""
    xT_b = []
    for b in range(B):
        xt = np.ascontiguousarray(x[b].T)              # [E, S] f32
        x8, dx8 = _split8(xt)
        xs = np.stack([x8, dx8])                       # [s, E, S]
        xs = xs.reshape(2, 4, 2, 128, 8, 512)          # [s, t, i, p, s0, tok]
        xT_b.append(np.ascontiguousarray(xs.transpose(4, 3, 1, 0, 2, 5)))
        # xT_b[b][s0, p, t, s, i, tok] = xs[s, 256t+128i+p, 512*s0+tok]
    in_maps = []
    for c in range(8):
        b, hg = c // 4, c % 4
        heads = range(4 * hg, 4 * hg + 4)
        qcols = np.concatenate([np.arange(h * 192, h * 192 + 64)
                                for h in heads])
        kcols = qcols + 64
        vcols = qcols + 128
        wqk_cols = np.concatenate([qcols, kcols])           # [512]
        wqk = _pack_w8(Wqkv[:, wqk_cols], 512)
        wv = _pack_w8(Wqkv[:, vcols], 256)
        orows = np.concatenate([np.arange(h * 64, h * 64 + 64)
                                for h in heads])
        wo = np.ascontiguousarray(Wo[orows].reshape(2, 128, 1024)).astype(bf16)
        in_maps.append({"xT": xT_b[b], "wqk": wqk, "wv": wv, "wo": wo})
    return in_maps



def _build_generic(vbias=True, st_bufs=2, po_bufs=1, bc_bufs=1, cx_bufs=2,
           mask_eng='dve', bccopy_eng='act', pt_bufs=8,
           osbcopy_eng='dve', bcast_via='pe', paired=True, depth=1,
           fuse_b=True, pb_bufs=2, b_lead=3, b_prol=2,
           norm_src='sbuf', ctxcopy_eng='act'):
    if fuse_b:
        pt_bufs = min(pt_bufs, 6)
    _nb = 2 if fuse_b else 3
    nc = bacc.Bacc("TRN2", target_bir_lowering=False, debug=False, num_devices=8)

    XT = nc.dram_tensor("xT", [128, 16, 8, 256], dt.float32r, kind="ExternalInput")
    WQK = nc.dram_tensor("wqk", [128, 8, 4, 128], dt.float32r, kind="ExternalInput")
    WV = nc.dram_tensor("wv", [128, 8, 256], dt.float32r, kind="ExternalInput")
    WO = nc.dram_tensor("wo", [2, 128, 1024], dt.float32r, kind="ExternalInput")
    BQK = nc.dram_tensor("bqk", [128, 4], dt.float32, kind="ExternalInput")
    BV = nc.dram_tensor("bv", [1, 256], dt.float32, kind="ExternalInput")
    MV8 = nc.dram_tensor("mv8", [128, 32], dt.float32, kind="ExternalInput")
    OUT = nc.dram_tensor("out", [S, E], dt.float32, kind="ExternalOutput")

    # constant 0/1 triangular band masks for u in {-2,-1,2,3}
    p_i = np.arange(128)[:, None]
    r_i = np.arange(256)[None, :]
    mask_np = {}
    for u in (-2, -1, 2, 3):
        mask_np[u] = ((u * 128 + p_i - r_i >= -w) & (u * 128 + p_i - r_i <= w)
                      ).astype(np.float32)
    MASKS = nc.inline_tensor(
        np.ascontiguousarray(
            np.stack([mask_np[u] for u in (-2, -1, 2, 3)]).transpose(1, 0, 2)),
        name="trimasks")
    ONES = nc.inline_tensor(np.ones((1, 128), dtype=np.float32), name="onesrow")

    with tile.TileContext(nc) as tc:
        with tc.tile_pool(name="const", bufs=1) as cpool, \
             tc.tile_pool(name="qkT", bufs=1) as qkpool, \
             tc.tile_pool(name="vaug", bufs=1) as vpool, \
             tc.tile_pool(name="ctxT", bufs=1) as ctxpool:

            wo = [cpool.tile([128, 1024], dt.float32r, name=f"wo{p}") for p in range(2)]
            bqk = cpool.tile([128, 4], dt.float32)
            nc.gpsimd.dma_start(out=bqk, in_=BQK[:, :])
            bv_f = cpool.tile([1, 256], dt.float32)
            nc.gpsimd.dma_start(out=bv_f, in_=BV[:, :])
            mv8 = cpool.tile([128, 32], dt.float32)
            nc.gpsimd.dma_start(out=mv8, in_=MV8[:, :])
            masks = cpool.tile([128, 4, 256], dt.float32)
            mask_idx = {-2: 0, -1: 1, 2: 2, 3: 3}
            ones_f = cpool.tile([1, 128], dt.float32)
            nc.gpsimd.dma_start(out=ones_f, in_=ONES[:, :])
            ones_r = cpool.tile([1, 128], dt.float32r)
            bv_r = cpool.tile([1, 256], dt.float32r)
            with nc.allow_low_precision(reason="f32r matmul pipeline"):
                nc.vector.tensor_copy(ones_r, ones_f)
                nc.vector.tensor_copy(bv_r, bv_f)

            # persistent intermediates
            qkT = [qkpool.tile([128, S], dt.float32r, name=f"qkT{cb}")
                   for cb in range(4)]  # 0,1: q pairs; 2,3: k pairs
            vaug = [vpool.tile([128, NT, 65], dt.float32r, name=f"vaug{h}")
                    for h in range(NH_CORE)]
            ones32 = cpool.tile([128, NT], dt.float32)
            nc.vector.memset(ones32, 1.0)
            for h in range(NH_CORE):
                with nc.allow_low_precision(reason="f32r"):
                    nc.vector.tensor_copy(vaug[h][:, :, 64], ones32)
            ctxT = [ctxpool.tile([128, S], dt.float32r, name=f"ctxT{p}")
                    for p in range(2)]

            # ---------------- Phase B: QKV projection ----------------
            # Emitted either up front (fuse_b=False) or as fine-grained work
            # items interleaved into the attention loop's idle PE slots.
            bwpool = ctx_pools = None
            import contextlib
            _bstack = contextlib.ExitStack()
            bwpool = _bstack.enter_context(tc.tile_pool(name="bw", bufs=1))
            xqpool = _bstack.enter_context(
                tc.tile_pool(name="xq", bufs=(2 if fuse_b else 3)))
            pbpool = _bstack.enter_context(
                tc.tile_pool(name="pb", bufs=(pb_bufs if fuse_b else 8),
                             space="PSUM"))
            wqk = bwpool.tile([128, 8, 4, 128], dt.float32r)
            wv = bwpool.tile([128, 8, 256], dt.float32r)
            xq0 = [xqpool.tile([128, 4, 256], dt.float32r, tag=f"xq{i}",
                               name="xq") for i in range(2)]
            for i in range(2):
                nc.sync.dma_start(out=xq0[i], in_=XT[:, 0, i * 4:(i + 1) * 4, :])
            for kt in range(8):
                nc.sync.dma_start(out=wqk[:, kt, :, :], in_=WQK[:, kt, :, :])
            nc.sync.dma_start(out=wv[:, 0:4, :], in_=WV[:, 0:4, :])
            nc.sync.dma_start(out=wv[:, 4:8, :], in_=WV[:, 4:8, :])

            def b_items():
                for s0 in range(16):  # 256-token chunks of S
                    if s0 == 0:
                        xq = xq0
                    else:
                        xq = [xqpool.tile([128, 4, 256], dt.float32r,
                                          tag=f"xq{i}", name="xq")
                              for i in range(2)]
                        for i in range(2):
                            nc.sync.dma_start(
                                out=xq[i], in_=XT[:, s0, i * 4:(i + 1) * 4, :])

                    def qk_item(s0=s0, xq=xq, cb=0):
                        pg = pbpool.tile([128, 256], dt.float32, tag="pb",
                                         name="pqk")
                        for k8 in range(8):
                            nc.tensor.matmul(pg, wqk[:, k8, cb, :],
                                             xq[k8 // 4][:, k8 % 4, :],
                                             start=(k8 == 0), stop=(k8 == 7))
                        nc.scalar.activation(
                            qkT[cb][:, s0 * 256:(s0 + 1) * 256], pg,
                            mybir.ActivationFunctionType.Identity,
                            bias=bqk[:, cb:cb + 1])
                    for cb in range(4):
                        yield (lambda s0=s0, xq=xq, cb=cb: qk_item(s0, xq, cb))

                    def v_item(s0=s0, xq=xq, hf=0):
                        pv = pbpool.tile([128, 256], dt.float32, tag="pb",
                                         name="pv")
                        for k8 in range(8):
                            nc.tensor.matmul(
                                pv,
                                xq[k8 // 4][:, k8 % 4, hf * 128:(hf + 1) * 128],
                                wv[:, k8, :], start=(k8 == 0),
                                stop=(k8 == 7 and not vbias))
                        if vbias:
                            nc.tensor.matmul(pv, ones_r, bv_r,
                                             start=False, stop=True)
                        st = s0 * 2 + hf
                        for h in range(NH_CORE):
                            with nc.allow_low_precision(reason="f32r"):
                                nc.vector.tensor_copy(
                                    vaug[h][:, st, 0:64],
                                    pv[:, h * 64:(h + 1) * 64])
                    for hf in range(2):
                        yield (lambda s0=s0, xq=xq, hf=hf: v_item(s0, xq, hf))

            b_gen = b_items()
            b_total = 16 * 6
            b_emitted = 0

            def emit_b(n):
                emitted = 0
                for _ in range(n):
                    item = next(b_gen, None)
                    if item is None:
                        break
                    item()
                    emitted += 1
                return emitted

            if not fuse_b:
                b_emitted += emit_b(b_total)
                _bstack.close()

            nc.gpsimd.dma_start(out=masks, in_=MASKS[:, :, :])
            for p in range(2):
                nc.gpsimd.dma_start(out=wo[p], in_=WO[p, :, :])
            # ------- Phase C: band attention, with output projection folded in -------
            import contextlib
            _cstack = contextlib.ExitStack()
            with _cstack:
                stpool = _cstack.enter_context(
                    tc.tile_pool(name="stp", bufs=st_bufs, space="PSUM"))
                cxpool = _cstack.enter_context(
                    tc.tile_pool(name="ctxp", bufs=cx_bufs, space="PSUM"))
                if bcast_via == 'pe':
                    bcpool = _cstack.enter_context(
                        tc.tile_pool(name="bcp", bufs=bc_bufs, space="PSUM"))
                else:
                    drpool = _cstack.enter_context(
                        tc.tile_pool(name="dr", bufs=4, space="DRAM"))
                popool = _cstack.enter_context(
                    tc.tile_pool(name="po", bufs=po_bufs, space="PSUM"))
                ptpool = _cstack.enter_context(
                    tc.tile_pool(name="pt", bufs=pt_bufs))
                bcsb = _cstack.enter_context(tc.tile_pool(name="bcs", bufs=_nb))
                opool = _cstack.enter_context(tc.tile_pool(name="osb", bufs=2))
                rcpool = _cstack.enter_context(tc.tile_pool(name="rcp", bufs=_nb))

                def score_stage(h, cc):
                    # returns list of (gts, pt, jslices) where pt holds exp'd
                    # probabilities for the key tiles in gts
                    pr, po = h // 2, (h % 2) * 64
                    out = []
                    if paired:
                        # all-ones padding: exp has no per-key bias, so key
                        # tiles are processed in aligned pairs (one psum bank,
                        # one exp, one mask-mul per pair)
                        for ub in (-2, 0, 2):
                            gts = [2 * cc + ub, 2 * cc + ub + 1]
                            if gts[0] < 0 or gts[1] >= NT:
                                continue
                            stp = stpool.tile([128, 2, 256], dt.float32,
                                              tag="st", name="stp")
                            for j, gt in enumerate(gts):
                                nc.tensor.matmul(
                                    stp[:, j, :],
                                    qkT[2 + pr][po:po + 64,
                                                gt * 128:(gt + 1) * 128],
                                    qkT[pr][po:po + 64,
                                            cc * 256:(cc + 1) * 256])
                            pt = ptpool.tile([128, 2, 256], dt.float32r,
                                             tag="pt", name="pt")
                            nc.scalar.activation(
                                pt, stp, mybir.ActivationFunctionType.Exp,
                                scale=1.0 / np.sqrt(HD))
                            if ub != 0:
                                mi = 0 if ub == -2 else 2
                                with nc.allow_low_precision(reason="f32r"):
                                    eng = (nc.gpsimd if mask_eng == 'gpsimd'
                                           else nc.vector)
                                    eng.tensor_mul(pt, pt,
                                                   masks[:, mi:mi + 2, :])
                            out.append((gts, pt))
                        return out
                    for u in range(-2, 4):
                        gt = 2 * cc + u
                        if not 0 <= gt < NT:
                            continue
                        stp = stpool.tile([128, 256], dt.float32, tag="st",
                                          name="stp")
                        nc.tensor.matmul(
                            stp,
                            qkT[2 + pr][po:po + 64, gt * 128:(gt + 1) * 128],
                            qkT[pr][po:po + 64, cc * 256:(cc + 1) * 256])
                        pt = ptpool.tile([128, 256], dt.float32r, tag="pt",
                                         name="pt")
                        nc.scalar.activation(pt, stp,
                                             mybir.ActivationFunctionType.Exp,
                                             bias=mv8[:, gt:gt + 1],
                                             scale=1.0 / np.sqrt(HD))
                        if u in mask_idx:
                            with nc.allow_low_precision(reason="f32r"):
                                eng = (nc.gpsimd if mask_eng == 'gpsimd'
                                       else nc.vector)
                                eng.tensor_mul(pt, pt,
                                               masks[:, mask_idx[u], :])
                        out.append(([gt], pt))
                    return out

                def pv_stage(h, cc, pts):
                    if _dq:
                        emit_d(_dq.popleft())
                    pr, po = h // 2, (h % 2) * 64
                    ctx = cxpool.tile([65, 256], dt.float32, tag="cx",
                                      name="ctx")
                    nmm = sum(len(gts) for gts, _ in pts)
                    j = 0
                    for gts, pt in pts:
                        for jj, gt in enumerate(gts):
                            rhs = pt[:, jj, :] if len(gts) > 1 else pt
                            nc.tensor.matmul(ctx, vaug[h][:, gt, :], rhs,
                                             start=(j == 0),
                                             stop=(j == nmm - 1))
                            j += 1
                    if norm_src == 'sbuf':
                        # copy ctx out of PSUM first: frees the cx slot early
                        # and the final multiply reads bc straight from PSUM
                        cxs = bcsb.tile([65, 256], dt.float32, tag="bcs",
                                        name="cxs")
                        if ctxcopy_eng == 'act':
                            nc.scalar.copy(cxs, ctx)
                        else:
                            nc.vector.tensor_copy(cxs, ctx)
                        ctx = cxs
                    rec = rcpool.tile([1, 256], dt.float32r, tag="rc",
                                      name="rec")
                    with nc.allow_low_precision(reason="f32r"):
                        nc.vector.reciprocal(rec, ctx[64:65, :])
                    bcs = None
                    if norm_src != 'sbuf':
                        bcs = bcsb.tile([64, 256], dt.float32, tag="bcs",
                                        name="bcs")
                    if bcast_via == 'dma':
                        drec = drpool.tile([1, 256], dt.float32r, tag="dr",
                                           name="drec")
                        nc.sync.dma_start(out=drec, in_=rec)
                        dbc = bass.AP(tensor=drec.tensor, offset=drec.offset,
                                      ap=[[0, 64]] + drec.ap[1:])
                        nc.sync.dma_start(out=bcs.bitcast(dt.float32r), in_=dbc)
                    else:
                        bc = bcpool.tile([64, 256], dt.float32, tag="bc",
                                         name="bc")
                        nc.tensor.matmul(bc, ones_r[:, 0:64], rec)
                        if norm_src == 'sbuf':
                            bcs = bc
                        elif bccopy_eng == 'act':
                            nc.scalar.copy(bcs, bc)
                        else:
                            nc.vector.tensor_copy(bcs, bc)
                    with nc.allow_low_precision(reason="f32r"):
                        nc.vector.tensor_mul(
                            ctxT[pr][po:po + 64, cc * 256:(cc + 1) * 256],
                            ctx[0:64, :], bcs)
                    if h == NH_CORE - 1:
                        _dq.append(2 * cc)
                        _dq.append(2 * cc + 1)

                def emit_d(qt):
                    osb = opool.tile([128, 1024], dt.float32, tag="osb",
                                     name="osb")
                    for nn in range(2):
                        pD = popool.tile([128, 512], dt.float32, tag="po",
                                         name="pD")
                        for p in range(2):
                            nc.tensor.matmul(pD,
                                             ctxT[p][:, qt * 128:(qt + 1) * 128],
                                             wo[p][:, nn * 512:(nn + 1) * 512],
                                             start=(p == 0), stop=(p == 1))
                        if osbcopy_eng == 'act':
                            nc.scalar.copy(osb[:, nn * 512:(nn + 1) * 512], pD)
                        else:
                            nc.vector.tensor_copy(osb[:, nn * 512:(nn + 1) * 512], pD)
                    nc.gpsimd.dma_start(out=OUT[qt * 128:(qt + 1) * 128, :],
                                        in_=osb)

                from collections import deque
                pending = deque()
                _dq = deque()
                if fuse_b:
                    # prologue: cover key tiles for the first two query chunks
                    b_emitted += emit_b(6 * b_prol)
                step = 0
                for cc in range(NCC):
                    for h in range(NH_CORE):
                        if fuse_b:
                            # pace remaining B so chunk cc+2 is done before
                            # attention chunk cc+1 starts
                            target = min(b_total, 6 * (cc + b_lead))
                            want = target - b_emitted
                            per = max(1, (want + (NH_CORE - h) - 1)
                                      // (NH_CORE - h))
                            if want > 0:
                                b_emitted += emit_b(per)
                        pts = score_stage(h, cc)
                        pending.append((h, cc, pts))
                        if len(pending) > depth:
                            pv_stage(*pending.popleft())
                        step += 1
                while pending:
                    pv_stage(*pending.popleft())
                while _dq:
                    emit_d(_dq.popleft())
                if fuse_b:
                    b_emitted += emit_b(b_total)

            _bstack.close()

    nc.compile()
    return nc



def _prep_generic(x, Wqkv, bqkv, Wo, pm):
    in_maps = []
    xT_b = []
    for b in range(B):
        xt = np.ascontiguousarray(x[b].T)                      # [E, S]
        xT_b.append(np.ascontiguousarray(
            xt.reshape(8, 128, 16, 256).transpose(1, 2, 0, 3)))
    mv8_b = []
    for b in range(B):
        # mv8[p, t] = (0 if valid else NEG)/8 for key index t*128+p
        mv = np.where(pm[b], 0.0, NEG).astype(np.float32) / 8.0
        mv8_b.append(np.ascontiguousarray(mv.reshape(32, 128).T))

    for c in range(8):
        b, hg = c // 4, c % 4
        heads = range(4 * hg, 4 * hg + 4)
        qcols = np.concatenate([np.arange(h * 192, h * 192 + 64) for h in heads])
        kcols = qcols + 64
        vcols = qcols + 128
        wqk_cols = np.concatenate([qcols, kcols])               # [512]
        wqk = np.ascontiguousarray(
            Wqkv[:, wqk_cols].reshape(8, 128, 4, 128).transpose(1, 0, 2, 3))
        wv = np.ascontiguousarray(
            Wqkv[:, vcols].reshape(8, 128, 256).transpose(1, 0, 2))
        orows = np.concatenate([np.arange(h * 64, h * 64 + 64) for h in heads])
        wo = np.ascontiguousarray(Wo[orows].reshape(2, 128, 1024))
        in_maps.append({
            "xT": xT_b[b],
            "wqk": wqk,
            "wv": wv,
            "wo": wo,
            "bqk": np.ascontiguousarray(bqkv[wqk_cols].reshape(4, 128).T),
            "bv": np.ascontiguousarray(bqkv[vcols].reshape(1, 256)),
            "mv8": mv8_b[b],
        })
    return in_maps


def kernel(x, Wqkv, bqkv, Wo, bo, padding_mask, num_heads, window_size):
    assert int(num_heads) == H and int(window_size) == W
    x = np.asarray(x, dtype=np.float32)
    Wqkv = np.asarray(Wqkv, dtype=np.float32)
    bqkv = np.asarray(bqkv, dtype=np.float32)
    Wo = np.asarray(Wo, dtype=np.float32)
    bo = np.asarray(bo, dtype=np.float32)
    pm = np.asarray(padding_mask).astype(bool)
    assert x.shape == (B, S, E)

    fast = bool(pm.all()) and not np.any(bqkv)
    if fast:
        if "fast" not in _cache:
            _cache["fast"] = _build_fast(depth=3, tp_delay=1, d_delay=2,
                                         warm_n=8, osb_eng='sync',
                                         pd_pool='split', xq_eng='sync',
                                         mask_eng='gpsimd', b_slack=1,
                                         b_prol=1, pv_first=2, tail_q=26,
                                         q_via='bf16')
        nc = _cache["fast"]
        in_maps = _prep_fast(x, Wqkv, Wo)
    else:
        vbias = bool(np.any(bqkv.reshape(H, 3, HD)[:, 2, :] != 0.0))
        key = ("nc", vbias, False)
        if key not in _cache:
            _cache[key] = _build_generic(vbias=vbias, paired=False)
        nc = _cache[key]
        in_maps = _prep_generic(x, Wqkv, bqkv, Wo, pm)

    res = run_bass_kernel_spmd(nc, in_maps, list(range(8)))
    kernel._last_results = res

    out = np.empty((B, S, E), dtype=np.float32)
    for b in range(B):
        acc = res.results[4 * b]["out"].astype(np.float32)
        for g in range(1, 4):
            acc = acc + res.results[4 * b + g]["out"].astype(np.float32)
        out[b] = acc + bo
    return out

